# revision 1
# baseline (speedup 1.0000x reference)
"""Causal self-attention (B=1, T=4096, C=768, H=12) on 8 TRN2 NeuronCores.

Strategy (single SPMD NEFF, no collectives):
  - Sequence-parallel over queries: core c owns q-tiles {c, c+8, c+16, c+24}
    (128 rows each, descending-extent column order). Slot s of every core
    processes key-blocks 0..8(s+1)-1 (uniform instruction stream across
    cores); the true causal boundary is enforced by a tiny per-core binary
    mask library passed as input data, so ONE program serves all 8 cores.
  - K/V projection is computed replicated on every core (an on-chip AllGather
    of the 12.6 MB K/V at ~62 GB/s bus would cost ~180+ us - slower than the
    ~124 us of redundant PE work, which overlaps the ACT-bound softmax).
  - The kernel is a single fused pipeline: each "wave" projects K^T/V for two
    512-row key chunks, then runs attention for those 8 key-blocks across all
    12 heads; PV partials accumulate in an SBUF fp32 accumulator (freeing
    PSUM banks: 2 proj + 4 S^T + 2 PV = 8).
  - Everything stays "transposed": S^T = K @ Q^T puts keys on partitions, exp
    runs PSUM->SBUF on ScalarE (no max-subtraction needed: |S|/8 <= ~8), and
    P^T feeds the PV matmul as the moving operand - zero transposes anywhere.
    The softmax denominator falls out of a 65th all-ones column appended to V.
  - QK matmuls (contraction d=64) are packed two-heads-per-pass into the
    128x128 PE array via partition-offset row tiling (tile_position).
  - bf16 operands / fp32 PSUM accumulation; output fp32. Measured end-to-end
    relative error vs the fp32 reference: 4.6e-3.
"""

from dataclasses import dataclass

import ml_dtypes
import numpy as np

import concourse.bass as bass
import concourse.mybir as mybir
import concourse.tile as tile
from concourse import bacc
from concourse.bass_utils import run_bass_kernel_spmd

BF16 = mybir.dt.bfloat16
F32 = mybir.dt.float32
NPBF16 = ml_dtypes.bfloat16


@dataclass(frozen=True)
class Cfg:
    T: int = 4096
    H: int = 12
    D: int = 64
    ncores: int = 8

    @property
    def C(self):
        return self.H * self.D

    @property
    def HP(self):  # head pairs
        return self.H // 2

    @property
    def NKB(self):  # 128-row key blocks
        return self.T // 128

    @property
    def QTC(self):  # q-tiles per core
        return self.T // 128 // self.ncores

    @property
    def QW(self):  # q columns per core
        return 128 * self.QTC

    @property
    def NCT(self):  # 128-row contraction tiles over C
        return self.C // 128

    def nb(self, b):  # valid q-column prefix width for key-block b
        return 128 * (self.QTC - b // self.ncores)

    def qtiles(self, c):  # global q-tile indices for core c, descending extent
        return [c + self.ncores * (self.QTC - 1 - g) for g in range(self.QTC)]


CFG = Cfg()


def _exp_batches(cfg):
    """Pack key-blocks into 2-PSUM-bank (1024 fp32 col) exp batches.

    Returns a list of batches; each batch is a list of (b, col_offset) with
    every block's [col_offset, col_offset+nb(b)) range inside a single
    512-col bank. One ScalarE exp call covers each batch.
    """
    batches, cur = [], []
    bank, off = 0, 0
    for b in range(cfg.NKB):
        n = cfg.nb(b)
        if off + n > 512:
            bank += 1
            off = 0
            if bank == 2:
                batches.append(cur)
                cur = []
                bank = 0
        cur.append((b, bank * 512 + off))
        off += n
    if cur:
        batches.append(cur)
    return batches


def build_kernel(tc, outs, ins, cfg=CFG, phases=(1, 2, 3)):
    nc = tc.nc
    C, H, HP, NCT = cfg.C, cfg.H, cfg.HP, cfg.NCT
    NKB, QW = cfg.NKB, cfg.QW
    Exp = mybir.ActivationFunctionType.Exp
    scale = 1.0 / np.sqrt(cfg.D)

    xT, xTq = ins["xT"], ins["xTq"]
    wA, wP = ins["wA"], ins["wP"]
    bA, bP = ins["bA"], ins["bP"]
    maskq = ins["maskq"]
    y = outs["y"]

    import contextlib

    stack = contextlib.ExitStack()
    with stack:
        persist = stack.enter_context(tc.tile_pool(name="persist", bufs=1))

        # ---- persistent SBUF tensors -------------------------------------
        kt_t = persist.tile([128, HP, cfg.T], BF16, name="kt_t")
        vaug = persist.tile([128, NKB, 65 * H], BF16, name="vaug")
        qt_t = persist.tile([128, HP, QW], BF16, name="qt_t")
        ytf = persist.tile([128, HP, QW], BF16, name="ytf")
        mask_sb = persist.tile([128, cfg.ncores * 128], BF16, name="mask_sb")
        wp_sb = persist.tile([128, NCT, C], BF16, name="wp_sb")
        xq_sb = persist.tile([128, NCT, QW], BF16, name="xq_sb")
        bq_sb = persist.tile([128, HP], F32, name="bq_sb")
        bk_sb = persist.tile([128, HP], F32, name="bk_sb")
        bv_bc = persist.tile([128, C], F32, name="bv_bc")
        bp_bc = persist.tile([128, C], F32, name="bp_bc")
        ones11 = persist.tile([1, 64], F32, name="ones11")

        nc.sync.dma_start(out=mask_sb, in_=maskq)
        for ct in range(NCT):
            nc.sync.dma_start(out=wp_sb[:, ct, :], in_=wP[128 * ct : 128 * (ct + 1), :])
            nc.sync.dma_start(out=xq_sb[:, ct, :], in_=xTq[128 * ct : 128 * (ct + 1), :])
        for hp in range(HP):
            nc.sync.dma_start(
                out=bq_sb[:, hp : hp + 1], in_=bA[128 * hp : 128 * (hp + 1)]
            )
            nc.sync.dma_start(
                out=bk_sb[:, hp : hp + 1], in_=bA[C + 128 * hp : C + 128 * (hp + 1)]
            )
        # broadcast-DMA a [C] vector across all 128 partitions
        bv_src = bass.AP(tensor=bA.tensor, offset=bA.offset + 2 * C, ap=[[0, 128], [1, C]])
        nc.gpsimd.dma_start(out=bv_bc, in_=bv_src)
        bp_src = bass.AP(tensor=bP.tensor, offset=bP.offset, ap=[[0, 128], [1, C]])
        nc.gpsimd.dma_start(out=bp_bc, in_=bp_src)
        nc.vector.memset(ones11, 1.0)
        # ones columns of [V | 1]
        vaug4 = vaug.rearrange("p b (h e) -> p b h e", e=65)
        nc.vector.memset(vaug4[:, :, :, 64:65], 1.0)

        # ---- phase 1: projections ---------------------------------------
        if 1 in phases:
          with (
            tc.tile_pool(name="wpool", bufs=1) as wpool,
            tc.tile_pool(name="xpool", bufs=2) as xpool,
            tc.tile_pool(name="pp", bufs=2, space="PSUM") as pp,
          ):
            w_sb = wpool.tile([128, NCT, 3 * C], BF16, name="w_sb")
            for ct in range(NCT):
                nc.sync.dma_start(
                    out=w_sb[:, ct, :], in_=wA[128 * ct : 128 * (ct + 1), :]
                )

            # Q^T: [C, QW] from the core's own (strided) q columns
            for hp in range(HP):
                ps_q = pp.tile([128, QW], F32, name="ps_q", tag="ps_q")
                for ct in range(NCT):
                    nc.tensor.matmul(
                        ps_q,
                        w_sb[:, ct, 128 * hp : 128 * (hp + 1)],
                        xq_sb[:, ct, :],
                        start=(ct == 0),
                        stop=(ct == NCT - 1),
                    )
                nc.vector.tensor_scalar_add(
                    qt_t[:, hp, :], ps_q, bq_sb[:, hp : hp + 1]
                )

            # K^T and V over all T rows (replicated across cores)
            for ch in range(cfg.T // 512):
                xch = xpool.tile([128, NCT, 512], BF16, name="xch", tag="xch")
                for ct in range(NCT):
                    nc.sync.dma_start(
                        out=xch[:, ct, :],
                        in_=xT[128 * ct : 128 * (ct + 1), 512 * ch : 512 * (ch + 1)],
                    )
                for hp in range(HP):
                    ps_k = pp.tile([128, 512], F32, name="ps_k", tag="ps_k")
                    for ct in range(NCT):
                        nc.tensor.matmul(
                            ps_k,
                            w_sb[:, ct, C + 128 * hp : C + 128 * (hp + 1)],
                            xch[:, ct, :],
                            start=(ct == 0),
                            stop=(ct == NCT - 1),
                        )
                    nc.vector.tensor_scalar_add(
                        kt_t[:, hp, 512 * ch : 512 * (ch + 1)],
                        ps_k,
                        bk_sb[:, hp : hp + 1],
                    )
                for tt in range(4):
                    ps_v = pp.tile([128, C], F32, name="ps_v", tag="ps_v")
                    for n0, n1 in ((0, 512), (512, C)) if C > 512 else ((0, C),):
                        for ct in range(NCT):
                            nc.tensor.matmul(
                                ps_v[:, n0:n1],
                                xch[:, ct, 128 * tt : 128 * (tt + 1)],
                                w_sb[:, ct, 2 * C + n0 : 2 * C + n1],
                                start=(ct == 0),
                                stop=(ct == NCT - 1),
                            )
                    b_abs = 4 * ch + tt
                    nc.vector.tensor_add(
                        vaug4[:, b_abs, :, 0:64],
                        ps_v.rearrange("p (h e) -> p h e", e=64),
                        bv_bc.rearrange("p (h e) -> p h e", e=64),
                    )

        # ---- phase 2: attention ------------------------------------------
        batches = _exp_batches(cfg)
        if 2 in phases:
          with (
            tc.tile_pool(name="aps", bufs=1, space="PSUM") as aps,
            tc.tile_pool(name="ptp", bufs=2) as ptp,
            tc.tile_pool(name="nrm", bufs=2) as nrm,
          ):
            qproj = tc.alloc_tile_pool(name="qproj", bufs=1)
            s_ps = [aps.tile([128, 1024], F32, name=f"s_ps{h}") for h in range(2)]
            yt_ps = [aps.tile([128, QW], F32, name=f"yt_ps{h}") for h in range(2)]
            rc_ps = aps.tile([64, QW], F32, name="rc_ps")
            for h in range(2):
                nc.vector.memset(s_ps[h], 0.0)

            for hp in range(HP):
                for bat in batches:
                    width = max(co + cfg.nb(b) for b, co in bat)
                    for h in range(2):
                        pt = ptp.tile([128, 1024], BF16, name=f"pt{h}", tag=f"pt{h}")
                        for b, co in bat:
                            n = cfg.nb(b)
                            nc.tensor.matmul(
                                s_ps[h][:, co : co + n],
                                kt_t[64 * h : 64 * (h + 1), hp, 128 * b : 128 * (b + 1)],
                                qt_t[64 * h : 64 * (h + 1), hp, 0:n],
                                start=True,
                                stop=True,
                            )
                        nc.scalar.activation(
                            pt[:, 0:width], s_ps[h][:, 0:width], Exp, scale=scale
                        )
                        for b, co in bat:
                            n = cfg.nb(b)
                            r = b % cfg.ncores
                            nc.vector.tensor_mul(
                                pt[:, co + n - 128 : co + n],
                                pt[:, co + n - 128 : co + n],
                                mask_sb[:, 128 * r : 128 * (r + 1)],
                            )
                        hd = 2 * hp + h
                        for b, co in bat:
                            n = cfg.nb(b)
                            nc.tensor.matmul(
                                yt_ps[h][0:65, 0:n],
                                vaug[:, b, 65 * hd : 65 * (hd + 1)],
                                pt[:, co : co + n],
                                start=(b == 0),
                                stop=(b == NKB - 1),
                            )
                # normalize: Y^T[d, q] * (1 / l[q])
                for h in range(2):
                    rec = nrm.tile([1, QW], F32, name="rec", tag="rec")
                    rbc = nrm.tile([64, QW], F32, name="rbc", tag="rbc")
                    tmp = nrm.tile([64, QW], BF16, name="tmpn", tag="tmpn")
                    nc.vector.reciprocal(rec, yt_ps[h][64:65, 0:QW])
                    nc.tensor.matmul(
                        rc_ps[0:64, :], ones11[0:1, :], rec, start=True, stop=True
                    )
                    nc.vector.tensor_copy(rbc, rc_ps[0:64, :])
                    if h == 0:
                        nc.vector.tensor_mul(
                            ytf[0:64, hp, :], yt_ps[h][0:64, 0:QW], rbc
                        )
                    else:
                        nc.vector.tensor_mul(tmp, yt_ps[h][0:64, 0:QW], rbc)
                        nc.sync.dma_start(out=ytf[64:128, hp, :], in_=tmp)

        # ---- phase 3: output projection ----------------------------------
        if 3 in phases:
          with (
            tc.tile_pool(name="ops", bufs=2, space="PSUM") as ops,
            tc.tile_pool(name="osb", bufs=2) as osb,
          ):
            for g in range(cfg.QTC):
                ps_o = ops.tile([128, C], F32, name="ps_o", tag="ps_o")
                for n0, n1 in ((0, 512), (512, C)) if C > 512 else ((0, C),):
                    for hp in range(HP):
                        nc.tensor.matmul(
                            ps_o[:, n0:n1],
                            ytf[:, hp, 128 * g : 128 * (g + 1)],
                            wp_sb[:, hp, n0:n1],
                            start=(hp == 0),
                            stop=(hp == HP - 1),
                        )
                yo = osb.tile([128, C], F32, name="yo", tag="yo")
                nc.vector.tensor_add(yo, ps_o, bp_bc)
                nc.sync.dma_start(out=y[128 * g : 128 * (g + 1), :], in_=yo)


def build_kernel_fused(tc, outs, ins, cfg=CFG, cpw=2):
    """Fused builder: K/V projection is interleaved chunk-by-chunk with
    attention for ALL head pairs (PV partials accumulate in SBUF, freeing
    PSUM so the PE-heavy projection hides under the ACT-bound softmax)."""
    nc = tc.nc
    C, H, HP, NCT = cfg.C, cfg.H, cfg.HP, cfg.NCT
    NKB, QW = cfg.NKB, cfg.QW
    NCH = cfg.T // 512
    Exp = mybir.ActivationFunctionType.Exp
    scale = 1.0 / np.sqrt(cfg.D)

    xT, xTq = ins["xT"], ins["xTq"]
    wA, wP = ins["wA"], ins["wP"]
    bA, bP = ins["bA"], ins["bP"]
    maskq = ins["maskq"]
    y = outs["y"]

    import contextlib

    stack = contextlib.ExitStack()
    with stack:
        persist = stack.enter_context(tc.tile_pool(name="persist", bufs=1))

        kt_t = persist.tile([128, HP, cfg.T], BF16, name="kt_t")
        vaug = persist.tile([128, NKB, 65 * H], BF16, name="vaug")
        qt_t = persist.tile([128, HP, QW], BF16, name="qt_t")
        ytf = persist.tile([128, HP, QW], BF16, name="ytf")
        yacc = persist.tile([128, H, QW], F32, name="yacc")  # rows 0:65 used
        mask_sb = persist.tile([128, cfg.ncores * 128], BF16, name="mask_sb")
        wp_sb = persist.tile([128, NCT, C], BF16, name="wp_sb")
        w_sb = persist.tile([128, NCT, 2 * C], BF16, name="w_sb")
        bq_sb = persist.tile([128, HP], F32, name="bq_sb")
        bk_sb = persist.tile([128, HP], F32, name="bk_sb")
        bv_bc = persist.tile([128, C], F32, name="bv_bc")
        bp_bc = persist.tile([128, C], F32, name="bp_bc")
        ones11 = persist.tile([1, 64], F32, name="ones11")

        for hp in range(HP):
            nc.sync.dma_start(
                out=bq_sb[:, hp : hp + 1], in_=bA[128 * hp : 128 * (hp + 1)]
            )
            nc.sync.dma_start(
                out=bk_sb[:, hp : hp + 1], in_=bA[C + 128 * hp : C + 128 * (hp + 1)]
            )
        nc.vector.memset(ones11, 1.0)
        # touch Exp early so the ACT table set loads during startup DMAs
        nc.scalar.activation(ones11, ones11, mybir.ActivationFunctionType.Exp,
                             scale=0.0)
        nc.vector.memset(ones11, 1.0)
        vaug4 = vaug.rearrange("p b (h e) -> p b h e", e=65)
        nc.vector.memset(vaug4[:, :, :, 64:65], 1.0)

        with (
            tc.tile_pool(name="xpool", bufs=2) as xpool,
            tc.tile_pool(name="pkv", bufs=2, space="PSUM") as pkv,
            tc.tile_pool(name="aps", bufs=1, space="PSUM") as aps,
            tc.tile_pool(name="pvp", bufs=2, space="PSUM") as pvp,
            tc.tile_pool(name="ptp", bufs=2) as ptp,
            tc.tile_pool(name="nrm", bufs=1) as nrm,
        ):
            qproj = tc.alloc_tile_pool(name="qproj", bufs=1)
            s_ps = [
                [aps.tile([128, 512], F32, name=f"s_ps{h}{i}") for i in range(2)]
                for h in range(2)
            ]
            for h in range(2):
                for i in range(2):
                    nc.vector.memset(s_ps[h][i], 0.0)

            def load_xch(ch):
                t = xpool.tile([128, NCT, 512], BF16, name="xch", tag="xch")
                for ct in range(NCT):
                    nc.sync.dma_start(
                        out=t[:, ct, :],
                        in_=xT[128 * ct : 128 * (ct + 1),
                               512 * ch : 512 * (ch + 1)],
                    )
                return t

            # startup DMA order: first x chunk, K weights, V weights, masks,
            # Q inputs - so the PE never waits on a cold queue
            xch_pre = {0: load_xch(0)}
            for ct in range(NCT):
                nc.sync.dma_start(
                    out=w_sb[:, ct, 0:C],
                    in_=wA[128 * ct : 128 * (ct + 1), C : 2 * C],
                )
            if NCH > 1 and cpw > 1:
                xch_pre[1] = load_xch(1)
            for ct in range(NCT):
                nc.sync.dma_start(
                    out=w_sb[:, ct, C : 2 * C],
                    in_=wA[128 * ct : 128 * (ct + 1), 2 * C : 3 * C],
                )
            bv_src = bass.AP(
                tensor=bA.tensor, offset=bA.offset + 2 * C, ap=[[0, 128], [1, C]]
            )
            nc.gpsimd.dma_start(out=bv_bc, in_=bv_src)
            nc.sync.dma_start(out=mask_sb, in_=maskq)
            wq_sb = qproj.tile([128, NCT, C], BF16, name="wq_sb")
            xq_sb = qproj.tile([128, NCT, QW], BF16, name="xq_sb")
            for ct in range(NCT):
                nc.sync.dma_start(
                    out=wq_sb[:, ct, :], in_=wA[128 * ct : 128 * (ct + 1), 0:C]
                )
                nc.sync.dma_start(
                    out=xq_sb[:, ct, :], in_=xTq[128 * ct : 128 * (ct + 1), :]
                )

            for cp in range(NCH // cpw):
                # ---- project K^T / V for this wave's chunks ---------------
                v_chunks = [(0, C)] if C <= 512 else [(0, 384), (384, 768)]
                for ch in range(cpw * cp, cpw * cp + cpw):
                    xch = xch_pre.pop(ch) if ch in xch_pre else load_xch(ch)
                    for hp in range(HP):
                        ps_k = pkv.tile([128, 512], F32, name="ps_k", tag="pkv")
                        for ct in range(NCT):
                            nc.tensor.matmul(
                                ps_k,
                                w_sb[:, ct, 128 * hp : 128 * (hp + 1)],
                                xch[:, ct, :],
                                start=(ct == 0),
                                stop=(ct == NCT - 1),
                            )
                        nc.vector.tensor_scalar_add(
                            kt_t[:, hp, 512 * ch : 512 * (ch + 1)], ps_k,
                            bk_sb[:, hp : hp + 1],
                        )
                    for tt in range(4):
                        b_abs = 4 * ch + tt
                        for n0, n1 in v_chunks:
                            h0, h1 = n0 // 64, n1 // 64
                            ps_v = pkv.tile([128, n1 - n0], F32, name="ps_v",
                                            tag="pkv")
                            for ct in range(NCT):
                                nc.tensor.matmul(
                                    ps_v,
                                    xch[:, ct, 128 * tt : 128 * (tt + 1)],
                                    w_sb[:, ct, C + n0 : C + n1],
                                    start=(ct == 0),
                                    stop=(ct == NCT - 1),
                                )
                            nc.vector.tensor_add(
                                vaug4[:, b_abs, h0:h1, 0:64],
                                ps_v.rearrange("p (h e) -> p h e", e=64),
                                bv_bc.rearrange("p (h e) -> p h e", e=64)[
                                    :, h0:h1, :
                                ],
                            )

                if cp == min(1, NCH // cpw - 1):
                    # prefetch output-projection weights mid-loop
                    for ct in range(NCT):
                        nc.sync.dma_start(
                            out=wp_sb[:, ct, :],
                            in_=wP[128 * ct : 128 * (ct + 1), :],
                        )
                    bp_src = bass.AP(
                        tensor=bP.tensor, offset=bP.offset, ap=[[0, 128], [1, C]]
                    )
                    nc.gpsimd.dma_start(out=bp_bc, in_=bp_src)
                if cp == 0:
                    # Q^T projection - emitted here so the PE chews K/V
                    # projection first while the Q inputs stream in
                    for hp in range(HP):
                        ps_q = pvp.tile([128, QW], F32, name="ps_q", tag="ps_y")
                        for ct in range(NCT):
                            nc.tensor.matmul(
                                ps_q,
                                wq_sb[:, ct, 128 * hp : 128 * (hp + 1)],
                                xq_sb[:, ct, :],
                                start=(ct == 0),
                                stop=(ct == NCT - 1),
                            )
                        nc.scalar.activation(
                            qt_t[:, hp, :], ps_q,
                            mybir.ActivationFunctionType.Identity,
                            bias=bq_sb[:, hp : hp + 1],
                        )
                    qproj.release()

                # ---- attention for this wave's key-blocks -----------------
                blocks = list(range(4 * cpw * cp, 4 * cpw * cp + 4 * cpw))
                n = cfg.nb(blocks[0])  # constant across the wave
                per = 512 // n  # blocks per single-bank exp batch
                bat_list = [
                    [(b, i * n) for i, b in enumerate(blocks[j : j + per])]
                    for j in range(0, len(blocks), per)
                ]
                for hp in range(HP):
                    for h in range(2):
                        hd = 2 * hp + h
                        ps_y = pvp.tile([128, 512], F32, name="ps_y", tag="ps_y")
                        for bi, bat in enumerate(bat_list):
                            sps = s_ps[h][bi % 2]
                            width = max(co + n for _, co in bat)
                            pt = ptp.tile(
                                [128, 512], BF16, name=f"pt{h}", tag=f"pt{h}"
                            )
                            for b, co in bat:
                                nc.tensor.matmul(
                                    sps[:, co : co + n],
                                    kt_t[64 * h : 64 * (h + 1), hp,
                                         128 * b : 128 * (b + 1)],
                                    qt_t[64 * h : 64 * (h + 1), hp, 0:n],
                                    start=True,
                                    stop=True,
                                )
                            nc.scalar.activation(
                                pt[:, 0:width], sps[:, 0:width], Exp,
                                scale=scale
                            )
                            nb_ = len(bat)
                            r0 = bat[0][0] % cfg.ncores
                            if nb_ == 1:
                                nc.vector.tensor_mul(
                                    pt[:, n - 128 : n],
                                    pt[:, n - 128 : n],
                                    mask_sb[:, 128 * r0 : 128 * (r0 + 1)],
                                )
                            else:
                                pts = pt[:, 0 : n * nb_].rearrange(
                                    "p (b n) -> p b n", n=n
                                )[:, :, n - 128 : n]
                                msk = mask_sb[
                                    :, 128 * r0 : 128 * (r0 + nb_)
                                ].rearrange("p (b n) -> p b n", n=128)
                                nc.vector.tensor_mul(pts, pts, msk)
                            for b, co in bat:
                                nc.tensor.matmul(
                                    ps_y[0:65, 0:n],
                                    vaug[:, b, 65 * hd : 65 * (hd + 1)],
                                    pt[:, co : co + n],
                                    start=(b == blocks[0]),
                                    stop=(b == blocks[-1]),
                                )
                        if cp == 0:
                            nc.vector.tensor_copy(
                                yacc[0:65, hd, 0:n], ps_y[0:65, 0:n]
                            )
                        else:
                            nc.vector.tensor_add(
                                yacc[0:65, hd, 0:n],
                                yacc[0:65, hd, 0:n],
                                ps_y[0:65, 0:n],
                            )
                        if cp == NCH // cpw - 1:
                            # normalize this head now - overlaps the
                            # remaining heads' attention
                            rec = nrm.tile([1, QW], F32, name="rec", tag="rec")
                            rbc = nrm.tile([64, QW], F32, name="rbc", tag="rbc")
                            rc_ps = pkv.tile([64, QW], F32, name="rc_ps",
                                             tag="pkv")
                            nc.vector.reciprocal(rec, yacc[64:65, hd, :])
                            nc.tensor.matmul(
                                rc_ps, ones11[0:1, :], rec, start=True,
                                stop=True
                            )
                            nc.scalar.copy(rbc, rc_ps)
                            nc.vector.tensor_mul(
                                ytf[64 * h : 64 * (h + 1), hp, :],
                                yacc[0:64, hd, :], rbc
                            )

        # ---- output projection -------------------------------------------
        with (
            tc.tile_pool(name="ops", bufs=2, space="PSUM") as ops,
            tc.tile_pool(name="osb", bufs=2) as osb,
        ):
            for g in range(cfg.QTC):
                ps_o = ops.tile([128, C], F32, name="ps_o", tag="ps_o")
                for n0, n1 in ((0, 512), (512, C)) if C > 512 else ((0, C),):
                    for hp in range(HP):
                        nc.tensor.matmul(
                            ps_o[:, n0:n1],
                            ytf[:, hp, 128 * g : 128 * (g + 1)],
                            wp_sb[:, hp, n0:n1],
                            start=(hp == 0),
                            stop=(hp == HP - 1),
                        )
                yo = osb.tile([128, C], F32, name="yo", tag="yo")
                nc.vector.tensor_add(yo, ps_o, bp_bc)
                nc.sync.dma_start(out=y[128 * g : 128 * (g + 1), :], in_=yo)


# ---------------------------------------------------------------------------
# host side
# ---------------------------------------------------------------------------


def make_in_maps(x, w_attn, b_attn, w_proj, b_proj, cfg=CFG):
    xT = np.ascontiguousarray(x.reshape(cfg.T, cfg.C).T).astype(NPBF16)
    wA = w_attn.astype(NPBF16)
    wP = w_proj.astype(NPBF16)
    bA = np.ascontiguousarray(b_attn.astype(np.float32))
    bP = np.ascontiguousarray(b_proj.astype(np.float32))
    jl = np.arange(128)[:, None]
    ii = np.arange(128)[None, :]
    in_maps = []
    for c in range(cfg.ncores):
        cols = np.concatenate(
            [xT[:, 128 * t : 128 * (t + 1)] for t in cfg.qtiles(c)], axis=1
        )
        masks = np.stack(
            [(jl - ii <= 128 * (c - r)) for r in range(cfg.ncores)]
        ).astype(np.float32)
        maskq = np.ascontiguousarray(
            masks.transpose(1, 0, 2).reshape(128, cfg.ncores * 128)
        ).astype(NPBF16)
        in_maps.append(
            {
                "xT": xT,
                "xTq": np.ascontiguousarray(cols),
                "wA": wA,
                "wP": wP,
                "bA": bA,
                "bP": bP,
                "maskq": maskq,
            }
        )
    return in_maps


def declare_io(nc, cfg=CFG):
    ins = {
        "xT": nc.dram_tensor("xT", [cfg.C, cfg.T], BF16, kind="ExternalInput").ap(),
        "xTq": nc.dram_tensor("xTq", [cfg.C, cfg.QW], BF16, kind="ExternalInput").ap(),
        "wA": nc.dram_tensor("wA", [cfg.C, 3 * cfg.C], BF16, kind="ExternalInput").ap(),
        "wP": nc.dram_tensor("wP", [cfg.C, cfg.C], BF16, kind="ExternalInput").ap(),
        "bA": nc.dram_tensor("bA", [3 * cfg.C], F32, kind="ExternalInput").ap(),
        "bP": nc.dram_tensor("bP", [cfg.C], F32, kind="ExternalInput").ap(),
        "maskq": nc.dram_tensor(
            "maskq", [128, cfg.ncores * 128], BF16, kind="ExternalInput"
        ).ap(),
    }
    outs = {
        "y": nc.dram_tensor("y", [cfg.QW, cfg.C], F32, kind="ExternalOutput").ap()
    }
    return ins, outs


def build_program(cfg=CFG, repeat=1, phases=(1, 2, 3), fused=True, cpw=2):
    nc = bacc.Bacc("TRN2", target_bir_lowering=False, debug=False,
                   num_devices=cfg.ncores)
    ins, outs = declare_io(nc, cfg)
    builder = build_kernel_fused if fused else build_kernel
    with tile.TileContext(nc) as tc:
        for _ in range(repeat):
            if fused:
                builder(tc, outs, ins, cfg, cpw=cpw)
            else:
                builder(tc, outs, ins, cfg, phases=phases)
    nc.compile()
    return nc


def assemble_output(results, cfg=CFG):
    y = np.empty((cfg.T, cfg.C), np.float32)
    for c in range(cfg.ncores):
        yc = results[c]["y"]
        for g, t in enumerate(cfg.qtiles(c)):
            y[128 * t : 128 * (t + 1)] = yc[128 * g : 128 * (g + 1)]
    return y.reshape(1, cfg.T, cfg.C)


_PROGRAM = None


def kernel(x, w_attn, b_attn, w_proj, b_proj):
    global _PROGRAM
    cfg = CFG
    x = np.asarray(x, np.float32)
    if _PROGRAM is None:
        _PROGRAM = build_program(cfg)
    in_maps = make_in_maps(
        x, np.asarray(w_attn), np.asarray(b_attn), np.asarray(w_proj),
        np.asarray(b_proj), cfg
    )
    res = run_bass_kernel_spmd(_PROGRAM, in_maps, core_ids=list(range(cfg.ncores)))
    return assemble_output(res.results, cfg)


if __name__ == "__main__":
    import reference

    inputs = {k: np.asarray(v) for k, v in reference.setup_inputs().items()}
    out = kernel(**inputs)
    print("kernel output", out.shape, out.dtype)



# revision 31
# speedup vs baseline: 1.0985x; 1.0985x over previous
"""Causal self-attention (B=1, T=4096, C=768, H=12) on 8 TRN2 NeuronCores.

Strategy (single SPMD NEFF, no collectives):
  - Sequence-parallel over queries: core c owns q-tiles {c, c+8, c+16, c+24}
    (128 rows each, descending-extent column order). Slot s of every core
    processes key-blocks 8s..8s+7 (uniform instruction stream across cores);
    the true causal boundary is enforced by a per-core binary mask library
    passed as input data, so ONE program serves all 8 cores.
  - K/V/Q projections run as error-compensated fp8 DoubleRowSwInterleave
    matmuls: host splits x and 16*w_attn into e4m3 (hi, lo) pairs and the
    kernel computes xh*wh + xh*wl + xl*wh (the lo*lo term is negligible).
    Each DRI matmul contracts TWO 128-row k-tiles per pass at 0.5 cyc/row,
    so the 9-matmul group costs 0.75x the bf16 equivalent with bf16-class
    accuracy (measured end-to-end rel err 3.4e-3 for the projections).
  - Attention scores stay transposed: S^T = K @ Q^T with keys on partitions;
    exp runs PSUM->SBUF on ScalarE with scale 1/2048 (the 16x weight
    prescale squares into S) and bias -2 so exp output fits fp8e4 range.
  - P^T is written as fp8e4; PV uses DRI pairing two CONSECUTIVE KEY BLOCKS
    per pass (keys are the contraction dim), with V stored as interleaved
    fp8 (hi, lo) stationaries: y = P*vh + P*vl keeps v at bf16-class
    precision while PV runs at 2x bf16 speed. The V bias is folded into an
    effective output-projection bias on the host (exact).
  - K/V live in small rolling per-wave buffers (each wave's blocks are only
    read by that wave's attention). The softmax denominator falls out of a
    65th all-ones column of the padded-to-128 interleaved V stationary.
  - Measured end-to-end relative error vs the fp32 reference: ~8e-3.
"""

import contextlib
from dataclasses import dataclass

import ml_dtypes
import numpy as np

import concourse.bass as bass
import concourse.mybir as mybir
import concourse.tile as tile
from concourse import bacc
from concourse.bass_utils import run_bass_kernel_spmd

BF16 = mybir.dt.bfloat16
F32 = mybir.dt.float32
E4 = mybir.dt.float8e4
NPBF16 = ml_dtypes.bfloat16
NPE4 = ml_dtypes.float8_e4m3
DRI = mybir.MatmulPerfMode.DoubleRowSwInterleave

SW = 16.0  # weight prescale (power of two: commutes with rounding)
EXP_SCALE = 1.0 / (8.0 * SW * SW)  # 1/(sqrt(D) * SW^2)
EXP_BIAS = -2.0  # keeps exp output within fp8e4 range; cancels in softmax


@dataclass(frozen=True)
class Cfg:
    T: int = 4096
    H: int = 12
    D: int = 64
    ncores: int = 8

    @property
    def C(self):
        return self.H * self.D

    @property
    def HP(self):  # head pairs
        return self.H // 2

    @property
    def NKB(self):  # 128-row key blocks
        return self.T // 128

    @property
    def NCH(self):  # 512-row key chunks
        return self.T // 512

    @property
    def QTC(self):  # q-tiles per core
        return self.T // 128 // self.ncores

    @property
    def QW(self):  # q columns per core
        return 128 * self.QTC

    @property
    def NCT(self):  # 128-row contraction tiles over C
        return self.C // 128

    @property
    def NJP(self):  # contraction k-tile pairs
        return self.NCT // 2

    def nb(self, b):  # valid q-column prefix width for key-block b
        return 128 * (self.QTC - b // self.ncores)

    def qtiles(self, c):  # global q-tile indices for core c, descending extent
        return [c + self.ncores * (self.QTC - 1 - g) for g in range(self.QTC)]


CFG = Cfg()


def build_kernel_v3(tc, outs, ins, cfg=CFG):
    nc = tc.nc
    C, H, HP, NJP = cfg.C, cfg.H, cfg.HP, cfg.NJP
    QW, NCH = cfg.QW, cfg.NCH
    Exp = mybir.ActivationFunctionType.Exp
    Ident = mybir.ActivationFunctionType.Identity

    xh, xl = ins["xh"], ins["xl"]
    xilh, xill = ins["xilh"], ins["xill"]
    xqh, xql = ins["xqh"], ins["xql"]
    wkilh, wkill = ins["wkilh"], ins["wkill"]
    wqilh, wqill = ins["wqilh"], ins["wqill"]
    wvrh, wvrl = ins["wvrh"], ins["wvrl"]
    wP = ins["wP"]
    bq_in, bk_in, bP_in = ins["bq"], ins["bk"], ins["bP"]
    maskq = ins["maskq"]
    y = outs["y"]

    stack = contextlib.ExitStack()
    with stack:
        persist = stack.enter_context(tc.tile_pool(name="persist", bufs=1))

        # rolling per-wave K^T (bf16, scaled 16x) and interleaved V (fp8 hi/lo)
        kt_roll = persist.tile([128, 2, HP, 1024], BF16, name="kt_roll")
        vh_roll = persist.tile([128, 2, 4, H, 256], E4, name="vh_roll")
        vl_roll = persist.tile([128, 2, 4, H, 256], E4, name="vl_roll")
        qt_t = persist.tile([128, HP, QW], BF16, name="qt_t")
        ytf = persist.tile([128, HP, QW], BF16, name="ytf")
        yacc = persist.tile([128, H, QW], F32, name="yacc")  # rows 0:65 used
        mask_sb = persist.tile([128, cfg.ncores * 128], BF16, name="mask_sb")
        wp_sb = persist.tile([128, cfg.NCT, C], BF16, name="wp_sb")
        wkh_sb = persist.tile([128, NJP, HP, 256], E4, name="wkh_sb")
        wkl_sb = persist.tile([128, NJP, HP, 256], E4, name="wkl_sb")
        wvh_sb = persist.tile([128, cfg.NCT, C], E4, name="wvh_sb")
        wvl_sb = persist.tile([128, cfg.NCT, C], E4, name="wvl_sb")
        bq_sb = persist.tile([128, HP], F32, name="bq_sb")
        bk_sb = persist.tile([128, HP], F32, name="bk_sb")
        bp_bc = persist.tile([128, C], F32, name="bp_bc")
        ones11 = persist.tile([1, 64], F32, name="ones11")
        ebias = persist.tile([128, 1], F32, name="ebias")

        nc.sync.dma_start(out=bq_sb, in_=bq_in.rearrange("(hp p) -> p hp", p=128))
        nc.sync.dma_start(out=bk_sb, in_=bk_in.rearrange("(hp p) -> p hp", p=128))
        nc.vector.memset(ebias, EXP_BIAS)
        nc.vector.memset(ones11, 1.0 / SW)
        # touch Exp early so the ACT table set loads during startup DMAs
        nc.scalar.activation(ones11, ones11, Exp, scale=0.0)
        nc.vector.memset(ones11, 1.0 / SW)
        # V stationaries: zero the pad region once (gpsimd - memset runs at
        # full efficiency there and Pool is idle during startup); set the
        # ones column (logical col 64 of 128 -> interleaved positions 126-127).
        vh4 = vh_roll.rearrange("p w q h (t two) -> p w q h t two", two=2)
        vl4 = vl_roll.rearrange("p w q h (t two) -> p w q h t two", two=2)
        nc.gpsimd.memset(vh4[:, :, :, :, 0:63, :], 0.0)
        nc.gpsimd.memset(vl4[:, :, :, :, 0:64, :], 0.0)
        nc.vector.memset(vh4[:, :, :, :, 63:64, :], 1.0)

        with (
            tc.tile_pool(name="xpool", bufs=3) as xpool,
            tc.tile_pool(name="pkv", bufs=2, space="PSUM") as pkv,
            tc.tile_pool(name="aps", bufs=2, space="PSUM") as aps,
            tc.tile_pool(name="pvp", bufs=2, space="PSUM") as pvp,
            tc.tile_pool(name="ptp", bufs=3) as ptp,
            tc.tile_pool(name="nrm", bufs=1) as nrm,
        ):
            qproj = tc.alloc_tile_pool(name="qproj", bufs=1)

            xhr = xh.rearrange("(j p) t -> p j t", p=128)
            xlr = xl.rearrange("(j p) t -> p j t", p=128)

            def load_xch(ch):
                # four parallel DMA queues so one chunk's pieces stream
                # concurrently instead of serializing on the SP queue
                th = xpool.tile([128, cfg.NCT, 512], E4, name="xch_h", tag="xh")
                tl = xpool.tile([128, cfg.NCT, 512], E4, name="xch_l", tag="xl")
                tih = xpool.tile([128, NJP, 4, 256], E4, name="xil_h", tag="xih")
                til = xpool.tile([128, NJP, 4, 256], E4, name="xil_l", tag="xil")
                nc.sync.dma_start(out=th, in_=xhr[:, :, 512 * ch : 512 * (ch + 1)])
                nc.sync.dma_start(out=tl, in_=xlr[:, :, 512 * ch : 512 * (ch + 1)])
                nc.sync.dma_start(out=tih, in_=xilh[:, :, 4 * ch : 4 * ch + 4, :])
                nc.sync.dma_start(out=til, in_=xill[:, :, 4 * ch : 4 * ch + 4, :])
                return th, tl, tih, til

            # startup DMA order: first x chunk, K weights, V weights, masks,
            # Q inputs - so the PE never waits on a cold queue
            xch_pre = {0: load_xch(0)}
            nc.sync.dma_start(out=wkh_sb, in_=wkilh)
            nc.sync.dma_start(out=wkl_sb, in_=wkill)
            xch_pre[1] = load_xch(1)
            nc.sync.dma_start(
                out=wvh_sb, in_=wvrh.rearrange("(j p) t -> p j t", p=128)
            )
            nc.sync.dma_start(
                out=wvl_sb, in_=wvrl.rearrange("(j p) t -> p j t", p=128)
            )
            nc.sync.dma_start(out=mask_sb, in_=maskq)
            wqh_sb = qproj.tile([128, NJP, HP, 256], E4, name="wqh_sb")
            wql_sb = qproj.tile([128, NJP, HP, 256], E4, name="wql_sb")
            xqh_sb = qproj.tile([128, cfg.NCT, QW], E4, name="xqh_sb")
            xql_sb = qproj.tile([128, cfg.NCT, QW], E4, name="xql_sb")
            nc.sync.dma_start(out=wqh_sb, in_=wqilh)
            nc.sync.dma_start(out=wql_sb, in_=wqill)
            nc.sync.dma_start(out=xqh_sb, in_=xqh.rearrange("(j p) t -> p j t", p=128))
            nc.sync.dma_start(out=xql_sb, in_=xql.rearrange("(j p) t -> p j t", p=128))

            def comp_dri(ps, wil_h, wil_l, xp_h, xp_l, n0=None, n1=None):
                """9-term compensated DRI group into `ps`.

                wil_*: callables j -> stationary AP [128, 2*M interleaved]
                xp_*: callables j -> moving AP [128, 2, N]
                """
                terms = [(wil_h, xp_h), (wil_l, xp_h), (wil_h, xp_l)]
                nmm = 0
                for wf, xf in terms:
                    for j in range(NJP):
                        nc.tensor.matmul(
                            ps,
                            wf(j).rearrange("p (m two) -> p m two", two=2),
                            xf(j),
                            start=(nmm == 0),
                            stop=(nmm == 3 * NJP - 1),
                            perf_mode=DRI,
                        )
                        nmm += 1

            for cp in range(NCH // 2):
                par = cp % 2
                chunks = (2 * cp, 2 * cp + 1)
                # ---- project K^T / V for this wave's two chunks ------------
                for half, ch in enumerate(chunks):
                    th, tl, tih, til = (
                        xch_pre.pop(ch) if ch in xch_pre else load_xch(ch)
                    )
                    sched = [("k", hp) for hp in range(HP)] + [
                        ("v", (tt, nn)) for tt in range(4) for nn in range(2)
                    ]
                    for kind, item in sched:
                      if kind == "k":
                        hp = item
                        ps_k = pkv.tile([128, 512], F32, name="ps_k", tag="pkv")
                        comp_dri(
                            ps_k,
                            lambda j, hp=hp: wkh_sb[:, j, hp, :],
                            lambda j, hp=hp: wkl_sb[:, j, hp, :],
                            lambda j: th[:, 2 * j : 2 * j + 2, :],
                            lambda j: tl[:, 2 * j : 2 * j + 2, :],
                        )
                        nc.vector.tensor_scalar_add(
                            kt_roll[:, par, hp, 512 * half : 512 * (half + 1)],
                            ps_k,
                            bk_sb[:, hp : hp + 1],
                        )
                      else:
                        tt, nn = item
                        pi = 2 * half + tt // 2  # pair index in wave
                        pb = tt % 2  # block within pair
                        for n0, n1 in (((0, 384),) if nn == 0 else ((384, 768),)):
                            h0, h1 = n0 // 64, n1 // 64
                            ps_v = pkv.tile([128, 384], F32, name="ps_v", tag="pkv")
                            nmm = 0
                            for xf, wf in (
                                (tih, wvh_sb),
                                (tih, wvl_sb),
                                (til, wvh_sb),
                            ):
                                for j in range(NJP):
                                    nc.tensor.matmul(
                                        ps_v,
                                        xf[:, j, tt, :].rearrange(
                                            "p (m two) -> p m two", two=2
                                        ),
                                        wf[:, 2 * j : 2 * j + 2, n0:n1],
                                        start=(nmm == 0),
                                        stop=(nmm == 3 * NJP - 1),
                                        perf_mode=DRI,
                                    )
                                    nmm += 1
                            # v_hi = e4m3(v); v_lo = v - v_hi (bias folded into
                            # the output projection host-side)
                            psr = ps_v.rearrange("p (h e) -> p h e", e=64)
                            vh4w = vh_roll.rearrange(
                                "p w q h (t two) -> p w q h t two", two=2
                            )[:, par, pi, h0:h1, 64:128, pb]
                            vl4w = vl_roll.rearrange(
                                "p w q h (t two) -> p w q h t two", two=2
                            )[:, par, pi, h0:h1, 64:128, pb]
                            nc.vector.tensor_copy(vh4w, psr)
                            nc.vector.tensor_sub(vl4w, psr, vh4w)

                if cp == min(1, NCH // 2 - 1):
                    # prefetch output-projection weights mid-loop
                    for ct in range(cfg.NCT):
                        nc.sync.dma_start(
                            out=wp_sb[:, ct, :],
                            in_=wP[128 * ct : 128 * (ct + 1), :],
                        )
                    bp_src = bass.AP(
                        tensor=bP_in.tensor, offset=bP_in.offset, ap=[[0, 128], [1, C]]
                    )
                    nc.gpsimd.dma_start(out=bp_bc, in_=bp_src)
                if cp == 0:
                    # Q^T projection - emitted here so the PE chews K/V
                    # projection first while the Q inputs stream in
                    for hp in range(HP):
                        ps_q = pvp.tile([128, QW], F32, name="ps_q", tag="ps_y")
                        comp_dri(
                            ps_q,
                            lambda j, hp=hp: wqh_sb[:, j, hp, :],
                            lambda j, hp=hp: wql_sb[:, j, hp, :],
                            lambda j: xqh_sb[:, 2 * j : 2 * j + 2, :],
                            lambda j: xql_sb[:, 2 * j : 2 * j + 2, :],
                        )
                        nc.scalar.activation(
                            qt_t[:, hp, :], ps_q, Ident, bias=bq_sb[:, hp : hp + 1]
                        )
                    qproj.release()

                # ---- attention for this wave's 8 key-blocks ----------------
                nA = cfg.nb(4 * chunks[0])  # widths per half-wave
                for hp in range(HP):
                    for h in range(2):
                        hd = 2 * hp + h
                        ps_y = pvp.tile([128, 512], F32, name="ps_y", tag="ps_y")
                        for pi in range(4):
                            half = pi // 2
                            ch = chunks[half]
                            pl = pi % 2  # pair within the half-wave
                            ba = 4 * ch + 2 * pl
                            n = cfg.nb(ba)
                            pt = ptp.tile([128, 1024], E4, name=f"pt{h}",
                                          tag=f"pt{h}")
                            # pair layout: blocks at offsets 0 and 512 in both
                            # the 2-bank score tile and pt
                            sps = aps.tile([128, 1024], F32, name="sps",
                                           tag="sps")
                            blkv = pt.rearrange("p (b n) -> p b n", n=512)[
                                :, :, 0:n
                            ]
                            for pb in (0, 1):
                                bw = 4 * half + 2 * pl + pb  # kt_roll block
                                nc.tensor.matmul(
                                    sps[:, 512 * pb : 512 * pb + n],
                                    kt_roll[64 * h : 64 * (h + 1), par, hp,
                                            128 * bw : 128 * (bw + 1)],
                                    qt_t[64 * h : 64 * (h + 1), hp, 0:n],
                                    start=True,
                                    stop=True,
                                )
                            spsv = sps.rearrange("p (b n) -> p b n", n=512)
                            nc.scalar.activation(
                                blkv, spsv[:, :, 0:n],
                                Exp, scale=EXP_SCALE, bias=ebias,
                            )
                            # causal boundary: mask last 128 q-cols of each blk
                            r0 = ba % cfg.ncores
                            pts = blkv[:, :, n - 128 : n]
                            msk = mask_sb[:, 128 * r0 : 128 * (r0 + 2)].rearrange(
                                "p (b n) -> p b n", n=128
                            )
                            meng = nc.vector if pi % 2 == 0 else nc.gpsimd
                            meng.tensor_mul(pts, pts, msk)
                            # PV: two DRI matmuls (v_hi, v_lo), contraction
                            # over both blocks of the pair
                            for vroll in (vh_roll, vl_roll):
                                nc.tensor.matmul(
                                    ps_y[:, 0:n],
                                    vroll[:, par, pi, hd, :].rearrange(
                                        "p (m two) -> p m two", two=2
                                    ),
                                    blkv,
                                    start=(pi == 0 and vroll is vh_roll),
                                    stop=(pi == 3 and vroll is vl_roll),
                                    perf_mode=DRI,
                                    skip_group_check=True,
                                )
                        if cp == 0:
                            nc.vector.tensor_copy(
                                yacc[0:65, hd, 0:nA], ps_y[0:65, 0:nA]
                            )
                        else:
                            nc.vector.tensor_add(
                                yacc[0:65, hd, 0:nA],
                                yacc[0:65, hd, 0:nA],
                                ps_y[0:65, 0:nA],
                            )
                        if cp == NCH // 2 - 1:
                            # normalize this head now - overlaps the
                            # remaining heads' attention
                            rec = nrm.tile([1, QW], F32, name="rec", tag="rec")
                            rc_ps = pkv.tile([64, QW], F32, name="rc_ps",
                                             tag="pkv")
                            nc.vector.reciprocal(rec, yacc[64:65, hd, :])
                            nc.tensor.matmul(
                                rc_ps, ones11[0:1, :], rec, start=True, stop=True
                            )
                            nc.vector.tensor_mul(
                                ytf[64 * h : 64 * (h + 1), hp, :],
                                yacc[0:64, hd, :], rc_ps,
                            )

        # ---- output projection -------------------------------------------
        with (
            tc.tile_pool(name="ops", bufs=2, space="PSUM") as ops,
            tc.tile_pool(name="osb", bufs=2) as osb,
        ):
            for g in range(cfg.QTC):
                ps_o = ops.tile([128, C], F32, name="ps_o", tag="ps_o")
                for n0, n1 in ((0, 512), (512, C)):
                    for hp in range(HP):
                        nc.tensor.matmul(
                            ps_o[:, n0:n1],
                            ytf[:, hp, 128 * g : 128 * (g + 1)],
                            wp_sb[:, hp, n0:n1],
                            start=(hp == 0),
                            stop=(hp == HP - 1),
                        )
                yo = osb.tile([128, C], F32, name="yo", tag="yo")
                nc.vector.tensor_add(yo, ps_o, bp_bc)
                nc.sync.dma_start(out=y[128 * g : 128 * (g + 1), :], in_=yo)


# ---------------------------------------------------------------------------
# host side
# ---------------------------------------------------------------------------


def _hilo(a):
    hi = np.asarray(a, NPE4)
    lo = np.asarray(a - hi.astype(np.float32), NPE4)
    return hi, lo


def _ileave4(W4):
    """[NCT, 128, G, M] -> interleaved [128, NCT/2, G, 2M] walrus layout."""
    A = W4[0::2]  # [NJP, 128, G, M]
    B = W4[1::2]
    il = np.empty(A.shape[:3] + (2 * A.shape[3],), A.dtype)
    il[..., 0::2] = A[..., ::-1]
    il[..., 1::2] = B[..., ::-1]
    return np.ascontiguousarray(il.transpose(1, 0, 2, 3))


def make_in_maps(x, w_attn, b_attn, w_proj, b_proj, cfg=CFG):
    T, C, H, HP, NCT = cfg.T, cfg.C, cfg.H, cfg.HP, cfg.NCT
    xT = np.ascontiguousarray(x.reshape(T, C).T).astype(np.float32)  # [C,T]
    xh, xl = _hilo(xT)

    w16 = (np.asarray(w_attn, np.float32)) * SW
    wq16, wk16, wv16 = w16[:, 0:C], w16[:, C : 2 * C], w16[:, 2 * C :]

    def wil_pair(wsec):
        h, l = _hilo(wsec)
        W4h = h.reshape(NCT, 128, HP, 128)
        W4l = l.reshape(NCT, 128, HP, 128)
        return _ileave4(W4h), _ileave4(W4l)

    wqilh, wqill = wil_pair(wq16)
    wkilh, wkill = wil_pair(wk16)

    # V moving operand: per-head reversed d order (so the strided interleaved
    # SBUF write runs with a positive stride)
    wvr = np.ascontiguousarray(
        wv16.reshape(C, H, 64)[:, :, ::-1].reshape(C, C)
    )
    wvrh, wvrl = _hilo(wvr)

    # V stationary: x k-tile pairs interleaved per 128-key tile
    X4h = xh.astype(np.float32).reshape(NCT, 128, 32, 128)
    X4l = xl.astype(np.float32).reshape(NCT, 128, 32, 128)
    xilh = _ileave4(X4h.astype(NPE4))
    xill = _ileave4(X4l.astype(NPE4))

    wP = np.asarray(w_proj, np.float32).astype(NPBF16)
    bq = np.ascontiguousarray(np.asarray(b_attn[0:C], np.float32) * SW)
    bk = np.ascontiguousarray(np.asarray(b_attn[C : 2 * C], np.float32) * SW)
    # V bias folded into the output projection (exact)
    bP = np.ascontiguousarray(
        np.asarray(b_proj, np.float32)
        + np.asarray(b_attn[2 * C :], np.float32) @ np.asarray(w_proj, np.float32)
    )

    jl = np.arange(128)[:, None]
    ii = np.arange(128)[None, :]
    in_maps = []
    for c in range(cfg.ncores):
        colsh = np.concatenate(
            [xh[:, 128 * t : 128 * (t + 1)] for t in cfg.qtiles(c)], axis=1
        )
        colsl = np.concatenate(
            [xl[:, 128 * t : 128 * (t + 1)] for t in cfg.qtiles(c)], axis=1
        )
        # multiplicative {0,1} masks on the fp8 P slabs, per key-block residue
        masks = np.stack(
            [(jl - ii <= 128 * (c - r)) for r in range(cfg.ncores)]
        ).astype(np.float32)
        maskq = np.ascontiguousarray(
            masks.transpose(1, 0, 2).reshape(128, cfg.ncores * 128)
        ).astype(NPBF16)
        in_maps.append(
            {
                "xh": xh,
                "xl": xl,
                "xilh": xilh,
                "xill": xill,
                "xqh": np.ascontiguousarray(colsh),
                "xql": np.ascontiguousarray(colsl),
                "wqilh": wqilh,
                "wqill": wqill,
                "wkilh": wkilh,
                "wkill": wkill,
                "wvrh": wvrh,
                "wvrl": wvrl,
                "wP": wP,
                "bq": bq,
                "bk": bk,
                "bP": bP,
                "maskq": maskq,
            }
        )
    return in_maps


def declare_io(nc, cfg=CFG):
    C, T, HP, NJP, QW = cfg.C, cfg.T, cfg.HP, cfg.NJP, cfg.QW
    dt = nc.dram_tensor
    ins = {
        "xh": dt("xh", [C, T], E4, kind="ExternalInput").ap(),
        "xl": dt("xl", [C, T], E4, kind="ExternalInput").ap(),
        "xilh": dt("xilh", [128, NJP, 32, 256], E4, kind="ExternalInput").ap(),
        "xill": dt("xill", [128, NJP, 32, 256], E4, kind="ExternalInput").ap(),
        "xqh": dt("xqh", [C, QW], E4, kind="ExternalInput").ap(),
        "xql": dt("xql", [C, QW], E4, kind="ExternalInput").ap(),
        "wqilh": dt("wqilh", [128, NJP, HP, 256], E4, kind="ExternalInput").ap(),
        "wqill": dt("wqill", [128, NJP, HP, 256], E4, kind="ExternalInput").ap(),
        "wkilh": dt("wkilh", [128, NJP, HP, 256], E4, kind="ExternalInput").ap(),
        "wkill": dt("wkill", [128, NJP, HP, 256], E4, kind="ExternalInput").ap(),
        "wvrh": dt("wvrh", [C, C], E4, kind="ExternalInput").ap(),
        "wvrl": dt("wvrl", [C, C], E4, kind="ExternalInput").ap(),
        "wP": dt("wP", [C, C], BF16, kind="ExternalInput").ap(),
        "bq": dt("bq", [C], F32, kind="ExternalInput").ap(),
        "bk": dt("bk", [C], F32, kind="ExternalInput").ap(),
        "bP": dt("bP", [C], F32, kind="ExternalInput").ap(),
        "maskq": dt("maskq", [128, cfg.ncores * 128], BF16,
                    kind="ExternalInput").ap(),
    }
    outs = {
        "y": dt("y", [QW, C], F32, kind="ExternalOutput").ap()
    }
    return ins, outs


def build_program(cfg=CFG, repeat=1):
    nc = bacc.Bacc("TRN2", target_bir_lowering=False, debug=False,
                   num_devices=cfg.ncores)
    ins, outs = declare_io(nc, cfg)
    with tile.TileContext(nc) as tc:
        for _ in range(repeat):
            build_kernel_v3(tc, outs, ins, cfg)
    nc.compile()
    return nc


def assemble_output(results, cfg=CFG):
    y = np.empty((cfg.T, cfg.C), np.float32)
    for c in range(cfg.ncores):
        yc = results[c]["y"]
        for g, t in enumerate(cfg.qtiles(c)):
            y[128 * t : 128 * (t + 1)] = yc[128 * g : 128 * (g + 1)]
    return y.reshape(1, cfg.T, cfg.C)


_PROGRAM = None


def kernel(x, w_attn, b_attn, w_proj, b_proj):
    global _PROGRAM
    cfg = CFG
    x = np.asarray(x, np.float32)
    if _PROGRAM is None:
        _PROGRAM = build_program(cfg)
    in_maps = make_in_maps(
        x, np.asarray(w_attn), np.asarray(b_attn), np.asarray(w_proj),
        np.asarray(b_proj), cfg
    )
    res = run_bass_kernel_spmd(_PROGRAM, in_maps, core_ids=list(range(cfg.ncores)))
    return assemble_output(res.results, cfg)


if __name__ == "__main__":
    import reference

    inputs = {k: np.asarray(v) for k, v in reference.setup_inputs().items()}
    out = kernel(**inputs)
    print("kernel output", out.shape, out.dtype)


# revision 39
# speedup vs baseline: 1.1397x; 1.0375x over previous
"""Causal self-attention (B=1, T=4096, C=768, H=12) on 8 TRN2 NeuronCores.

Strategy (single SPMD NEFF, no collectives):
  - Sequence-parallel over queries: core c owns q-tiles {c, c+8, c+16, c+24}
    (128 rows each, descending-extent column order). Slot s of every core
    processes key-blocks 8s..8s+7 (uniform instruction stream across cores);
    the true causal boundary is enforced by a per-core binary mask library
    passed as input data, so ONE program serves all 8 cores.
  - K/V/Q projections run as error-compensated fp8 DoubleRowSwInterleave
    matmuls: host splits x and 16*w_attn into e4m3 (hi, lo) pairs and the
    kernel computes xh*wh + xh*wl + xl*wh (the lo*lo term is negligible).
    Each DRI matmul contracts TWO 128-row k-tiles per pass at 0.5 cyc/row,
    so the 9-matmul group costs 0.75x the bf16 equivalent with bf16-class
    accuracy (measured end-to-end rel err 3.4e-3 for the projections).
  - Attention scores stay transposed: S^T = K @ Q^T with keys on partitions;
    exp runs PSUM->SBUF on ScalarE with scale 1/2048 (the 16x weight
    prescale squares into S) and bias -2 so exp output fits fp8e4 range.
  - P^T is written as fp8e4; PV uses DRI pairing two CONSECUTIVE KEY BLOCKS
    per pass (keys are the contraction dim), with V stored as interleaved
    fp8 (hi, lo) stationaries: y = P*vh + P*vl keeps v at bf16-class
    precision while PV runs at 2x bf16 speed. The V bias is folded into an
    effective output-projection bias on the host (exact).
  - K/V live in small rolling per-wave buffers (each wave's blocks are only
    read by that wave's attention). The softmax denominator falls out of a
    65th all-ones column of the padded-to-128 interleaved V stationary.
  - Per (head, wave): sweep 1 computes QK + exp + mask for all four block
    pairs (pt tiles buffered), sweep 2 fires the eight PV matmuls back to
    back so the PE never waits on a freshly produced mask; masks run 1/4 on
    DVE and 3/4 on GPSIMD to balance the elementwise queues.
  - Measured end-to-end relative error vs the fp32 reference: 1.2e-2
    (matching a numpy emulation of the same quantization points).
"""

import contextlib
from dataclasses import dataclass

import ml_dtypes
import numpy as np

import concourse.bass as bass
import concourse.mybir as mybir
import concourse.tile as tile
from concourse import bacc
from concourse.bass_utils import run_bass_kernel_spmd

BF16 = mybir.dt.bfloat16
F32 = mybir.dt.float32
E4 = mybir.dt.float8e4
NPBF16 = ml_dtypes.bfloat16
NPE4 = ml_dtypes.float8_e4m3
DRI = mybir.MatmulPerfMode.DoubleRowSwInterleave

SW = 16.0  # weight prescale (power of two: commutes with rounding)
EXP_SCALE = 1.0 / (8.0 * SW * SW)  # 1/(sqrt(D) * SW^2)
EXP_BIAS = -2.0  # keeps exp output within fp8e4 range; cancels in softmax


@dataclass(frozen=True)
class Cfg:
    T: int = 4096
    H: int = 12
    D: int = 64
    ncores: int = 8

    @property
    def C(self):
        return self.H * self.D

    @property
    def HP(self):  # head pairs
        return self.H // 2

    @property
    def NKB(self):  # 128-row key blocks
        return self.T // 128

    @property
    def NCH(self):  # 512-row key chunks
        return self.T // 512

    @property
    def QTC(self):  # q-tiles per core
        return self.T // 128 // self.ncores

    @property
    def QW(self):  # q columns per core
        return 128 * self.QTC

    @property
    def NCT(self):  # 128-row contraction tiles over C
        return self.C // 128

    @property
    def NJP(self):  # contraction k-tile pairs
        return self.NCT // 2

    def nb(self, b):  # valid q-column prefix width for key-block b
        return 128 * (self.QTC - b // self.ncores)

    def qtiles(self, c):  # global q-tile indices for core c, descending extent
        return [c + self.ncores * (self.QTC - 1 - g) for g in range(self.QTC)]


CFG = Cfg()


def build_kernel_v3(tc, outs, ins, cfg=CFG):
    nc = tc.nc
    C, H, HP, NJP = cfg.C, cfg.H, cfg.HP, cfg.NJP
    QW, NCH = cfg.QW, cfg.NCH
    Exp = mybir.ActivationFunctionType.Exp
    Ident = mybir.ActivationFunctionType.Identity

    xh, xl = ins["xh"], ins["xl"]
    xilh, xill = ins["xilh"], ins["xill"]
    xqh, xql = ins["xqh"], ins["xql"]
    wkilh, wkill = ins["wkilh"], ins["wkill"]
    wqilh, wqill = ins["wqilh"], ins["wqill"]
    wvrh, wvrl = ins["wvrh"], ins["wvrl"]
    wP = ins["wP"]
    bq_in, bk_in, bP_in = ins["bq"], ins["bk"], ins["bP"]
    maskq = ins["maskq"]
    y = outs["y"]

    stack = contextlib.ExitStack()
    with stack:
        persist = stack.enter_context(tc.tile_pool(name="persist", bufs=1))

        # rolling per-wave K^T (bf16, scaled 16x) and interleaved V (fp8 hi/lo)
        kt_roll = persist.tile([128, 2, HP, 1024], BF16, name="kt_roll")
        vh_roll = persist.tile([128, 2, 4, H, 256], E4, name="vh_roll")
        vl_roll = persist.tile([128, 2, 4, H, 256], E4, name="vl_roll")
        qt_t = persist.tile([128, HP, QW], BF16, name="qt_t")
        ytf = persist.tile([128, HP, QW], BF16, name="ytf")
        yacc = persist.tile([128, H, QW], F32, name="yacc")  # rows 0:65 used
        mask_sb = persist.tile([128, cfg.ncores * 128], BF16, name="mask_sb")
        wp_sb = persist.tile([128, cfg.NCT, C], BF16, name="wp_sb")
        wkh_sb = persist.tile([128, NJP, HP, 256], E4, name="wkh_sb")
        wkl_sb = persist.tile([128, NJP, HP, 256], E4, name="wkl_sb")
        wvh_sb = persist.tile([128, cfg.NCT, C], E4, name="wvh_sb")
        wvl_sb = persist.tile([128, cfg.NCT, C], E4, name="wvl_sb")
        bq_sb = persist.tile([128, HP], F32, name="bq_sb")
        bk_sb = persist.tile([128, HP], F32, name="bk_sb")
        bp_bc = persist.tile([128, C], F32, name="bp_bc")
        ones11 = persist.tile([1, 64], F32, name="ones11")
        ebias = persist.tile([128, 1], F32, name="ebias")

        nc.sync.dma_start(out=bq_sb, in_=bq_in.rearrange("(hp p) -> p hp", p=128))
        nc.sync.dma_start(out=bk_sb, in_=bk_in.rearrange("(hp p) -> p hp", p=128))
        nc.vector.memset(ebias, EXP_BIAS)
        nc.vector.memset(ones11, 1.0 / SW)
        # touch Exp early so the ACT table set loads during startup DMAs
        nc.scalar.activation(ones11, ones11, Exp, scale=0.0)
        nc.vector.memset(ones11, 1.0 / SW)
        # V stationaries: zero the pad region once (gpsimd - memset runs at
        # full efficiency there and Pool is idle during startup); set the
        # ones column (logical col 64 of 128 -> interleaved positions 126-127).
        vh4 = vh_roll.rearrange("p w q h (t two) -> p w q h t two", two=2)
        vl4 = vl_roll.rearrange("p w q h (t two) -> p w q h t two", two=2)
        nc.gpsimd.memset(vh4[:, :, :, :, 0:63, :], 0.0)
        nc.gpsimd.memset(vl4[:, :, :, :, 0:64, :], 0.0)
        nc.vector.memset(vh4[:, :, :, :, 63:64, :], 1.0)

        with (
            tc.tile_pool(name="xpool", bufs=3) as xpool,
            tc.tile_pool(name="pkv", bufs=2, space="PSUM") as pkv,
            tc.tile_pool(name="aps", bufs=2, space="PSUM") as aps,
            tc.tile_pool(name="pvp", bufs=2, space="PSUM") as pvp,
            tc.tile_pool(name="ptp", bufs=6) as ptp,
            tc.tile_pool(name="nrm", bufs=1) as nrm,
        ):
            qproj = tc.alloc_tile_pool(name="qproj", bufs=1)

            xhr = xh.rearrange("(j p) t -> p j t", p=128)
            xlr = xl.rearrange("(j p) t -> p j t", p=128)

            def load_xch(ch):
                th = xpool.tile([128, cfg.NCT, 512], E4, name="xch_h", tag="xh")
                tl = xpool.tile([128, cfg.NCT, 512], E4, name="xch_l", tag="xl")
                tih = xpool.tile([128, NJP, 4, 256], E4, name="xil_h", tag="xih")
                til = xpool.tile([128, NJP, 4, 256], E4, name="xil_l", tag="xil")
                nc.sync.dma_start(out=th, in_=xhr[:, :, 512 * ch : 512 * (ch + 1)])
                nc.sync.dma_start(out=tl, in_=xlr[:, :, 512 * ch : 512 * (ch + 1)])
                nc.sync.dma_start(out=tih, in_=xilh[:, :, 4 * ch : 4 * ch + 4, :])
                nc.sync.dma_start(out=til, in_=xill[:, :, 4 * ch : 4 * ch + 4, :])
                return th, tl, tih, til

            # startup DMA order: first x chunk, K weights, V weights, masks,
            # Q inputs - so the PE never waits on a cold queue
            xch_pre = {0: load_xch(0)}
            nc.sync.dma_start(out=wkh_sb, in_=wkilh)
            nc.sync.dma_start(out=wkl_sb, in_=wkill)
            xch_pre[1] = load_xch(1)
            nc.sync.dma_start(
                out=wvh_sb, in_=wvrh.rearrange("(j p) t -> p j t", p=128)
            )
            nc.sync.dma_start(
                out=wvl_sb, in_=wvrl.rearrange("(j p) t -> p j t", p=128)
            )
            nc.sync.dma_start(out=mask_sb, in_=maskq)
            wqh_sb = qproj.tile([128, NJP, HP, 256], E4, name="wqh_sb")
            wql_sb = qproj.tile([128, NJP, HP, 256], E4, name="wql_sb")
            xqh_sb = qproj.tile([128, cfg.NCT, QW], E4, name="xqh_sb")
            xql_sb = qproj.tile([128, cfg.NCT, QW], E4, name="xql_sb")
            nc.sync.dma_start(out=wqh_sb, in_=wqilh)
            nc.sync.dma_start(out=wql_sb, in_=wqill)
            nc.sync.dma_start(out=xqh_sb, in_=xqh.rearrange("(j p) t -> p j t", p=128))
            nc.sync.dma_start(out=xql_sb, in_=xql.rearrange("(j p) t -> p j t", p=128))

            def comp_dri(ps, wil_h, wil_l, xp_h, xp_l, n0=None, n1=None):
                """9-term compensated DRI group into `ps`.

                wil_*: callables j -> stationary AP [128, 2*M interleaved]
                xp_*: callables j -> moving AP [128, 2, N]
                """
                terms = [(wil_h, xp_h), (wil_l, xp_h), (wil_h, xp_l)]
                nmm = 0
                for wf, xf in terms:
                    for j in range(NJP):
                        nc.tensor.matmul(
                            ps,
                            wf(j).rearrange("p (m two) -> p m two", two=2),
                            xf(j),
                            start=(nmm == 0),
                            stop=(nmm == 3 * NJP - 1),
                            perf_mode=DRI,
                        )
                        nmm += 1

            for cp in range(NCH // 2):
                par = cp % 2
                chunks = (2 * cp, 2 * cp + 1)
                # ---- project K^T / V for this wave's two chunks ------------
                for half, ch in enumerate(chunks):
                    th, tl, tih, til = (
                        xch_pre.pop(ch) if ch in xch_pre else load_xch(ch)
                    )
                    sched = [("k", hp) for hp in range(HP)] + [
                        ("v", (tt, nn)) for tt in range(4) for nn in range(2)
                    ]
                    for kind, item in sched:
                      if kind == "k":
                        hp = item
                        ps_k = pkv.tile([128, 512], F32, name="ps_k", tag="pkv")
                        comp_dri(
                            ps_k,
                            lambda j, hp=hp: wkh_sb[:, j, hp, :],
                            lambda j, hp=hp: wkl_sb[:, j, hp, :],
                            lambda j: th[:, 2 * j : 2 * j + 2, :],
                            lambda j: tl[:, 2 * j : 2 * j + 2, :],
                        )
                        nc.vector.tensor_scalar_add(
                            kt_roll[:, par, hp, 512 * half : 512 * (half + 1)],
                            ps_k,
                            bk_sb[:, hp : hp + 1],
                        )
                      else:
                        tt, nn = item
                        pi = 2 * half + tt // 2  # pair index in wave
                        pb = tt % 2  # block within pair
                        for n0, n1 in (((0, 384),) if nn == 0 else ((384, 768),)):
                            h0, h1 = n0 // 64, n1 // 64
                            ps_v = pkv.tile([128, 384], F32, name="ps_v", tag="pkv")
                            nmm = 0
                            for xf, wf in (
                                (tih, wvh_sb),
                                (tih, wvl_sb),
                                (til, wvh_sb),
                            ):
                                for j in range(NJP):
                                    nc.tensor.matmul(
                                        ps_v,
                                        xf[:, j, tt, :].rearrange(
                                            "p (m two) -> p m two", two=2
                                        ),
                                        wf[:, 2 * j : 2 * j + 2, n0:n1],
                                        start=(nmm == 0),
                                        stop=(nmm == 3 * NJP - 1),
                                        perf_mode=DRI,
                                    )
                                    nmm += 1
                            # v_hi = e4m3(v); v_lo = v - v_hi (bias folded into
                            # the output projection host-side)
                            psr = ps_v.rearrange("p (h e) -> p h e", e=64)
                            vh4w = vh_roll.rearrange(
                                "p w q h (t two) -> p w q h t two", two=2
                            )[:, par, pi, h0:h1, 64:128, pb]
                            vl4w = vl_roll.rearrange(
                                "p w q h (t two) -> p w q h t two", two=2
                            )[:, par, pi, h0:h1, 64:128, pb]
                            nc.vector.tensor_copy(vh4w, psr)
                            nc.vector.tensor_sub(vl4w, psr, vh4w)

                if cp == min(1, NCH // 2 - 1):
                    # prefetch output-projection weights mid-loop
                    for ct in range(cfg.NCT):
                        nc.sync.dma_start(
                            out=wp_sb[:, ct, :],
                            in_=wP[128 * ct : 128 * (ct + 1), :],
                        )
                    bp_src = bass.AP(
                        tensor=bP_in.tensor, offset=bP_in.offset, ap=[[0, 128], [1, C]]
                    )
                    nc.gpsimd.dma_start(out=bp_bc, in_=bp_src)
                if cp == 0:
                    # Q^T projection - emitted here so the PE chews K/V
                    # projection first while the Q inputs stream in
                    for hp in range(HP):
                        ps_q = pvp.tile([128, QW], F32, name="ps_q", tag="ps_y")
                        comp_dri(
                            ps_q,
                            lambda j, hp=hp: wqh_sb[:, j, hp, :],
                            lambda j, hp=hp: wql_sb[:, j, hp, :],
                            lambda j: xqh_sb[:, 2 * j : 2 * j + 2, :],
                            lambda j: xql_sb[:, 2 * j : 2 * j + 2, :],
                        )
                        nc.scalar.activation(
                            qt_t[:, hp, :], ps_q, Ident, bias=bq_sb[:, hp : hp + 1]
                        )
                    qproj.release()

                # ---- attention for this wave's 8 key-blocks ----------------
                nA = cfg.nb(4 * chunks[0])  # widths per half-wave
                for hp in range(HP):
                    for h in range(2):
                        hd = 2 * hp + h
                        ps_y = pvp.tile([128, 512], F32, name="ps_y", tag="ps_y")
                        # sweep 1: QK + exp + mask for all four pairs (pt
                        # tiles held); sweep 2: all eight PV matmuls back to
                        # back - PV never waits on a freshly computed mask
                        ptl = []
                        for pi in range(4):
                            half = pi // 2
                            ch = chunks[half]
                            pl = pi % 2  # pair within the half-wave
                            ba = 4 * ch + 2 * pl
                            n = cfg.nb(ba)
                            pt = ptp.tile([128, 1024], E4, name=f"pt{h}",
                                          tag=f"pt{h}")
                            # pair layout: blocks at offsets 0 and 512 in both
                            # the 2-bank score tile and pt
                            sps = aps.tile([128, 1024], F32, name="sps",
                                           tag="sps")
                            blkv = pt.rearrange("p (b n) -> p b n", n=512)[
                                :, :, 0:n
                            ]
                            for pb in (0, 1):
                                bw = 4 * half + 2 * pl + pb  # kt_roll block
                                nc.tensor.matmul(
                                    sps[:, 512 * pb : 512 * pb + n],
                                    kt_roll[64 * h : 64 * (h + 1), par, hp,
                                            128 * bw : 128 * (bw + 1)],
                                    qt_t[64 * h : 64 * (h + 1), hp, 0:n],
                                    start=True,
                                    stop=True,
                                )
                            spsv = sps.rearrange("p (b n) -> p b n", n=512)
                            nc.scalar.activation(
                                blkv, spsv[:, :, 0:n],
                                Exp, scale=EXP_SCALE, bias=ebias,
                            )
                            # causal boundary: mask last 128 q-cols of each blk
                            r0 = ba % cfg.ncores
                            pts = blkv[:, :, n - 128 : n]
                            msk = mask_sb[:, 128 * r0 : 128 * (r0 + 2)].rearrange(
                                "p (b n) -> p b n", n=128
                            )
                            meng = nc.vector if pi == 0 else nc.gpsimd
                            meng.tensor_mul(pts, pts, msk)
                            ptl.append((pi, n, blkv))
                        for pi, n, blkv in ptl:
                            # PV: two DRI matmuls (v_hi, v_lo), contraction
                            # over both blocks of the pair
                            for vroll in (vh_roll, vl_roll):
                                nc.tensor.matmul(
                                    ps_y[:, 0:n],
                                    vroll[:, par, pi, hd, :].rearrange(
                                        "p (m two) -> p m two", two=2
                                    ),
                                    blkv,
                                    start=(pi == 0 and vroll is vh_roll),
                                    stop=(pi == 3 and vroll is vl_roll),
                                    perf_mode=DRI,
                                    skip_group_check=True,
                                )
                        if cp == 0:
                            nc.vector.tensor_copy(
                                yacc[0:65, hd, 0:nA], ps_y[0:65, 0:nA]
                            )
                        else:
                            nc.vector.tensor_add(
                                yacc[0:65, hd, 0:nA],
                                yacc[0:65, hd, 0:nA],
                                ps_y[0:65, 0:nA],
                            )
                        if cp == NCH // 2 - 1:
                            # normalize this head now - overlaps the
                            # remaining heads' attention
                            rec = nrm.tile([1, QW], F32, name="rec", tag="rec")
                            rc_ps = pkv.tile([64, QW], F32, name="rc_ps",
                                             tag="pkv")
                            nc.vector.reciprocal(rec, yacc[64:65, hd, :])
                            nc.tensor.matmul(
                                rc_ps, ones11[0:1, :], rec, start=True, stop=True
                            )
                            nc.vector.tensor_mul(
                                ytf[64 * h : 64 * (h + 1), hp, :],
                                yacc[0:64, hd, :], rc_ps,
                            )

        # ---- output projection -------------------------------------------
        with (
            tc.tile_pool(name="ops", bufs=2, space="PSUM") as ops,
            tc.tile_pool(name="osb", bufs=2) as osb,
        ):
            for g in range(cfg.QTC):
                ps_o = ops.tile([128, C], F32, name="ps_o", tag="ps_o")
                for n0, n1 in ((0, 512), (512, C)):
                    for hp in range(HP):
                        nc.tensor.matmul(
                            ps_o[:, n0:n1],
                            ytf[:, hp, 128 * g : 128 * (g + 1)],
                            wp_sb[:, hp, n0:n1],
                            start=(hp == 0),
                            stop=(hp == HP - 1),
                        )
                yo = osb.tile([128, C], F32, name="yo", tag="yo")
                nc.vector.tensor_add(yo, ps_o, bp_bc)
                nc.sync.dma_start(out=y[128 * g : 128 * (g + 1), :], in_=yo)


# ---------------------------------------------------------------------------
# host side
# ---------------------------------------------------------------------------


def _hilo(a):
    hi = np.asarray(a, NPE4)
    lo = np.asarray(a - hi.astype(np.float32), NPE4)
    return hi, lo


def _ileave4(W4):
    """[NCT, 128, G, M] -> interleaved [128, NCT/2, G, 2M] walrus layout."""
    A = W4[0::2]  # [NJP, 128, G, M]
    B = W4[1::2]
    il = np.empty(A.shape[:3] + (2 * A.shape[3],), A.dtype)
    il[..., 0::2] = A[..., ::-1]
    il[..., 1::2] = B[..., ::-1]
    return np.ascontiguousarray(il.transpose(1, 0, 2, 3))


def make_in_maps(x, w_attn, b_attn, w_proj, b_proj, cfg=CFG):
    T, C, H, HP, NCT = cfg.T, cfg.C, cfg.H, cfg.HP, cfg.NCT
    xT = np.ascontiguousarray(x.reshape(T, C).T).astype(np.float32)  # [C,T]
    xh, xl = _hilo(xT)

    w16 = (np.asarray(w_attn, np.float32)) * SW
    wq16, wk16, wv16 = w16[:, 0:C], w16[:, C : 2 * C], w16[:, 2 * C :]

    def wil_pair(wsec):
        h, l = _hilo(wsec)
        W4h = h.reshape(NCT, 128, HP, 128)
        W4l = l.reshape(NCT, 128, HP, 128)
        return _ileave4(W4h), _ileave4(W4l)

    wqilh, wqill = wil_pair(wq16)
    wkilh, wkill = wil_pair(wk16)

    # V moving operand: per-head reversed d order (so the strided interleaved
    # SBUF write runs with a positive stride)
    wvr = np.ascontiguousarray(
        wv16.reshape(C, H, 64)[:, :, ::-1].reshape(C, C)
    )
    wvrh, wvrl = _hilo(wvr)

    # V stationary: x k-tile pairs interleaved per 128-key tile
    X4h = xh.astype(np.float32).reshape(NCT, 128, 32, 128)
    X4l = xl.astype(np.float32).reshape(NCT, 128, 32, 128)
    xilh = _ileave4(X4h.astype(NPE4))
    xill = _ileave4(X4l.astype(NPE4))

    wP = np.asarray(w_proj, np.float32).astype(NPBF16)
    bq = np.ascontiguousarray(np.asarray(b_attn[0:C], np.float32) * SW)
    bk = np.ascontiguousarray(np.asarray(b_attn[C : 2 * C], np.float32) * SW)
    # V bias folded into the output projection (exact)
    bP = np.ascontiguousarray(
        np.asarray(b_proj, np.float32)
        + np.asarray(b_attn[2 * C :], np.float32) @ np.asarray(w_proj, np.float32)
    )

    jl = np.arange(128)[:, None]
    ii = np.arange(128)[None, :]
    in_maps = []
    for c in range(cfg.ncores):
        colsh = np.concatenate(
            [xh[:, 128 * t : 128 * (t + 1)] for t in cfg.qtiles(c)], axis=1
        )
        colsl = np.concatenate(
            [xl[:, 128 * t : 128 * (t + 1)] for t in cfg.qtiles(c)], axis=1
        )
        # multiplicative {0,1} masks on the fp8 P slabs, per key-block residue
        masks = np.stack(
            [(jl - ii <= 128 * (c - r)) for r in range(cfg.ncores)]
        ).astype(np.float32)
        maskq = np.ascontiguousarray(
            masks.transpose(1, 0, 2).reshape(128, cfg.ncores * 128)
        ).astype(NPBF16)
        in_maps.append(
            {
                "xh": xh,
                "xl": xl,
                "xilh": xilh,
                "xill": xill,
                "xqh": np.ascontiguousarray(colsh),
                "xql": np.ascontiguousarray(colsl),
                "wqilh": wqilh,
                "wqill": wqill,
                "wkilh": wkilh,
                "wkill": wkill,
                "wvrh": wvrh,
                "wvrl": wvrl,
                "wP": wP,
                "bq": bq,
                "bk": bk,
                "bP": bP,
                "maskq": maskq,
            }
        )
    return in_maps


def declare_io(nc, cfg=CFG):
    C, T, HP, NJP, QW = cfg.C, cfg.T, cfg.HP, cfg.NJP, cfg.QW
    dt = nc.dram_tensor
    ins = {
        "xh": dt("xh", [C, T], E4, kind="ExternalInput").ap(),
        "xl": dt("xl", [C, T], E4, kind="ExternalInput").ap(),
        "xilh": dt("xilh", [128, NJP, 32, 256], E4, kind="ExternalInput").ap(),
        "xill": dt("xill", [128, NJP, 32, 256], E4, kind="ExternalInput").ap(),
        "xqh": dt("xqh", [C, QW], E4, kind="ExternalInput").ap(),
        "xql": dt("xql", [C, QW], E4, kind="ExternalInput").ap(),
        "wqilh": dt("wqilh", [128, NJP, HP, 256], E4, kind="ExternalInput").ap(),
        "wqill": dt("wqill", [128, NJP, HP, 256], E4, kind="ExternalInput").ap(),
        "wkilh": dt("wkilh", [128, NJP, HP, 256], E4, kind="ExternalInput").ap(),
        "wkill": dt("wkill", [128, NJP, HP, 256], E4, kind="ExternalInput").ap(),
        "wvrh": dt("wvrh", [C, C], E4, kind="ExternalInput").ap(),
        "wvrl": dt("wvrl", [C, C], E4, kind="ExternalInput").ap(),
        "wP": dt("wP", [C, C], BF16, kind="ExternalInput").ap(),
        "bq": dt("bq", [C], F32, kind="ExternalInput").ap(),
        "bk": dt("bk", [C], F32, kind="ExternalInput").ap(),
        "bP": dt("bP", [C], F32, kind="ExternalInput").ap(),
        "maskq": dt("maskq", [128, cfg.ncores * 128], BF16,
                    kind="ExternalInput").ap(),
    }
    outs = {
        "y": dt("y", [QW, C], F32, kind="ExternalOutput").ap()
    }
    return ins, outs


def build_program(cfg=CFG, repeat=1):
    nc = bacc.Bacc("TRN2", target_bir_lowering=False, debug=False,
                   num_devices=cfg.ncores)
    ins, outs = declare_io(nc, cfg)
    with tile.TileContext(nc) as tc:
        for _ in range(repeat):
            build_kernel_v3(tc, outs, ins, cfg)
    nc.compile()
    return nc


def assemble_output(results, cfg=CFG):
    y = np.empty((cfg.T, cfg.C), np.float32)
    for c in range(cfg.ncores):
        yc = results[c]["y"]
        for g, t in enumerate(cfg.qtiles(c)):
            y[128 * t : 128 * (t + 1)] = yc[128 * g : 128 * (g + 1)]
    return y.reshape(1, cfg.T, cfg.C)


_PROGRAM = None


def kernel(x, w_attn, b_attn, w_proj, b_proj):
    global _PROGRAM
    cfg = CFG
    x = np.asarray(x, np.float32)
    if _PROGRAM is None:
        _PROGRAM = build_program(cfg)
    in_maps = make_in_maps(
        x, np.asarray(w_attn), np.asarray(b_attn), np.asarray(w_proj),
        np.asarray(b_proj), cfg
    )
    res = run_bass_kernel_spmd(_PROGRAM, in_maps, core_ids=list(range(cfg.ncores)))
    return assemble_output(res.results, cfg)


if __name__ == "__main__":
    import reference

    inputs = {k: np.asarray(v) for k, v in reference.setup_inputs().items()}
    out = kernel(**inputs)
    print("kernel output", out.shape, out.dtype)


# revision 65
# speedup vs baseline: 1.1443x; 1.0040x over previous
"""Causal self-attention (B=1, T=4096, C=768, H=12) on 8 TRN2 NeuronCores.

Strategy (single SPMD NEFF, no collectives):
  - Sequence-parallel over queries: core c owns q-tiles {c, c+8, c+16, c+24}
    (128 rows each, descending-extent column order). Slot s of every core
    processes key-blocks 8s..8s+7 (uniform instruction stream across cores);
    the true causal boundary is enforced by a per-core binary mask library
    passed as input data, so ONE program serves all 8 cores.
  - K/V/Q projections run as error-compensated fp8 DoubleRowSwInterleave
    matmuls: host splits x and 16*w_attn into e4m3 (hi, lo) pairs and the
    kernel computes xh*wh + xh*wl + xl*wh (the lo*lo term is negligible).
    Each DRI matmul contracts TWO 128-row k-tiles per pass at 0.5 cyc/row,
    so the 9-matmul group costs 0.75x the bf16 equivalent with bf16-class
    accuracy (measured end-to-end rel err 3.4e-3 for the projections).
  - Attention scores stay transposed: S^T = K @ Q^T with keys on partitions;
    exp runs PSUM->SBUF on ScalarE with scale 1/2048 (the 16x weight
    prescale squares into S) and bias -2 so exp output fits fp8e4 range.
  - P^T is written as fp8e4; PV uses DRI pairing two CONSECUTIVE KEY BLOCKS
    per pass (keys are the contraction dim), with V stored as interleaved
    fp8 (hi, lo) stationaries: y = P*vh + P*vl keeps v at bf16-class
    precision while PV runs at 2x bf16 speed. The V bias is folded into an
    effective output-projection bias on the host (exact).
  - K/V live in small rolling per-wave buffers (each wave's blocks are only
    read by that wave's attention). The softmax denominator falls out of a
    65th all-ones column of the padded-to-128 interleaved V stationary.
  - Per (head, wave): sweep 1 computes QK + exp + mask for all four block
    pairs (pt tiles buffered), sweep 2 fires the eight PV matmuls back to
    back so the PE never waits on a freshly produced mask; masks run 1/4 on
    DVE and 3/4 on GPSIMD to balance the elementwise queues.
  - Measured end-to-end relative error vs the fp32 reference: 1.2e-2
    (matching a numpy emulation of the same quantization points).
"""

import contextlib
from dataclasses import dataclass

import ml_dtypes
import numpy as np

import concourse.bass as bass
import concourse.mybir as mybir
import concourse.tile as tile
from concourse import bacc
from concourse.bass_utils import run_bass_kernel_spmd

BF16 = mybir.dt.bfloat16
F32 = mybir.dt.float32
E4 = mybir.dt.float8e4
NPBF16 = ml_dtypes.bfloat16
NPE4 = ml_dtypes.float8_e4m3
DRI = mybir.MatmulPerfMode.DoubleRowSwInterleave

SW = 16.0  # weight prescale (power of two: commutes with rounding)
EXP_SCALE = 1.0 / (8.0 * SW * SW)  # 1/(sqrt(D) * SW^2)
EXP_BIAS = -2.0  # keeps exp output within fp8e4 range; cancels in softmax


@dataclass(frozen=True)
class Cfg:
    T: int = 4096
    H: int = 12
    D: int = 64
    ncores: int = 8

    @property
    def C(self):
        return self.H * self.D

    @property
    def HP(self):  # head pairs
        return self.H // 2

    @property
    def NKB(self):  # 128-row key blocks
        return self.T // 128

    @property
    def NCH(self):  # 512-row key chunks
        return self.T // 512

    @property
    def QTC(self):  # q-tiles per core
        return self.T // 128 // self.ncores

    @property
    def QW(self):  # q columns per core
        return 128 * self.QTC

    @property
    def NCT(self):  # 128-row contraction tiles over C
        return self.C // 128

    @property
    def NJP(self):  # contraction k-tile pairs
        return self.NCT // 2

    def nb(self, b):  # valid q-column prefix width for key-block b
        return 128 * (self.QTC - b // self.ncores)

    def qtiles(self, c):  # global q-tile indices for core c, descending extent
        return [c + self.ncores * (self.QTC - 1 - g) for g in range(self.QTC)]


CFG = Cfg()


def build_kernel_v3(tc, outs, ins, cfg=CFG):
    nc = tc.nc
    C, H, HP, NJP = cfg.C, cfg.H, cfg.HP, cfg.NJP
    QW, NCH = cfg.QW, cfg.NCH
    Exp = mybir.ActivationFunctionType.Exp
    Ident = mybir.ActivationFunctionType.Identity

    xh, xl = ins["xh"], ins["xl"]
    xilh, xill = ins["xilh"], ins["xill"]
    xqh, xql = ins["xqh"], ins["xql"]
    wkilh, wkill = ins["wkilh"], ins["wkill"]
    wqilh, wqill = ins["wqilh"], ins["wqill"]
    wvrh, wvrl = ins["wvrh"], ins["wvrl"]
    wP = ins["wP"]
    bq_in, bk_in, bP_in = ins["bq"], ins["bk"], ins["bP"]
    maskq = ins["maskq"]
    y = outs["y"]

    stack = contextlib.ExitStack()
    with stack:
        persist = stack.enter_context(tc.tile_pool(name="persist", bufs=1))

        # rolling per-wave K^T (bf16, scaled 16x) and interleaved V (fp8 hi/lo)
        kt_roll = persist.tile([128, 2, HP, 1024], BF16, name="kt_roll")
        vh_roll = persist.tile([128, 2, 4, H, 256], E4, name="vh_roll")
        vl_roll = persist.tile([128, 2, 4, H, 256], E4, name="vl_roll")
        qt_t = persist.tile([128, HP, QW], BF16, name="qt_t")
        ytf = persist.tile([128, HP, QW], BF16, name="ytf")
        yacc = persist.tile([128, H, QW], F32, name="yacc")  # rows 0:65 used
        mask_sb = persist.tile([128, cfg.ncores * 128], BF16, name="mask_sb")
        wp_sb = persist.tile([128, cfg.NCT, C], BF16, name="wp_sb")
        wkh_sb = persist.tile([128, NJP, HP, 256], E4, name="wkh_sb")
        wkl_sb = persist.tile([128, NJP, HP, 256], E4, name="wkl_sb")
        wvh_sb = persist.tile([128, cfg.NCT, C], E4, name="wvh_sb")
        wvl_sb = persist.tile([128, cfg.NCT, C], E4, name="wvl_sb")
        bq_sb = persist.tile([128, HP], F32, name="bq_sb")
        bk_sb = persist.tile([128, HP], F32, name="bk_sb")
        bp_bc = persist.tile([128, C], F32, name="bp_bc")
        ones11 = persist.tile([1, 64], F32, name="ones11")
        ebias = persist.tile([128, 1], F32, name="ebias")

        nc.vector.memset(ebias, EXP_BIAS)
        nc.vector.memset(ones11, 1.0 / SW)
        # touch Exp early so the ACT table set loads during startup DMAs
        nc.scalar.activation(ones11, ones11, Exp, scale=0.0)
        nc.vector.memset(ones11, 1.0 / SW)
        # V stationaries: zero the pad region once (gpsimd memset); set the
        # ones column (logical col 64 of 128 -> interleaved positions 126-127).
        vh4 = vh_roll.rearrange("p w q h (t two) -> p w q h t two", two=2)
        vl4 = vl_roll.rearrange("p w q h (t two) -> p w q h t two", two=2)
        nc.gpsimd.memset(vh4[:, :, :, :, 0:63, :], 0.0)
        nc.gpsimd.memset(vl4[:, :, :, :, 0:64, :], 0.0)
        nc.vector.memset(vh4[:, :, :, :, 63:64, :], 1.0)

        with (
            tc.tile_pool(name="xpool", bufs=3) as xpool,
            tc.tile_pool(name="pkv", bufs=2, space="PSUM") as pkv,
            tc.tile_pool(name="aps", bufs=2, space="PSUM") as aps,
            tc.tile_pool(name="pvp", bufs=2, space="PSUM") as pvp,
            tc.tile_pool(name="ptp", bufs=6) as ptp,
            tc.tile_pool(name="nrm", bufs=1) as nrm,
        ):
            qproj = tc.alloc_tile_pool(name="qproj", bufs=1)

            xhr = xh.rearrange("(j p) t -> p j t", p=128)
            xlr = xl.rearrange("(j p) t -> p j t", p=128)

            def load_xch(ch):
                th = xpool.tile([128, cfg.NCT, 512], E4, name="xch_h", tag="xh")
                tl = xpool.tile([128, cfg.NCT, 512], E4, name="xch_l", tag="xl")
                tih = xpool.tile([128, NJP, 4, 256], E4, name="xil_h", tag="xih")
                til = xpool.tile([128, NJP, 4, 256], E4, name="xil_l", tag="xil")
                nc.sync.dma_start(out=th, in_=xhr[:, :, 512 * ch : 512 * (ch + 1)])
                nc.sync.dma_start(out=tl, in_=xlr[:, :, 512 * ch : 512 * (ch + 1)])
                nc.sync.dma_start(out=tih, in_=xilh[:, :, 4 * ch : 4 * ch + 4, :])
                nc.sync.dma_start(out=til, in_=xill[:, :, 4 * ch : 4 * ch + 4, :])
                return th, tl, tih, til

            # startup DMA order: first x chunk, K weights, V weights, masks,
            # Q inputs - so the PE never waits on a cold queue
            nc.sync.dma_start(out=bq_sb,
                              in_=bq_in.rearrange("(hp p) -> p hp", p=128))
            nc.sync.dma_start(out=bk_sb,
                              in_=bk_in.rearrange("(hp p) -> p hp", p=128))
            nc.sync.dma_start(out=wkh_sb, in_=wkilh)
            nc.sync.dma_start(out=wkl_sb, in_=wkill)
            xch_pre = {0: load_xch(0)}
            xch_pre[1] = load_xch(1)
            nc.sync.dma_start(
                out=wvh_sb, in_=wvrh.rearrange("(j p) t -> p j t", p=128)
            )
            nc.sync.dma_start(
                out=wvl_sb, in_=wvrl.rearrange("(j p) t -> p j t", p=128)
            )
            nc.sync.dma_start(out=mask_sb, in_=maskq)
            wqh_sb = qproj.tile([128, NJP, HP, 256], E4, name="wqh_sb")
            wql_sb = qproj.tile([128, NJP, HP, 256], E4, name="wql_sb")
            xqh_sb = qproj.tile([128, cfg.NCT, QW], E4, name="xqh_sb")
            xql_sb = qproj.tile([128, cfg.NCT, QW], E4, name="xql_sb")
            nc.sync.dma_start(out=wqh_sb, in_=wqilh)
            nc.sync.dma_start(out=wql_sb, in_=wqill)
            nc.sync.dma_start(out=xqh_sb, in_=xqh.rearrange("(j p) t -> p j t", p=128))
            nc.sync.dma_start(out=xql_sb, in_=xql.rearrange("(j p) t -> p j t", p=128))

            def comp_dri(ps, wil_h, wil_l, xp_h, xp_l, n0=None, n1=None):
                """9-term compensated DRI group into `ps`.

                wil_*: callables j -> stationary AP [128, 2*M interleaved]
                xp_*: callables j -> moving AP [128, 2, N]
                """
                terms = [(wil_h, xp_h), (wil_l, xp_h), (wil_h, xp_l)]
                nmm = 0
                for wf, xf in terms:
                    for j in range(NJP):
                        nc.tensor.matmul(
                            ps,
                            wf(j).rearrange("p (m two) -> p m two", two=2),
                            xf(j),
                            start=(nmm == 0),
                            stop=(nmm == 3 * NJP - 1),
                            perf_mode=DRI,
                        )
                        nmm += 1

            for cp in range(NCH // 2):
                par = cp % 2
                first, last = cp == 0, cp == NCH // 2 - 1
                chunks = (2 * cp, 2 * cp + 1)
                # ---- project K^T / V for this wave's two chunks ------------
                for half, ch in enumerate(chunks):
                    th, tl, tih, til = (
                        xch_pre.pop(ch) if ch in xch_pre else load_xch(ch)
                    )
                    sched = [("k", hp) for hp in range(HP)] + [
                        ("v", (tt, nn)) for tt in range(4) for nn in range(2)
                    ]
                    for kind, item in sched:
                      if kind == "k":
                        hp = item
                        ps_k = pkv.tile([128, 512], F32, name="ps_k", tag="pkv")
                        comp_dri(
                            ps_k,
                            lambda j, hp=hp: wkh_sb[:, j, hp, :],
                            lambda j, hp=hp: wkl_sb[:, j, hp, :],
                            lambda j: th[:, 2 * j : 2 * j + 2, :],
                            lambda j: tl[:, 2 * j : 2 * j + 2, :],
                        )
                        nc.vector.tensor_scalar_add(
                            kt_roll[:, par, hp, 512 * half : 512 * (half + 1)],
                            ps_k,
                            bk_sb[:, hp : hp + 1],
                        )
                      else:
                        tt, nn = item
                        pi = 2 * half + tt // 2  # pair index in wave
                        pb = tt % 2  # block within pair
                        for n0, n1 in (((0, 384),) if nn == 0 else ((384, 768),)):
                            h0, h1 = n0 // 64, n1 // 64
                            ps_v = pkv.tile([128, 384], F32, name="ps_v", tag="pkv")
                            nmm = 0
                            for xf, wf in (
                                (tih, wvh_sb),
                                (tih, wvl_sb),
                                (til, wvh_sb),
                            ):
                                for j in range(NJP):
                                    nc.tensor.matmul(
                                        ps_v,
                                        xf[:, j, tt, :].rearrange(
                                            "p (m two) -> p m two", two=2
                                        ),
                                        wf[:, 2 * j : 2 * j + 2, n0:n1],
                                        start=(nmm == 0),
                                        stop=(nmm == 3 * NJP - 1),
                                        perf_mode=DRI,
                                    )
                                    nmm += 1
                            # v_hi = e4m3(v); v_lo = v - v_hi (bias folded into
                            # the output projection host-side)
                            psr = ps_v.rearrange("p (h e) -> p h e", e=64)
                            vh4w = vh_roll.rearrange(
                                "p w q h (t two) -> p w q h t two", two=2
                            )[:, par, pi, h0:h1, 64:128, pb]
                            vl4w = vl_roll.rearrange(
                                "p w q h (t two) -> p w q h t two", two=2
                            )[:, par, pi, h0:h1, 64:128, pb]
                            nc.vector.tensor_copy(vh4w, psr)
                            nc.vector.tensor_sub(vl4w, psr, vh4w)

                if cp == min(1, NCH // 2 - 1):
                    # prefetch output-projection weights mid-loop
                    for ct in range(cfg.NCT):
                        nc.sync.dma_start(
                            out=wp_sb[:, ct, :],
                            in_=wP[128 * ct : 128 * (ct + 1), :],
                        )
                    bp_src = bass.AP(
                        tensor=bP_in.tensor, offset=bP_in.offset, ap=[[0, 128], [1, C]]
                    )
                    nc.gpsimd.dma_start(out=bp_bc, in_=bp_src)
                if cp == 0:
                    # Q^T projection - emitted here so the PE chews K/V
                    # projection first while the Q inputs stream in
                    for hp in range(HP):
                        ps_q = pvp.tile([128, QW], F32, name="ps_q", tag="ps_y")
                        comp_dri(
                            ps_q,
                            lambda j, hp=hp: wqh_sb[:, j, hp, :],
                            lambda j, hp=hp: wql_sb[:, j, hp, :],
                            lambda j: xqh_sb[:, 2 * j : 2 * j + 2, :],
                            lambda j: xql_sb[:, 2 * j : 2 * j + 2, :],
                        )
                        nc.scalar.activation(
                            qt_t[:, hp, :], ps_q, Ident, bias=bq_sb[:, hp : hp + 1]
                        )
                    qproj.release()

                # ---- attention for this wave's 8 key-blocks ----------------
                nA = cfg.nb(4 * chunks[0])  # widths per half-wave
                for hp in range(HP):
                    for h in range(2):
                        hd = 2 * hp + h
                        ps_y = pvp.tile([128, 512], F32, name="ps_y", tag="ps_y")
                        # sweep 1: QK + exp + mask for all four pairs (pt
                        # tiles held); sweep 2: all eight PV matmuls back to
                        # back - PV never waits on a freshly computed mask
                        ptl = []
                        if True:
                          for pi in range(4):
                            half = pi // 2
                            ch = chunks[half]
                            pl = pi % 2  # pair within the half-wave
                            ba = 4 * ch + 2 * pl
                            n = cfg.nb(ba)
                            pt = ptp.tile([128, 1024], E4, name=f"pt{h}",
                                          tag=f"pt{h}")
                            # pair layout: blocks at offsets 0 and 512 in both
                            # the 2-bank score tile and pt
                            sps = aps.tile([128, 1024], F32, name="sps",
                                           tag="sps")
                            blkv = pt.rearrange("p (b n) -> p b n", n=512)[
                                :, :, 0:n
                            ]
                            for pb in (0, 1):
                                bw = 4 * half + 2 * pl + pb  # kt_roll block
                                nc.tensor.matmul(
                                    sps[:, 512 * pb : 512 * pb + n],
                                    kt_roll[64 * h : 64 * (h + 1), par, hp,
                                            128 * bw : 128 * (bw + 1)],
                                    qt_t[64 * h : 64 * (h + 1), hp, 0:n],
                                    start=True,
                                    stop=True,
                                )
                            spsv = sps.rearrange("p (b n) -> p b n", n=512)
                            nc.scalar.activation(
                                blkv, spsv[:, :, 0:n],
                                Exp, scale=EXP_SCALE, bias=ebias,
                            )
                            # causal boundary: mask last 128 q-cols of each blk
                            r0 = ba % cfg.ncores
                            pts = blkv[:, :, n - 128 : n]
                            msk = mask_sb[:, 128 * r0 : 128 * (r0 + 2)].rearrange(
                                "p (b n) -> p b n", n=128
                            )
                            meng = nc.vector if pi == 0 else nc.gpsimd
                            meng.tensor_mul(pts, pts, msk)
                            ptl.append((pi, n, blkv))
                        for pi, n, blkv in ptl:
                            # PV: two DRI matmuls (v_hi, v_lo), contraction
                            # over both blocks of the pair
                            for vroll in (vh_roll, vl_roll):
                                nc.tensor.matmul(
                                    ps_y[:, 0:n],
                                    vroll[:, par, pi, hd, :].rearrange(
                                        "p (m two) -> p m two", two=2
                                    ),
                                    blkv,
                                    start=(pi == 0 and vroll is vh_roll),
                                    stop=(pi == 3 and vroll is vl_roll),
                                    perf_mode=DRI,
                                    skip_group_check=True,
                                )
                        if first:
                            nc.vector.tensor_copy(
                                yacc[0:65, hd, 0:nA], ps_y[0:65, 0:nA]
                            )
                        else:
                            nc.vector.tensor_add(
                                yacc[0:65, hd, 0:nA],
                                yacc[0:65, hd, 0:nA],
                                ps_y[0:65, 0:nA],
                            )
                        if last:
                            # normalize this head now - overlaps the
                            # remaining heads' attention
                            rec = nrm.tile([1, QW], F32, name="rec", tag="rec")
                            rc_ps = pkv.tile([64, QW], F32, name="rc_ps",
                                             tag="pkv")
                            nc.vector.reciprocal(rec, yacc[64:65, hd, :])
                            nc.tensor.matmul(
                                rc_ps, ones11[0:1, :], rec, start=True, stop=True
                            )
                            nc.vector.tensor_mul(
                                ytf[64 * h : 64 * (h + 1), hp, :],
                                yacc[0:64, hd, :], rc_ps,
                            )

        # ---- output projection -------------------------------------------
        with (
            tc.tile_pool(name="ops", bufs=2, space="PSUM") as ops,
            tc.tile_pool(name="osb", bufs=2) as osb,
        ):
            for g in range(cfg.QTC):
                ps_o = ops.tile([128, C], F32, name="ps_o", tag="ps_o")
                for n0, n1 in ((0, 512), (512, C)):
                    for hp in range(HP):
                        nc.tensor.matmul(
                            ps_o[:, n0:n1],
                            ytf[:, hp, 128 * g : 128 * (g + 1)],
                            wp_sb[:, hp, n0:n1],
                            start=(hp == 0),
                            stop=(hp == HP - 1),
                        )
                yo = osb.tile([128, C], F32, name="yo", tag="yo")
                nc.vector.tensor_add(yo, ps_o, bp_bc)
                nc.sync.dma_start(out=y[128 * g : 128 * (g + 1), :], in_=yo)


# ---------------------------------------------------------------------------
# host side
# ---------------------------------------------------------------------------


def _hilo(a):
    hi = np.asarray(a, NPE4)
    lo = np.asarray(a - hi.astype(np.float32), NPE4)
    return hi, lo


def _ileave4(W4):
    """[NCT, 128, G, M] -> interleaved [128, NCT/2, G, 2M] walrus layout."""
    A = W4[0::2]  # [NJP, 128, G, M]
    B = W4[1::2]
    il = np.empty(A.shape[:3] + (2 * A.shape[3],), A.dtype)
    il[..., 0::2] = A[..., ::-1]
    il[..., 1::2] = B[..., ::-1]
    return np.ascontiguousarray(il.transpose(1, 0, 2, 3))


def make_in_maps(x, w_attn, b_attn, w_proj, b_proj, cfg=CFG):
    T, C, H, HP, NCT = cfg.T, cfg.C, cfg.H, cfg.HP, cfg.NCT
    xT = np.ascontiguousarray(x.reshape(T, C).T).astype(np.float32)  # [C,T]
    xh, xl = _hilo(xT)

    w16 = (np.asarray(w_attn, np.float32)) * SW
    wq16, wk16, wv16 = w16[:, 0:C], w16[:, C : 2 * C], w16[:, 2 * C :]

    def wil_pair(wsec):
        h, l = _hilo(wsec)
        W4h = h.reshape(NCT, 128, HP, 128)
        W4l = l.reshape(NCT, 128, HP, 128)
        return _ileave4(W4h), _ileave4(W4l)

    wqilh, wqill = wil_pair(wq16)
    wkilh, wkill = wil_pair(wk16)

    # V moving operand: per-head reversed d order (so the strided interleaved
    # SBUF write runs with a positive stride)
    wvr = np.ascontiguousarray(
        wv16.reshape(C, H, 64)[:, :, ::-1].reshape(C, C)
    )
    wvrh, wvrl = _hilo(wvr)

    # V stationary: x k-tile pairs interleaved per 128-key tile
    X4h = xh.astype(np.float32).reshape(NCT, 128, 32, 128)
    X4l = xl.astype(np.float32).reshape(NCT, 128, 32, 128)
    xilh = _ileave4(X4h.astype(NPE4))
    xill = _ileave4(X4l.astype(NPE4))

    wP = np.asarray(w_proj, np.float32).astype(NPBF16)
    bq = np.ascontiguousarray(np.asarray(b_attn[0:C], np.float32) * SW)
    bk = np.ascontiguousarray(np.asarray(b_attn[C : 2 * C], np.float32) * SW)
    # V bias folded into the output projection (exact)
    bP = np.ascontiguousarray(
        np.asarray(b_proj, np.float32)
        + np.asarray(b_attn[2 * C :], np.float32) @ np.asarray(w_proj, np.float32)
    )

    jl = np.arange(128)[:, None]
    ii = np.arange(128)[None, :]
    in_maps = []
    for c in range(cfg.ncores):
        colsh = np.concatenate(
            [xh[:, 128 * t : 128 * (t + 1)] for t in cfg.qtiles(c)], axis=1
        )
        colsl = np.concatenate(
            [xl[:, 128 * t : 128 * (t + 1)] for t in cfg.qtiles(c)], axis=1
        )
        # multiplicative {0,1} masks on the fp8 P slabs, per key-block residue
        masks = np.stack(
            [(jl - ii <= 128 * (c - r)) for r in range(cfg.ncores)]
        ).astype(np.float32)
        maskq = np.ascontiguousarray(
            masks.transpose(1, 0, 2).reshape(128, cfg.ncores * 128)
        ).astype(NPBF16)
        in_maps.append(
            {
                "xh": xh,
                "xl": xl,
                "xilh": xilh,
                "xill": xill,
                "xqh": np.ascontiguousarray(colsh),
                "xql": np.ascontiguousarray(colsl),
                "wqilh": wqilh,
                "wqill": wqill,
                "wkilh": wkilh,
                "wkill": wkill,
                "wvrh": wvrh,
                "wvrl": wvrl,
                "wP": wP,
                "bq": bq,
                "bk": bk,
                "bP": bP,
                "maskq": maskq,
            }
        )
    return in_maps


def declare_io(nc, cfg=CFG):
    C, T, HP, NJP, QW = cfg.C, cfg.T, cfg.HP, cfg.NJP, cfg.QW
    dt = nc.dram_tensor
    ins = {
        "xh": dt("xh", [C, T], E4, kind="ExternalInput").ap(),
        "xl": dt("xl", [C, T], E4, kind="ExternalInput").ap(),
        "xilh": dt("xilh", [128, NJP, 32, 256], E4, kind="ExternalInput").ap(),
        "xill": dt("xill", [128, NJP, 32, 256], E4, kind="ExternalInput").ap(),
        "xqh": dt("xqh", [C, QW], E4, kind="ExternalInput").ap(),
        "xql": dt("xql", [C, QW], E4, kind="ExternalInput").ap(),
        "wqilh": dt("wqilh", [128, NJP, HP, 256], E4, kind="ExternalInput").ap(),
        "wqill": dt("wqill", [128, NJP, HP, 256], E4, kind="ExternalInput").ap(),
        "wkilh": dt("wkilh", [128, NJP, HP, 256], E4, kind="ExternalInput").ap(),
        "wkill": dt("wkill", [128, NJP, HP, 256], E4, kind="ExternalInput").ap(),
        "wvrh": dt("wvrh", [C, C], E4, kind="ExternalInput").ap(),
        "wvrl": dt("wvrl", [C, C], E4, kind="ExternalInput").ap(),
        "wP": dt("wP", [C, C], BF16, kind="ExternalInput").ap(),
        "bq": dt("bq", [C], F32, kind="ExternalInput").ap(),
        "bk": dt("bk", [C], F32, kind="ExternalInput").ap(),
        "bP": dt("bP", [C], F32, kind="ExternalInput").ap(),
        "maskq": dt("maskq", [128, cfg.ncores * 128], BF16,
                    kind="ExternalInput").ap(),
    }
    outs = {
        "y": dt("y", [QW, C], F32, kind="ExternalOutput").ap()
    }
    return ins, outs


def build_program(cfg=CFG, repeat=1):
    nc = bacc.Bacc("TRN2", target_bir_lowering=False, debug=False,
                   num_devices=cfg.ncores)
    ins, outs = declare_io(nc, cfg)
    with tile.TileContext(nc) as tc:
        for _ in range(repeat):
            build_kernel_v3(tc, outs, ins, cfg)
    nc.compile()
    return nc


def assemble_output(results, cfg=CFG):
    y = np.empty((cfg.T, cfg.C), np.float32)
    for c in range(cfg.ncores):
        yc = results[c]["y"]
        for g, t in enumerate(cfg.qtiles(c)):
            y[128 * t : 128 * (t + 1)] = yc[128 * g : 128 * (g + 1)]
    return y.reshape(1, cfg.T, cfg.C)


_PROGRAM = None


def kernel(x, w_attn, b_attn, w_proj, b_proj):
    global _PROGRAM
    cfg = CFG
    x = np.asarray(x, np.float32)
    if _PROGRAM is None:
        _PROGRAM = build_program(cfg)
    in_maps = make_in_maps(
        x, np.asarray(w_attn), np.asarray(b_attn), np.asarray(w_proj),
        np.asarray(b_proj), cfg
    )
    res = run_bass_kernel_spmd(_PROGRAM, in_maps, core_ids=list(range(cfg.ncores)))
    return assemble_output(res.results, cfg)


if __name__ == "__main__":
    import reference

    inputs = {k: np.asarray(v) for k, v in reference.setup_inputs().items()}
    out = kernel(**inputs)
    print("kernel output", out.shape, out.dtype)


# revision 68
# speedup vs baseline: 1.1570x; 1.0111x over previous
"""Causal self-attention (B=1, T=4096, C=768, H=12) on 8 TRN2 NeuronCores.

Strategy (single SPMD NEFF, no collectives):
  - Sequence-parallel over queries: core c owns q-tiles {c, c+8, c+16, c+24}
    (128 rows each, descending-extent column order). Slot s of every core
    processes key-blocks 8s..8s+7 (uniform instruction stream across cores);
    the true causal boundary is enforced by a per-core binary mask library
    passed as input data, so ONE program serves all 8 cores.
  - K/V/Q projections run as error-compensated fp8 DoubleRowSwInterleave
    matmuls: host splits x and 16*w_attn into e4m3 (hi, lo) pairs and the
    kernel computes xh*wh + xh*wl + xl*wh (the lo*lo term is negligible).
    Each DRI matmul contracts TWO 128-row k-tiles per pass at 0.5 cyc/row,
    so the 9-matmul group costs 0.75x the bf16 equivalent with bf16-class
    accuracy (measured end-to-end rel err 3.4e-3 for the projections).
  - Attention scores stay transposed: S^T = K @ Q^T with keys on partitions;
    exp runs PSUM->SBUF on ScalarE with scale 1/2048 (the 16x weight
    prescale squares into S) and bias -2 so exp output fits fp8e4 range.
  - P^T is written as fp8e4; PV uses DRI pairing two CONSECUTIVE KEY BLOCKS
    per pass (keys are the contraction dim), with V stored as interleaved
    fp8 (hi, lo) stationaries: y = P*vh + P*vl keeps v at bf16-class
    precision while PV runs at 2x bf16 speed. The V bias is folded into an
    effective output-projection bias on the host (exact).
  - K/V live in small rolling per-wave buffers (each wave's blocks are only
    read by that wave's attention). The softmax denominator falls out of a
    65th all-ones column of the padded-to-128 interleaved V stationary.
  - Per (head, wave): sweep 1 computes QK + exp + mask for all four block
    pairs (pt tiles buffered), sweep 2 fires the eight PV matmuls back to
    back so the PE never waits on a freshly produced mask; masks run 1/4 on
    DVE and 3/4 on GPSIMD to balance the elementwise queues.
  - Measured end-to-end relative error vs the fp32 reference: 1.2e-2
    (matching a numpy emulation of the same quantization points).
"""

import contextlib
from dataclasses import dataclass

import ml_dtypes
import numpy as np

import concourse.bass as bass
import concourse.mybir as mybir
import concourse.tile as tile
from concourse import bacc
from concourse.bass_utils import run_bass_kernel_spmd

BF16 = mybir.dt.bfloat16
F32 = mybir.dt.float32
E4 = mybir.dt.float8e4
NPBF16 = ml_dtypes.bfloat16
NPE4 = ml_dtypes.float8_e4m3
DRI = mybir.MatmulPerfMode.DoubleRowSwInterleave

SW = 16.0  # weight prescale (power of two: commutes with rounding)
EXP_SCALE = 1.0 / (8.0 * SW * SW)  # 1/(sqrt(D) * SW^2)
EXP_BIAS = -2.0  # keeps exp output within fp8e4 range; cancels in softmax


@dataclass(frozen=True)
class Cfg:
    T: int = 4096
    H: int = 12
    D: int = 64
    ncores: int = 8

    @property
    def C(self):
        return self.H * self.D

    @property
    def HP(self):  # head pairs
        return self.H // 2

    @property
    def NKB(self):  # 128-row key blocks
        return self.T // 128

    @property
    def NCH(self):  # 512-row key chunks
        return self.T // 512

    @property
    def QTC(self):  # q-tiles per core
        return self.T // 128 // self.ncores

    @property
    def QW(self):  # q columns per core
        return 128 * self.QTC

    @property
    def NCT(self):  # 128-row contraction tiles over C
        return self.C // 128

    @property
    def NJP(self):  # contraction k-tile pairs
        return self.NCT // 2

    def nb(self, b):  # valid q-column prefix width for key-block b
        return 128 * (self.QTC - b // self.ncores)

    def qtiles(self, c):  # global q-tile indices for core c, descending extent
        return [c + self.ncores * (self.QTC - 1 - g) for g in range(self.QTC)]


CFG = Cfg()


def build_kernel_v3(tc, outs, ins, cfg=CFG):
    nc = tc.nc
    C, H, HP, NJP = cfg.C, cfg.H, cfg.HP, cfg.NJP
    QW, NCH = cfg.QW, cfg.NCH
    Exp = mybir.ActivationFunctionType.Exp
    Ident = mybir.ActivationFunctionType.Identity

    xh, xl = ins["xh"], ins["xl"]
    xilh, xill = ins["xilh"], ins["xill"]
    xqh, xql = ins["xqh"], ins["xql"]
    wkilh, wkill = ins["wkilh"], ins["wkill"]
    wqilh, wqill = ins["wqilh"], ins["wqill"]
    wvrh, wvrl = ins["wvrh"], ins["wvrl"]
    wP = ins["wP"]
    bq_in, bk_in, bP_in = ins["bq"], ins["bk"], ins["bP"]
    maskq = ins["maskq"]
    y = outs["y"]

    stack = contextlib.ExitStack()
    with stack:
        persist = stack.enter_context(tc.tile_pool(name="persist", bufs=1))

        # rolling per-wave K^T (bf16, scaled 16x) and interleaved V (fp8 hi/lo)
        kt_roll = persist.tile([128, 2, HP, 1024], BF16, name="kt_roll")
        vh_roll = persist.tile([128, 2, 4, H, 256], E4, name="vh_roll")
        vl_roll = persist.tile([128, 2, 4, H, 256], E4, name="vl_roll")
        qt_t = persist.tile([128, HP, QW], BF16, name="qt_t")
        ytf = persist.tile([128, HP, QW], BF16, name="ytf")
        yacc = persist.tile([128, H, QW], F32, name="yacc")  # rows 0:65 used
        mask_sb = persist.tile([128, cfg.ncores * 128], BF16, name="mask_sb")
        wp_sb = persist.tile([128, cfg.NCT, C], BF16, name="wp_sb")
        wkh_sb = persist.tile([128, NJP, HP, 256], E4, name="wkh_sb")
        wkl_sb = persist.tile([128, NJP, HP, 256], E4, name="wkl_sb")
        wvh_sb = persist.tile([128, cfg.NCT, C], E4, name="wvh_sb")
        wvl_sb = persist.tile([128, cfg.NCT, C], E4, name="wvl_sb")
        bq_sb = persist.tile([128, HP], F32, name="bq_sb")
        bk_sb = persist.tile([128, HP], F32, name="bk_sb")
        bp_bc = persist.tile([128, C], F32, name="bp_bc")
        ones11 = persist.tile([1, 64], F32, name="ones11")
        ebias = persist.tile([128, 1], F32, name="ebias")

        nc.vector.memset(ebias, EXP_BIAS)
        nc.vector.memset(ones11, 1.0 / SW)
        # touch Exp early so the ACT table set loads during startup DMAs
        nc.scalar.activation(ones11, ones11, Exp, scale=0.0)
        nc.vector.memset(ones11, 1.0 / SW)
        # V stationaries: zero the pad region once (gpsimd memset); set the
        # ones column (logical col 64 of 128 -> interleaved positions 126-127).
        vh4 = vh_roll.rearrange("p w q h (t two) -> p w q h t two", two=2)
        vl4 = vl_roll.rearrange("p w q h (t two) -> p w q h t two", two=2)
        nc.gpsimd.memset(vh4[:, :, :, :, 0:63, :], 0.0)
        nc.gpsimd.memset(vl4[:, :, :, :, 0:64, :], 0.0)
        nc.vector.memset(vh4[:, :, :, :, 63:64, :], 1.0)

        with (
            tc.tile_pool(name="xpool", bufs=3) as xpool,
            tc.tile_pool(name="pkv", bufs=2, space="PSUM") as pkv,
            tc.tile_pool(name="aps", bufs=2, space="PSUM") as aps,
            tc.tile_pool(name="pvp", bufs=2, space="PSUM") as pvp,
            tc.tile_pool(name="ptp", bufs=6) as ptp,
            tc.tile_pool(name="nrm", bufs=1) as nrm,
        ):
            qproj = tc.alloc_tile_pool(name="qproj", bufs=1)

            xhr = xh.rearrange("(j p) t -> p j t", p=128)
            xlr = xl.rearrange("(j p) t -> p j t", p=128)

            def load_xch(ch, split=False):
                th = xpool.tile([128, cfg.NCT, 512], E4, name="xch_h", tag="xh")
                tl = xpool.tile([128, cfg.NCT, 512], E4, name="xch_l", tag="xl")
                tih = xpool.tile([128, NJP, 4, 256], E4, name="xil_h", tag="xih")
                til = xpool.tile([128, NJP, 4, 256], E4, name="xil_l", tag="xil")
                cs = 512 * ch
                if split:
                    # ct-pair pieces: the first projection group consumes
                    # pair j as soon as piece j lands
                    for j in range(NJP):
                        nc.sync.dma_start(
                            out=th[:, 2 * j : 2 * j + 2, :],
                            in_=xhr[:, 2 * j : 2 * j + 2, cs : cs + 512],
                        )
                        nc.sync.dma_start(
                            out=tl[:, 2 * j : 2 * j + 2, :],
                            in_=xlr[:, 2 * j : 2 * j + 2, cs : cs + 512],
                        )
                else:
                    nc.sync.dma_start(out=th, in_=xhr[:, :, cs : cs + 512])
                    nc.sync.dma_start(out=tl, in_=xlr[:, :, cs : cs + 512])
                nc.sync.dma_start(out=tih, in_=xilh[:, :, 4 * ch : 4 * ch + 4, :])
                nc.sync.dma_start(out=til, in_=xill[:, :, 4 * ch : 4 * ch + 4, :])
                return th, tl, tih, til

            # startup DMA order: first x chunk, K weights, V weights, masks,
            # Q inputs - so the PE never waits on a cold queue
            nc.sync.dma_start(out=bq_sb,
                              in_=bq_in.rearrange("(hp p) -> p hp", p=128))
            nc.sync.dma_start(out=bk_sb,
                              in_=bk_in.rearrange("(hp p) -> p hp", p=128))
            nc.sync.dma_start(out=wkh_sb, in_=wkilh)
            nc.sync.dma_start(out=wkl_sb, in_=wkill)
            xch_pre = {0: load_xch(0, split=True)}
            nc.sync.dma_start(
                out=wvh_sb, in_=wvrh.rearrange("(j p) t -> p j t", p=128)
            )
            nc.sync.dma_start(
                out=wvl_sb, in_=wvrl.rearrange("(j p) t -> p j t", p=128)
            )
            xch_pre[1] = load_xch(1)
            nc.sync.dma_start(out=mask_sb, in_=maskq)
            wqh_sb = qproj.tile([128, NJP, HP, 256], E4, name="wqh_sb")
            wql_sb = qproj.tile([128, NJP, HP, 256], E4, name="wql_sb")
            xqh_sb = qproj.tile([128, cfg.NCT, QW], E4, name="xqh_sb")
            xql_sb = qproj.tile([128, cfg.NCT, QW], E4, name="xql_sb")
            nc.sync.dma_start(out=wqh_sb, in_=wqilh)
            nc.sync.dma_start(out=wql_sb, in_=wqill)
            nc.sync.dma_start(out=xqh_sb, in_=xqh.rearrange("(j p) t -> p j t", p=128))
            nc.sync.dma_start(out=xql_sb, in_=xql.rearrange("(j p) t -> p j t", p=128))

            def comp_dri(ps, wil_h, wil_l, xp_h, xp_l, n0=None, n1=None):
                """9-term compensated DRI group into `ps`.

                wil_*: callables j -> stationary AP [128, 2*M interleaved]
                xp_*: callables j -> moving AP [128, 2, N]
                """
                terms = [(wil_h, xp_h), (wil_l, xp_h), (wil_h, xp_l)]
                nmm = 0
                for wf, xf in terms:
                    for j in range(NJP):
                        nc.tensor.matmul(
                            ps,
                            wf(j).rearrange("p (m two) -> p m two", two=2),
                            xf(j),
                            start=(nmm == 0),
                            stop=(nmm == 3 * NJP - 1),
                            perf_mode=DRI,
                        )
                        nmm += 1

            for cp in range(NCH // 2):
                par = cp % 2
                first, last = cp == 0, cp == NCH // 2 - 1
                chunks = (2 * cp, 2 * cp + 1)
                # ---- project K^T / V for this wave's two chunks ------------
                for half, ch in enumerate(chunks):
                    th, tl, tih, til = (
                        xch_pre.pop(ch) if ch in xch_pre else load_xch(ch)
                    )
                    sched = [("k", hp) for hp in range(HP)] + [
                        ("v", (tt, nn)) for tt in range(4) for nn in range(2)
                    ]
                    for kind, item in sched:
                      if kind == "k":
                        hp = item
                        ps_k = pkv.tile([128, 512], F32, name="ps_k", tag="pkv")
                        comp_dri(
                            ps_k,
                            lambda j, hp=hp: wkh_sb[:, j, hp, :],
                            lambda j, hp=hp: wkl_sb[:, j, hp, :],
                            lambda j: th[:, 2 * j : 2 * j + 2, :],
                            lambda j: tl[:, 2 * j : 2 * j + 2, :],
                        )
                        nc.vector.tensor_scalar_add(
                            kt_roll[:, par, hp, 512 * half : 512 * (half + 1)],
                            ps_k,
                            bk_sb[:, hp : hp + 1],
                        )
                      else:
                        tt, nn = item
                        pi = 2 * half + tt // 2  # pair index in wave
                        pb = tt % 2  # block within pair
                        for n0, n1 in (((0, 384),) if nn == 0 else ((384, 768),)):
                            h0, h1 = n0 // 64, n1 // 64
                            ps_v = pkv.tile([128, 384], F32, name="ps_v", tag="pkv")
                            nmm = 0
                            for xf, wf in (
                                (tih, wvh_sb),
                                (tih, wvl_sb),
                                (til, wvh_sb),
                            ):
                                for j in range(NJP):
                                    nc.tensor.matmul(
                                        ps_v,
                                        xf[:, j, tt, :].rearrange(
                                            "p (m two) -> p m two", two=2
                                        ),
                                        wf[:, 2 * j : 2 * j + 2, n0:n1],
                                        start=(nmm == 0),
                                        stop=(nmm == 3 * NJP - 1),
                                        perf_mode=DRI,
                                    )
                                    nmm += 1
                            # v_hi = e4m3(v); v_lo = v - v_hi (bias folded into
                            # the output projection host-side)
                            psr = ps_v.rearrange("p (h e) -> p h e", e=64)
                            vh4w = vh_roll.rearrange(
                                "p w q h (t two) -> p w q h t two", two=2
                            )[:, par, pi, h0:h1, 64:128, pb]
                            vl4w = vl_roll.rearrange(
                                "p w q h (t two) -> p w q h t two", two=2
                            )[:, par, pi, h0:h1, 64:128, pb]
                            nc.vector.tensor_copy(vh4w, psr)
                            nc.vector.tensor_sub(vl4w, psr, vh4w)

                if cp == min(1, NCH // 2 - 1):
                    # prefetch output-projection weights mid-loop
                    for ct in range(cfg.NCT):
                        nc.sync.dma_start(
                            out=wp_sb[:, ct, :],
                            in_=wP[128 * ct : 128 * (ct + 1), :],
                        )
                    bp_src = bass.AP(
                        tensor=bP_in.tensor, offset=bP_in.offset, ap=[[0, 128], [1, C]]
                    )
                    nc.gpsimd.dma_start(out=bp_bc, in_=bp_src)
                if cp == 0:
                    # Q^T projection - emitted here so the PE chews K/V
                    # projection first while the Q inputs stream in
                    for hp in range(HP):
                        ps_q = pvp.tile([128, QW], F32, name="ps_q", tag="ps_y")
                        comp_dri(
                            ps_q,
                            lambda j, hp=hp: wqh_sb[:, j, hp, :],
                            lambda j, hp=hp: wql_sb[:, j, hp, :],
                            lambda j: xqh_sb[:, 2 * j : 2 * j + 2, :],
                            lambda j: xql_sb[:, 2 * j : 2 * j + 2, :],
                        )
                        nc.scalar.activation(
                            qt_t[:, hp, :], ps_q, Ident, bias=bq_sb[:, hp : hp + 1]
                        )
                    qproj.release()

                # ---- attention for this wave's 8 key-blocks ----------------
                nA = cfg.nb(4 * chunks[0])  # widths per half-wave
                for hp in range(HP):
                    for h in range(2):
                        hd = 2 * hp + h
                        ps_y = pvp.tile([128, 512], F32, name="ps_y", tag="ps_y")
                        # sweep 1: QK + exp + mask for all four pairs (pt
                        # tiles held); sweep 2: all eight PV matmuls back to
                        # back - PV never waits on a freshly computed mask
                        ptl = []
                        if True:
                          for pi in range(4):
                            half = pi // 2
                            ch = chunks[half]
                            pl = pi % 2  # pair within the half-wave
                            ba = 4 * ch + 2 * pl
                            n = cfg.nb(ba)
                            pt = ptp.tile([128, 1024], E4, name=f"pt{h}",
                                          tag=f"pt{h}")
                            # pair layout: blocks at offsets 0 and 512 in both
                            # the 2-bank score tile and pt
                            sps = aps.tile([128, 1024], F32, name="sps",
                                           tag="sps")
                            blkv = pt.rearrange("p (b n) -> p b n", n=512)[
                                :, :, 0:n
                            ]
                            for pb in (0, 1):
                                bw = 4 * half + 2 * pl + pb  # kt_roll block
                                nc.tensor.matmul(
                                    sps[:, 512 * pb : 512 * pb + n],
                                    kt_roll[64 * h : 64 * (h + 1), par, hp,
                                            128 * bw : 128 * (bw + 1)],
                                    qt_t[64 * h : 64 * (h + 1), hp, 0:n],
                                    start=True,
                                    stop=True,
                                )
                            spsv = sps.rearrange("p (b n) -> p b n", n=512)
                            nc.scalar.activation(
                                blkv, spsv[:, :, 0:n],
                                Exp, scale=EXP_SCALE, bias=ebias,
                            )
                            # causal boundary: mask last 128 q-cols of each blk
                            r0 = ba % cfg.ncores
                            pts = blkv[:, :, n - 128 : n]
                            msk = mask_sb[:, 128 * r0 : 128 * (r0 + 2)].rearrange(
                                "p (b n) -> p b n", n=128
                            )
                            meng = nc.vector if pi == 0 else nc.gpsimd
                            meng.tensor_mul(pts, pts, msk)
                            ptl.append((pi, n, blkv))
                        for pi, n, blkv in ptl:
                            # PV: two DRI matmuls (v_hi, v_lo), contraction
                            # over both blocks of the pair
                            for vroll in (vh_roll, vl_roll):
                                nc.tensor.matmul(
                                    ps_y[:, 0:n],
                                    vroll[:, par, pi, hd, :].rearrange(
                                        "p (m two) -> p m two", two=2
                                    ),
                                    blkv,
                                    start=(pi == 0 and vroll is vh_roll),
                                    stop=(pi == 3 and vroll is vl_roll),
                                    perf_mode=DRI,
                                    skip_group_check=True,
                                )
                        if first:
                            nc.vector.tensor_copy(
                                yacc[0:65, hd, 0:nA], ps_y[0:65, 0:nA]
                            )
                        else:
                            nc.vector.tensor_add(
                                yacc[0:65, hd, 0:nA],
                                yacc[0:65, hd, 0:nA],
                                ps_y[0:65, 0:nA],
                            )
                        if last:
                            # normalize this head now - overlaps the
                            # remaining heads' attention
                            rec = nrm.tile([1, QW], F32, name="rec", tag="rec")
                            rc_ps = pkv.tile([64, QW], F32, name="rc_ps",
                                             tag="pkv")
                            nc.vector.reciprocal(rec, yacc[64:65, hd, :])
                            nc.tensor.matmul(
                                rc_ps, ones11[0:1, :], rec, start=True, stop=True
                            )
                            nc.vector.tensor_mul(
                                ytf[64 * h : 64 * (h + 1), hp, :],
                                yacc[0:64, hd, :], rc_ps,
                            )

        # ---- output projection -------------------------------------------
        with (
            tc.tile_pool(name="ops", bufs=2, space="PSUM") as ops,
            tc.tile_pool(name="osb", bufs=2) as osb,
        ):
            for g in range(cfg.QTC):
                ps_o = ops.tile([128, C], F32, name="ps_o", tag="ps_o")
                for n0, n1 in ((0, 512), (512, C)):
                    for hp in range(HP):
                        nc.tensor.matmul(
                            ps_o[:, n0:n1],
                            ytf[:, hp, 128 * g : 128 * (g + 1)],
                            wp_sb[:, hp, n0:n1],
                            start=(hp == 0),
                            stop=(hp == HP - 1),
                        )
                yo = osb.tile([128, C], F32, name="yo", tag="yo")
                nc.vector.tensor_add(yo, ps_o, bp_bc)
                nc.sync.dma_start(out=y[128 * g : 128 * (g + 1), :], in_=yo)


# ---------------------------------------------------------------------------
# host side
# ---------------------------------------------------------------------------


def _hilo(a):
    hi = np.asarray(a, NPE4)
    lo = np.asarray(a - hi.astype(np.float32), NPE4)
    return hi, lo


def _ileave4(W4):
    """[NCT, 128, G, M] -> interleaved [128, NCT/2, G, 2M] walrus layout."""
    A = W4[0::2]  # [NJP, 128, G, M]
    B = W4[1::2]
    il = np.empty(A.shape[:3] + (2 * A.shape[3],), A.dtype)
    il[..., 0::2] = A[..., ::-1]
    il[..., 1::2] = B[..., ::-1]
    return np.ascontiguousarray(il.transpose(1, 0, 2, 3))


def make_in_maps(x, w_attn, b_attn, w_proj, b_proj, cfg=CFG):
    T, C, H, HP, NCT = cfg.T, cfg.C, cfg.H, cfg.HP, cfg.NCT
    xT = np.ascontiguousarray(x.reshape(T, C).T).astype(np.float32)  # [C,T]
    xh, xl = _hilo(xT)

    w16 = (np.asarray(w_attn, np.float32)) * SW
    wq16, wk16, wv16 = w16[:, 0:C], w16[:, C : 2 * C], w16[:, 2 * C :]

    def wil_pair(wsec):
        h, l = _hilo(wsec)
        W4h = h.reshape(NCT, 128, HP, 128)
        W4l = l.reshape(NCT, 128, HP, 128)
        return _ileave4(W4h), _ileave4(W4l)

    wqilh, wqill = wil_pair(wq16)
    wkilh, wkill = wil_pair(wk16)

    # V moving operand: per-head reversed d order (so the strided interleaved
    # SBUF write runs with a positive stride)
    wvr = np.ascontiguousarray(
        wv16.reshape(C, H, 64)[:, :, ::-1].reshape(C, C)
    )
    wvrh, wvrl = _hilo(wvr)

    # V stationary: x k-tile pairs interleaved per 128-key tile
    X4h = xh.astype(np.float32).reshape(NCT, 128, 32, 128)
    X4l = xl.astype(np.float32).reshape(NCT, 128, 32, 128)
    xilh = _ileave4(X4h.astype(NPE4))
    xill = _ileave4(X4l.astype(NPE4))

    wP = np.asarray(w_proj, np.float32).astype(NPBF16)
    bq = np.ascontiguousarray(np.asarray(b_attn[0:C], np.float32) * SW)
    bk = np.ascontiguousarray(np.asarray(b_attn[C : 2 * C], np.float32) * SW)
    # V bias folded into the output projection (exact)
    bP = np.ascontiguousarray(
        np.asarray(b_proj, np.float32)
        + np.asarray(b_attn[2 * C :], np.float32) @ np.asarray(w_proj, np.float32)
    )

    jl = np.arange(128)[:, None]
    ii = np.arange(128)[None, :]
    in_maps = []
    for c in range(cfg.ncores):
        colsh = np.concatenate(
            [xh[:, 128 * t : 128 * (t + 1)] for t in cfg.qtiles(c)], axis=1
        )
        colsl = np.concatenate(
            [xl[:, 128 * t : 128 * (t + 1)] for t in cfg.qtiles(c)], axis=1
        )
        # multiplicative {0,1} masks on the fp8 P slabs, per key-block residue
        masks = np.stack(
            [(jl - ii <= 128 * (c - r)) for r in range(cfg.ncores)]
        ).astype(np.float32)
        maskq = np.ascontiguousarray(
            masks.transpose(1, 0, 2).reshape(128, cfg.ncores * 128)
        ).astype(NPBF16)
        in_maps.append(
            {
                "xh": xh,
                "xl": xl,
                "xilh": xilh,
                "xill": xill,
                "xqh": np.ascontiguousarray(colsh),
                "xql": np.ascontiguousarray(colsl),
                "wqilh": wqilh,
                "wqill": wqill,
                "wkilh": wkilh,
                "wkill": wkill,
                "wvrh": wvrh,
                "wvrl": wvrl,
                "wP": wP,
                "bq": bq,
                "bk": bk,
                "bP": bP,
                "maskq": maskq,
            }
        )
    return in_maps


def declare_io(nc, cfg=CFG):
    C, T, HP, NJP, QW = cfg.C, cfg.T, cfg.HP, cfg.NJP, cfg.QW
    dt = nc.dram_tensor
    ins = {
        "xh": dt("xh", [C, T], E4, kind="ExternalInput").ap(),
        "xl": dt("xl", [C, T], E4, kind="ExternalInput").ap(),
        "xilh": dt("xilh", [128, NJP, 32, 256], E4, kind="ExternalInput").ap(),
        "xill": dt("xill", [128, NJP, 32, 256], E4, kind="ExternalInput").ap(),
        "xqh": dt("xqh", [C, QW], E4, kind="ExternalInput").ap(),
        "xql": dt("xql", [C, QW], E4, kind="ExternalInput").ap(),
        "wqilh": dt("wqilh", [128, NJP, HP, 256], E4, kind="ExternalInput").ap(),
        "wqill": dt("wqill", [128, NJP, HP, 256], E4, kind="ExternalInput").ap(),
        "wkilh": dt("wkilh", [128, NJP, HP, 256], E4, kind="ExternalInput").ap(),
        "wkill": dt("wkill", [128, NJP, HP, 256], E4, kind="ExternalInput").ap(),
        "wvrh": dt("wvrh", [C, C], E4, kind="ExternalInput").ap(),
        "wvrl": dt("wvrl", [C, C], E4, kind="ExternalInput").ap(),
        "wP": dt("wP", [C, C], BF16, kind="ExternalInput").ap(),
        "bq": dt("bq", [C], F32, kind="ExternalInput").ap(),
        "bk": dt("bk", [C], F32, kind="ExternalInput").ap(),
        "bP": dt("bP", [C], F32, kind="ExternalInput").ap(),
        "maskq": dt("maskq", [128, cfg.ncores * 128], BF16,
                    kind="ExternalInput").ap(),
    }
    outs = {
        "y": dt("y", [QW, C], F32, kind="ExternalOutput").ap()
    }
    return ins, outs


def build_program(cfg=CFG, repeat=1):
    nc = bacc.Bacc("TRN2", target_bir_lowering=False, debug=False,
                   num_devices=cfg.ncores)
    ins, outs = declare_io(nc, cfg)
    with tile.TileContext(nc) as tc:
        for _ in range(repeat):
            build_kernel_v3(tc, outs, ins, cfg)
    nc.compile()
    return nc


def assemble_output(results, cfg=CFG):
    y = np.empty((cfg.T, cfg.C), np.float32)
    for c in range(cfg.ncores):
        yc = results[c]["y"]
        for g, t in enumerate(cfg.qtiles(c)):
            y[128 * t : 128 * (t + 1)] = yc[128 * g : 128 * (g + 1)]
    return y.reshape(1, cfg.T, cfg.C)


_PROGRAM = None


def kernel(x, w_attn, b_attn, w_proj, b_proj):
    global _PROGRAM
    cfg = CFG
    x = np.asarray(x, np.float32)
    if _PROGRAM is None:
        _PROGRAM = build_program(cfg)
    in_maps = make_in_maps(
        x, np.asarray(w_attn), np.asarray(b_attn), np.asarray(w_proj),
        np.asarray(b_proj), cfg
    )
    res = run_bass_kernel_spmd(_PROGRAM, in_maps, core_ids=list(range(cfg.ncores)))
    return assemble_output(res.results, cfg)


if __name__ == "__main__":
    import reference

    inputs = {k: np.asarray(v) for k, v in reference.setup_inputs().items()}
    out = kernel(**inputs)
    print("kernel output", out.shape, out.dtype)


# revision 71
# speedup vs baseline: 1.1757x; 1.0162x over previous
"""Causal self-attention (B=1, T=4096, C=768, H=12) on 8 TRN2 NeuronCores.

Strategy (single SPMD NEFF, no collectives):
  - Sequence-parallel over queries: core c owns q-tiles {c, c+8, c+16, c+24}
    (128 rows each, descending-extent column order). Slot s of every core
    processes key-blocks 8s..8s+7 (uniform instruction stream across cores);
    the true causal boundary is enforced by a per-core binary mask library
    passed as input data, so ONE program serves all 8 cores.
  - K/V/Q projections run as error-compensated fp8 DoubleRowSwInterleave
    matmuls: host splits x and 16*w_attn into e4m3 (hi, lo) pairs and the
    kernel computes xh*wh + xh*wl + xl*wh (the lo*lo term is negligible).
    Each DRI matmul contracts TWO 128-row k-tiles per pass at 0.5 cyc/row,
    so the 9-matmul group costs 0.75x the bf16 equivalent with bf16-class
    accuracy (measured end-to-end rel err 3.4e-3 for the projections).
  - Attention scores stay transposed: S^T = K @ Q^T with keys on partitions;
    exp runs PSUM->SBUF on ScalarE with scale 1/2048 (the 16x weight
    prescale squares into S) and bias -2 so exp output fits fp8e4 range.
  - P^T is written as fp8e4; PV uses DRI pairing two CONSECUTIVE KEY BLOCKS
    per pass (keys are the contraction dim), with V stored as interleaved
    fp8 (hi, lo) stationaries: y = P*vh + P*vl keeps v at bf16-class
    precision while PV runs at 2x bf16 speed. The V bias is folded into an
    effective output-projection bias on the host (exact).
  - K/V live in small rolling per-wave buffers (each wave's blocks are only
    read by that wave's attention). The softmax denominator falls out of a
    65th all-ones column of the padded-to-128 interleaved V stationary.
  - Per (head, wave): sweep 1 computes QK + exp + mask for all four block
    pairs (pt tiles buffered), sweep 2 fires the eight PV matmuls back to
    back so the PE never waits on a freshly produced mask; masks run 1/4 on
    DVE and 3/4 on GPSIMD to balance the elementwise queues.
  - Measured end-to-end relative error vs the fp32 reference: 1.2e-2
    (matching a numpy emulation of the same quantization points).
"""

import contextlib
from dataclasses import dataclass

import ml_dtypes
import numpy as np

import concourse.bass as bass
import concourse.mybir as mybir
import concourse.tile as tile
from concourse import bacc
from concourse.bass_utils import run_bass_kernel_spmd

BF16 = mybir.dt.bfloat16
F32 = mybir.dt.float32
E4 = mybir.dt.float8e4
NPBF16 = ml_dtypes.bfloat16
NPE4 = ml_dtypes.float8_e4m3
DRI = mybir.MatmulPerfMode.DoubleRowSwInterleave

SW = 16.0  # weight prescale (power of two: commutes with rounding)
EXP_SCALE = 1.0 / (8.0 * SW * SW)  # 1/(sqrt(D) * SW^2)
EXP_BIAS = -2.0  # keeps exp output within fp8e4 range; cancels in softmax


@dataclass(frozen=True)
class Cfg:
    T: int = 4096
    H: int = 12
    D: int = 64
    ncores: int = 8

    @property
    def C(self):
        return self.H * self.D

    @property
    def HP(self):  # head pairs
        return self.H // 2

    @property
    def NKB(self):  # 128-row key blocks
        return self.T // 128

    @property
    def NCH(self):  # 512-row key chunks
        return self.T // 512

    @property
    def QTC(self):  # q-tiles per core
        return self.T // 128 // self.ncores

    @property
    def QW(self):  # q columns per core
        return 128 * self.QTC

    @property
    def NCT(self):  # 128-row contraction tiles over C
        return self.C // 128

    @property
    def NJP(self):  # contraction k-tile pairs
        return self.NCT // 2

    def nb(self, b):  # valid q-column prefix width for key-block b
        return 128 * (self.QTC - b // self.ncores)

    def qtiles(self, c):  # global q-tile indices for core c, descending extent
        return [c + self.ncores * (self.QTC - 1 - g) for g in range(self.QTC)]


CFG = Cfg()


def build_kernel_v3(tc, outs, ins, cfg=CFG):
    nc = tc.nc
    C, H, HP, NJP = cfg.C, cfg.H, cfg.HP, cfg.NJP
    QW, NCH = cfg.QW, cfg.NCH
    Exp = mybir.ActivationFunctionType.Exp
    Ident = mybir.ActivationFunctionType.Identity

    xh, xl = ins["xh"], ins["xl"]
    xilh, xill = ins["xilh"], ins["xill"]
    xqh, xql = ins["xqh"], ins["xql"]
    wkilh, wkill = ins["wkilh"], ins["wkill"]
    wqilh, wqill = ins["wqilh"], ins["wqill"]
    wvrh, wvrl = ins["wvrh"], ins["wvrl"]
    wP = ins["wP"]
    bq_in, bk_in, bP_in = ins["bq"], ins["bk"], ins["bP"]
    maskq = ins["maskq"]
    y = outs["y"]

    stack = contextlib.ExitStack()
    with stack:
        persist = stack.enter_context(tc.tile_pool(name="persist", bufs=1))

        # rolling per-wave K^T (bf16, scaled 16x) and interleaved V (fp8 hi/lo)
        kt_roll = persist.tile([128, 2, HP, 1024], BF16, name="kt_roll")
        vh_roll = persist.tile([128, 2, 4, H, 256], E4, name="vh_roll")
        vl_roll = persist.tile([128, 2, 4, H, 256], E4, name="vl_roll")
        qt_t = persist.tile([128, HP, QW], BF16, name="qt_t")
        ytf = persist.tile([128, HP, QW], BF16, name="ytf")
        yacc = persist.tile([128, H, QW], F32, name="yacc")  # rows 0:65 used
        mask_sb = persist.tile([128, cfg.ncores * 128], BF16, name="mask_sb")
        wp_sb = persist.tile([128, cfg.NCT, C], BF16, name="wp_sb")
        wkh_sb = persist.tile([128, NJP, HP, 256], E4, name="wkh_sb")
        wkl_sb = persist.tile([128, NJP, HP, 256], E4, name="wkl_sb")
        wvh_sb = persist.tile([128, cfg.NCT, C], E4, name="wvh_sb")
        wvl_sb = persist.tile([128, cfg.NCT, C], E4, name="wvl_sb")
        bq_sb = persist.tile([128, HP], F32, name="bq_sb")
        bk_sb = persist.tile([128, HP], F32, name="bk_sb")
        bp_bc = persist.tile([128, C], F32, name="bp_bc")
        ones11 = persist.tile([1, 64], F32, name="ones11")
        ebias = persist.tile([128, 1], F32, name="ebias")

        nc.vector.memset(ebias, EXP_BIAS)
        nc.vector.memset(ones11, 1.0 / SW)
        # touch Exp early so the ACT table set loads during startup DMAs
        nc.scalar.activation(ones11, ones11, Exp, scale=0.0)
        nc.vector.memset(ones11, 1.0 / SW)
        # V stationaries: zero the pad region once (gpsimd memset); set the
        # ones column (logical col 64 of 128 -> interleaved positions 126-127).
        vh4 = vh_roll.rearrange("p w q h (t two) -> p w q h t two", two=2)
        vl4 = vl_roll.rearrange("p w q h (t two) -> p w q h t two", two=2)
        nc.gpsimd.memset(vh4[:, :, :, :, 0:63, :], 0.0)
        nc.gpsimd.memset(vl4[:, :, :, :, 0:64, :], 0.0)
        nc.vector.memset(vh4[:, :, :, :, 63:64, :], 1.0)

        with (
            tc.tile_pool(name="xpool", bufs=3) as xpool,
            tc.tile_pool(name="pkv", bufs=2, space="PSUM") as pkv,
            tc.tile_pool(name="pvp", bufs=2, space="PSUM") as pvp,
            tc.tile_pool(name="ptp", bufs=6) as ptp,
            tc.tile_pool(name="nrm", bufs=1) as nrm,
        ):
            qproj = tc.alloc_tile_pool(name="qproj", bufs=1)
            aps = tc.alloc_tile_pool(name="apsA", bufs=2, space="PSUM")

            xhr = xh.rearrange("(j p) t -> p j t", p=128)
            xlr = xl.rearrange("(j p) t -> p j t", p=128)

            def load_xch(ch, split=False):
                th = xpool.tile([128, cfg.NCT, 512], E4, name="xch_h", tag="xh")
                tl = xpool.tile([128, cfg.NCT, 512], E4, name="xch_l", tag="xl")
                tih = xpool.tile([128, NJP, 4, 256], E4, name="xil_h", tag="xih")
                til = xpool.tile([128, NJP, 4, 256], E4, name="xil_l", tag="xil")
                cs = 512 * ch
                if split:
                    # ct-pair pieces: the first projection group consumes
                    # pair j as soon as piece j lands
                    for j in range(NJP):
                        nc.sync.dma_start(
                            out=th[:, 2 * j : 2 * j + 2, :],
                            in_=xhr[:, 2 * j : 2 * j + 2, cs : cs + 512],
                        )
                        nc.sync.dma_start(
                            out=tl[:, 2 * j : 2 * j + 2, :],
                            in_=xlr[:, 2 * j : 2 * j + 2, cs : cs + 512],
                        )
                else:
                    nc.sync.dma_start(out=th, in_=xhr[:, :, cs : cs + 512])
                    nc.sync.dma_start(out=tl, in_=xlr[:, :, cs : cs + 512])
                nc.sync.dma_start(out=tih, in_=xilh[:, :, 4 * ch : 4 * ch + 4, :])
                nc.sync.dma_start(out=til, in_=xill[:, :, 4 * ch : 4 * ch + 4, :])
                return th, tl, tih, til

            # startup DMA order: first x chunk, K weights, V weights, masks,
            # Q inputs - so the PE never waits on a cold queue
            nc.sync.dma_start(out=wkh_sb, in_=wkilh)
            nc.sync.dma_start(out=wkl_sb, in_=wkill)
            # biases ride the idle ACT DMA queue so their issue+descgen
            # never sits ahead of the critical startup loads on SP
            nc.scalar.dma_start(out=bq_sb,
                                in_=bq_in.rearrange("(hp p) -> p hp", p=128))
            nc.scalar.dma_start(out=bk_sb,
                                in_=bk_in.rearrange("(hp p) -> p hp", p=128))
            xch_pre = {0: load_xch(0, split=True)}
            nc.sync.dma_start(
                out=wvh_sb, in_=wvrh.rearrange("(j p) t -> p j t", p=128)
            )
            nc.sync.dma_start(
                out=wvl_sb, in_=wvrl.rearrange("(j p) t -> p j t", p=128)
            )
            xch_pre[1] = load_xch(1)
            nc.sync.dma_start(out=mask_sb, in_=maskq)
            wqh_sb = qproj.tile([128, NJP, HP, 256], E4, name="wqh_sb")
            wql_sb = qproj.tile([128, NJP, HP, 256], E4, name="wql_sb")
            xqh_sb = qproj.tile([128, cfg.NCT, QW], E4, name="xqh_sb")
            xql_sb = qproj.tile([128, cfg.NCT, QW], E4, name="xql_sb")
            nc.sync.dma_start(out=wqh_sb, in_=wqilh)
            nc.sync.dma_start(out=wql_sb, in_=wqill)
            nc.sync.dma_start(out=xqh_sb, in_=xqh.rearrange("(j p) t -> p j t", p=128))
            nc.sync.dma_start(out=xql_sb, in_=xql.rearrange("(j p) t -> p j t", p=128))

            def comp_dri(ps, wil_h, wil_l, xp_h, xp_l, n0=None, n1=None):
                """9-term compensated DRI group into `ps`.

                wil_*: callables j -> stationary AP [128, 2*M interleaved]
                xp_*: callables j -> moving AP [128, 2, N]
                """
                terms = [(wil_h, xp_h), (wil_l, xp_h), (wil_h, xp_l)]
                nmm = 0
                for wf, xf in terms:
                    for j in range(NJP):
                        nc.tensor.matmul(
                            ps,
                            wf(j).rearrange("p (m two) -> p m two", two=2),
                            xf(j),
                            start=(nmm == 0),
                            stop=(nmm == 3 * NJP - 1),
                            perf_mode=DRI,
                        )
                        nmm += 1

            for cp in range(NCH // 2):
                par = cp % 2
                first, last = cp == 0, cp == NCH // 2 - 1
                chunks = (2 * cp, 2 * cp + 1)
                if cp == 2:
                    # small-n waves: swap the 2x2-bank score pool for a
                    # 4x1-bank pool (a pair fits one bank), doubling the
                    # QK->exp ping-pong depth
                    aps.release()
                    aps = tc.alloc_tile_pool(name="apsB", bufs=4,
                                             space="PSUM")
                smallw = cfg.nb(4 * chunks[0]) <= 256
                # ---- project K^T / V for this wave's two chunks ------------
                for half, ch in enumerate(chunks):
                    th, tl, tih, til = (
                        xch_pre.pop(ch) if ch in xch_pre else load_xch(ch)
                    )
                    sched = [("k", hp) for hp in range(HP)] + [
                        ("v", (tt, nn)) for tt in range(4) for nn in range(2)
                    ]
                    for kind, item in sched:
                      if kind == "k":
                        hp = item
                        ps_k = pkv.tile([128, 512], F32, name="ps_k", tag="pkv")
                        comp_dri(
                            ps_k,
                            lambda j, hp=hp: wkh_sb[:, j, hp, :],
                            lambda j, hp=hp: wkl_sb[:, j, hp, :],
                            lambda j: th[:, 2 * j : 2 * j + 2, :],
                            lambda j: tl[:, 2 * j : 2 * j + 2, :],
                        )
                        nc.vector.tensor_scalar_add(
                            kt_roll[:, par, hp, 512 * half : 512 * (half + 1)],
                            ps_k,
                            bk_sb[:, hp : hp + 1],
                        )
                      else:
                        tt, nn = item
                        pi = 2 * half + tt // 2  # pair index in wave
                        pb = tt % 2  # block within pair
                        for n0, n1 in (((0, 384),) if nn == 0 else ((384, 768),)):
                            h0, h1 = n0 // 64, n1 // 64
                            ps_v = pkv.tile([128, 384], F32, name="ps_v", tag="pkv")
                            nmm = 0
                            for xf, wf in (
                                (tih, wvh_sb),
                                (tih, wvl_sb),
                                (til, wvh_sb),
                            ):
                                for j in range(NJP):
                                    nc.tensor.matmul(
                                        ps_v,
                                        xf[:, j, tt, :].rearrange(
                                            "p (m two) -> p m two", two=2
                                        ),
                                        wf[:, 2 * j : 2 * j + 2, n0:n1],
                                        start=(nmm == 0),
                                        stop=(nmm == 3 * NJP - 1),
                                        perf_mode=DRI,
                                    )
                                    nmm += 1
                            # v_hi = e4m3(v); v_lo = v - v_hi (bias folded into
                            # the output projection host-side)
                            psr = ps_v.rearrange("p (h e) -> p h e", e=64)
                            vh4w = vh_roll.rearrange(
                                "p w q h (t two) -> p w q h t two", two=2
                            )[:, par, pi, h0:h1, 64:128, pb]
                            vl4w = vl_roll.rearrange(
                                "p w q h (t two) -> p w q h t two", two=2
                            )[:, par, pi, h0:h1, 64:128, pb]
                            nc.vector.tensor_copy(vh4w, psr)
                            nc.vector.tensor_sub(vl4w, psr, vh4w)

                if cp == min(1, NCH // 2 - 1):
                    # prefetch output-projection weights mid-loop
                    for ct in range(cfg.NCT):
                        nc.sync.dma_start(
                            out=wp_sb[:, ct, :],
                            in_=wP[128 * ct : 128 * (ct + 1), :],
                        )
                    bp_src = bass.AP(
                        tensor=bP_in.tensor, offset=bP_in.offset, ap=[[0, 128], [1, C]]
                    )
                    nc.gpsimd.dma_start(out=bp_bc, in_=bp_src)
                if cp == 0:
                    # Q^T projection - emitted here so the PE chews K/V
                    # projection first while the Q inputs stream in
                    for hp in range(HP):
                        ps_q = pvp.tile([128, QW], F32, name="ps_q", tag="ps_y")
                        comp_dri(
                            ps_q,
                            lambda j, hp=hp: wqh_sb[:, j, hp, :],
                            lambda j, hp=hp: wql_sb[:, j, hp, :],
                            lambda j: xqh_sb[:, 2 * j : 2 * j + 2, :],
                            lambda j: xql_sb[:, 2 * j : 2 * j + 2, :],
                        )
                        nc.scalar.activation(
                            qt_t[:, hp, :], ps_q, Ident, bias=bq_sb[:, hp : hp + 1]
                        )
                    qproj.release()

                # ---- attention for this wave's 8 key-blocks ----------------
                nA = cfg.nb(4 * chunks[0])  # widths per half-wave
                for hp in range(HP):
                    for h in range(2):
                        hd = 2 * hp + h
                        ps_y = pvp.tile([128, 512], F32, name="ps_y", tag="ps_y")
                        # sweep 1: QK + exp + mask for all four pairs (pt
                        # tiles held); sweep 2: all eight PV matmuls back to
                        # back - PV never waits on a freshly computed mask
                        ptl = []
                        if True:
                          for pi in range(4):
                            half = pi // 2
                            ch = chunks[half]
                            pl = pi % 2  # pair within the half-wave
                            ba = 4 * ch + 2 * pl
                            n = cfg.nb(ba)
                            pt = ptp.tile([128, 1024], E4, name=f"pt{h}",
                                          tag=f"pt{h}")
                            # big waves: blocks at offsets 0/512 in a 2-bank
                            # tile; small waves: contiguous at 0/n in 1 bank
                            sw = 1024 if not smallw else 512
                            off = 512 if not smallw else n
                            sps = aps.tile([128, sw], F32, name="sps",
                                           tag="sps")
                            blkv = pt[:, 0 : 2 * off].rearrange(
                                "p (b n) -> p b n", n=off
                            )[:, :, 0:n]
                            for pb in (0, 1):
                                bw = 4 * half + 2 * pl + pb  # kt_roll block
                                nc.tensor.matmul(
                                    sps[:, off * pb : off * pb + n],
                                    kt_roll[64 * h : 64 * (h + 1), par, hp,
                                            128 * bw : 128 * (bw + 1)],
                                    qt_t[64 * h : 64 * (h + 1), hp, 0:n],
                                    start=True,
                                    stop=True,
                                )
                            nc.scalar.activation(
                                blkv,
                                sps[:, 0 : 2 * off].rearrange(
                                    "p (b n) -> p b n", n=off
                                )[:, :, 0:n],
                                Exp, scale=EXP_SCALE, bias=ebias,
                            )
                            # causal boundary: mask last 128 q-cols of each blk
                            r0 = ba % cfg.ncores
                            pts = blkv[:, :, n - 128 : n]
                            msk = mask_sb[:, 128 * r0 : 128 * (r0 + 2)].rearrange(
                                "p (b n) -> p b n", n=128
                            )
                            meng = nc.vector if pi == 0 else nc.gpsimd
                            meng.tensor_mul(pts, pts, msk)
                            ptl.append((pi, n, blkv))
                        for pi, n, blkv in ptl:
                            # PV: two DRI matmuls (v_hi, v_lo), contraction
                            # over both blocks of the pair
                            for vroll in (vh_roll, vl_roll):
                                nc.tensor.matmul(
                                    ps_y[:, 0:n],
                                    vroll[:, par, pi, hd, :].rearrange(
                                        "p (m two) -> p m two", two=2
                                    ),
                                    blkv,
                                    start=(pi == 0 and vroll is vh_roll),
                                    stop=(pi == 3 and vroll is vl_roll),
                                    perf_mode=DRI,
                                    skip_group_check=True,
                                )
                        if first:
                            nc.vector.tensor_copy(
                                yacc[0:65, hd, 0:nA], ps_y[0:65, 0:nA]
                            )
                        else:
                            nc.vector.tensor_add(
                                yacc[0:65, hd, 0:nA],
                                yacc[0:65, hd, 0:nA],
                                ps_y[0:65, 0:nA],
                            )
                        if last:
                            # normalize this head now - overlaps the
                            # remaining heads' attention
                            rec = nrm.tile([1, QW], F32, name="rec", tag="rec")
                            rc_ps = pkv.tile([64, QW], F32, name="rc_ps",
                                             tag="pkv")
                            nc.vector.reciprocal(rec, yacc[64:65, hd, :])
                            nc.tensor.matmul(
                                rc_ps, ones11[0:1, :], rec, start=True, stop=True
                            )
                            nc.vector.tensor_mul(
                                ytf[64 * h : 64 * (h + 1), hp, :],
                                yacc[0:64, hd, :], rc_ps,
                            )

            aps.release()

        # ---- output projection -------------------------------------------
        with (
            tc.tile_pool(name="ops", bufs=2, space="PSUM") as ops,
            tc.tile_pool(name="osb", bufs=2) as osb,
        ):
            for g in range(cfg.QTC):
                ps_o = ops.tile([128, C], F32, name="ps_o", tag="ps_o")
                for n0, n1 in ((0, 512), (512, C)):
                    for hp in range(HP):
                        nc.tensor.matmul(
                            ps_o[:, n0:n1],
                            ytf[:, hp, 128 * g : 128 * (g + 1)],
                            wp_sb[:, hp, n0:n1],
                            start=(hp == 0),
                            stop=(hp == HP - 1),
                        )
                yo = osb.tile([128, C], F32, name="yo", tag="yo")
                nc.vector.tensor_add(yo, ps_o, bp_bc)
                nc.sync.dma_start(out=y[128 * g : 128 * (g + 1), :], in_=yo)


# ---------------------------------------------------------------------------
# host side
# ---------------------------------------------------------------------------


def _hilo(a):
    hi = np.asarray(a, NPE4)
    lo = np.asarray(a - hi.astype(np.float32), NPE4)
    return hi, lo


def _ileave4(W4):
    """[NCT, 128, G, M] -> interleaved [128, NCT/2, G, 2M] walrus layout."""
    A = W4[0::2]  # [NJP, 128, G, M]
    B = W4[1::2]
    il = np.empty(A.shape[:3] + (2 * A.shape[3],), A.dtype)
    il[..., 0::2] = A[..., ::-1]
    il[..., 1::2] = B[..., ::-1]
    return np.ascontiguousarray(il.transpose(1, 0, 2, 3))


def make_in_maps(x, w_attn, b_attn, w_proj, b_proj, cfg=CFG):
    T, C, H, HP, NCT = cfg.T, cfg.C, cfg.H, cfg.HP, cfg.NCT
    xT = np.ascontiguousarray(x.reshape(T, C).T).astype(np.float32)  # [C,T]
    xh, xl = _hilo(xT)

    w16 = (np.asarray(w_attn, np.float32)) * SW
    wq16, wk16, wv16 = w16[:, 0:C], w16[:, C : 2 * C], w16[:, 2 * C :]

    def wil_pair(wsec):
        h, l = _hilo(wsec)
        W4h = h.reshape(NCT, 128, HP, 128)
        W4l = l.reshape(NCT, 128, HP, 128)
        return _ileave4(W4h), _ileave4(W4l)

    wqilh, wqill = wil_pair(wq16)
    wkilh, wkill = wil_pair(wk16)

    # V moving operand: per-head reversed d order (so the strided interleaved
    # SBUF write runs with a positive stride)
    wvr = np.ascontiguousarray(
        wv16.reshape(C, H, 64)[:, :, ::-1].reshape(C, C)
    )
    wvrh, wvrl = _hilo(wvr)

    # V stationary: x k-tile pairs interleaved per 128-key tile
    X4h = xh.astype(np.float32).reshape(NCT, 128, 32, 128)
    X4l = xl.astype(np.float32).reshape(NCT, 128, 32, 128)
    xilh = _ileave4(X4h.astype(NPE4))
    xill = _ileave4(X4l.astype(NPE4))

    wP = np.asarray(w_proj, np.float32).astype(NPBF16)
    bq = np.ascontiguousarray(np.asarray(b_attn[0:C], np.float32) * SW)
    bk = np.ascontiguousarray(np.asarray(b_attn[C : 2 * C], np.float32) * SW)
    # V bias folded into the output projection (exact)
    bP = np.ascontiguousarray(
        np.asarray(b_proj, np.float32)
        + np.asarray(b_attn[2 * C :], np.float32) @ np.asarray(w_proj, np.float32)
    )

    jl = np.arange(128)[:, None]
    ii = np.arange(128)[None, :]
    in_maps = []
    for c in range(cfg.ncores):
        colsh = np.concatenate(
            [xh[:, 128 * t : 128 * (t + 1)] for t in cfg.qtiles(c)], axis=1
        )
        colsl = np.concatenate(
            [xl[:, 128 * t : 128 * (t + 1)] for t in cfg.qtiles(c)], axis=1
        )
        # multiplicative {0,1} masks on the fp8 P slabs, per key-block residue
        masks = np.stack(
            [(jl - ii <= 128 * (c - r)) for r in range(cfg.ncores)]
        ).astype(np.float32)
        maskq = np.ascontiguousarray(
            masks.transpose(1, 0, 2).reshape(128, cfg.ncores * 128)
        ).astype(NPBF16)
        in_maps.append(
            {
                "xh": xh,
                "xl": xl,
                "xilh": xilh,
                "xill": xill,
                "xqh": np.ascontiguousarray(colsh),
                "xql": np.ascontiguousarray(colsl),
                "wqilh": wqilh,
                "wqill": wqill,
                "wkilh": wkilh,
                "wkill": wkill,
                "wvrh": wvrh,
                "wvrl": wvrl,
                "wP": wP,
                "bq": bq,
                "bk": bk,
                "bP": bP,
                "maskq": maskq,
            }
        )
    return in_maps


def declare_io(nc, cfg=CFG):
    C, T, HP, NJP, QW = cfg.C, cfg.T, cfg.HP, cfg.NJP, cfg.QW
    dt = nc.dram_tensor
    ins = {
        "xh": dt("xh", [C, T], E4, kind="ExternalInput").ap(),
        "xl": dt("xl", [C, T], E4, kind="ExternalInput").ap(),
        "xilh": dt("xilh", [128, NJP, 32, 256], E4, kind="ExternalInput").ap(),
        "xill": dt("xill", [128, NJP, 32, 256], E4, kind="ExternalInput").ap(),
        "xqh": dt("xqh", [C, QW], E4, kind="ExternalInput").ap(),
        "xql": dt("xql", [C, QW], E4, kind="ExternalInput").ap(),
        "wqilh": dt("wqilh", [128, NJP, HP, 256], E4, kind="ExternalInput").ap(),
        "wqill": dt("wqill", [128, NJP, HP, 256], E4, kind="ExternalInput").ap(),
        "wkilh": dt("wkilh", [128, NJP, HP, 256], E4, kind="ExternalInput").ap(),
        "wkill": dt("wkill", [128, NJP, HP, 256], E4, kind="ExternalInput").ap(),
        "wvrh": dt("wvrh", [C, C], E4, kind="ExternalInput").ap(),
        "wvrl": dt("wvrl", [C, C], E4, kind="ExternalInput").ap(),
        "wP": dt("wP", [C, C], BF16, kind="ExternalInput").ap(),
        "bq": dt("bq", [C], F32, kind="ExternalInput").ap(),
        "bk": dt("bk", [C], F32, kind="ExternalInput").ap(),
        "bP": dt("bP", [C], F32, kind="ExternalInput").ap(),
        "maskq": dt("maskq", [128, cfg.ncores * 128], BF16,
                    kind="ExternalInput").ap(),
    }
    outs = {
        "y": dt("y", [QW, C], F32, kind="ExternalOutput").ap()
    }
    return ins, outs


def build_program(cfg=CFG, repeat=1):
    nc = bacc.Bacc("TRN2", target_bir_lowering=False, debug=False,
                   num_devices=cfg.ncores)
    ins, outs = declare_io(nc, cfg)
    with tile.TileContext(nc) as tc:
        for _ in range(repeat):
            build_kernel_v3(tc, outs, ins, cfg)
    nc.compile()
    return nc


def assemble_output(results, cfg=CFG):
    y = np.empty((cfg.T, cfg.C), np.float32)
    for c in range(cfg.ncores):
        yc = results[c]["y"]
        for g, t in enumerate(cfg.qtiles(c)):
            y[128 * t : 128 * (t + 1)] = yc[128 * g : 128 * (g + 1)]
    return y.reshape(1, cfg.T, cfg.C)


_PROGRAM = None


def kernel(x, w_attn, b_attn, w_proj, b_proj):
    global _PROGRAM
    cfg = CFG
    x = np.asarray(x, np.float32)
    if _PROGRAM is None:
        _PROGRAM = build_program(cfg)
    in_maps = make_in_maps(
        x, np.asarray(w_attn), np.asarray(b_attn), np.asarray(w_proj),
        np.asarray(b_proj), cfg
    )
    res = run_bass_kernel_spmd(_PROGRAM, in_maps, core_ids=list(range(cfg.ncores)))
    return assemble_output(res.results, cfg)


if __name__ == "__main__":
    import reference

    inputs = {k: np.asarray(v) for k, v in reference.setup_inputs().items()}
    out = kernel(**inputs)
    print("kernel output", out.shape, out.dtype)


# revision 73
# speedup vs baseline: 1.1880x; 1.0105x over previous
"""Causal self-attention (B=1, T=4096, C=768, H=12) on 8 TRN2 NeuronCores.

Strategy (single SPMD NEFF, no collectives):
  - Sequence-parallel over queries: core c owns q-tiles {c, c+8, c+16, c+24}
    (128 rows each, descending-extent column order). Slot s of every core
    processes key-blocks 8s..8s+7 (uniform instruction stream across cores);
    the true causal boundary is enforced by a per-core binary mask library
    passed as input data, so ONE program serves all 8 cores.
  - K/V/Q projections run as error-compensated fp8 DoubleRowSwInterleave
    matmuls: host splits x and 16*w_attn into e4m3 (hi, lo) pairs and the
    kernel computes xh*wh + xh*wl + xl*wh (the lo*lo term is negligible).
    Each DRI matmul contracts TWO 128-row k-tiles per pass at 0.5 cyc/row,
    so the 9-matmul group costs 0.75x the bf16 equivalent with bf16-class
    accuracy (measured end-to-end rel err 3.4e-3 for the projections).
  - Attention scores stay transposed: S^T = K @ Q^T with keys on partitions;
    exp runs PSUM->SBUF on ScalarE with scale 1/2048 (the 16x weight
    prescale squares into S) and bias -2 so exp output fits fp8e4 range.
  - P^T is written as fp8e4; PV uses DRI pairing two CONSECUTIVE KEY BLOCKS
    per pass (keys are the contraction dim), with V stored as interleaved
    fp8 (hi, lo) stationaries: y = P*vh + P*vl keeps v at bf16-class
    precision while PV runs at 2x bf16 speed. The V bias is folded into an
    effective output-projection bias on the host (exact).
  - K/V live in small rolling per-wave buffers (each wave's blocks are only
    read by that wave's attention). The softmax denominator falls out of a
    65th all-ones column of the padded-to-128 interleaved V stationary.
  - Per (head, wave): sweep 1 computes QK + exp + mask for all four block
    pairs (pt tiles buffered), sweep 2 fires the eight PV matmuls back to
    back so the PE never waits on a freshly produced mask; masks run 1/4 on
    DVE and 3/4 on GPSIMD to balance the elementwise queues.
  - Measured end-to-end relative error vs the fp32 reference: 1.2e-2
    (matching a numpy emulation of the same quantization points).
"""

import contextlib
from dataclasses import dataclass

import ml_dtypes
import numpy as np

import concourse.bass as bass
import concourse.mybir as mybir
import concourse.tile as tile
from concourse import bacc
from concourse.bass_utils import run_bass_kernel_spmd

BF16 = mybir.dt.bfloat16
F32 = mybir.dt.float32
E4 = mybir.dt.float8e4
NPBF16 = ml_dtypes.bfloat16
NPE4 = ml_dtypes.float8_e4m3
DRI = mybir.MatmulPerfMode.DoubleRowSwInterleave

SW = 16.0  # weight prescale (power of two: commutes with rounding)
EXP_SCALE = 1.0 / (8.0 * SW * SW)  # 1/(sqrt(D) * SW^2)
EXP_BIAS = -2.0  # keeps exp output within fp8e4 range; cancels in softmax


@dataclass(frozen=True)
class Cfg:
    T: int = 4096
    H: int = 12
    D: int = 64
    ncores: int = 8

    @property
    def C(self):
        return self.H * self.D

    @property
    def HP(self):  # head pairs
        return self.H // 2

    @property
    def NKB(self):  # 128-row key blocks
        return self.T // 128

    @property
    def NCH(self):  # 512-row key chunks
        return self.T // 512

    @property
    def QTC(self):  # q-tiles per core
        return self.T // 128 // self.ncores

    @property
    def QW(self):  # q columns per core
        return 128 * self.QTC

    @property
    def NCT(self):  # 128-row contraction tiles over C
        return self.C // 128

    @property
    def NJP(self):  # contraction k-tile pairs
        return self.NCT // 2

    def nb(self, b):  # valid q-column prefix width for key-block b
        return 128 * (self.QTC - b // self.ncores)

    def qtiles(self, c):  # global q-tile indices for core c, descending extent
        return [c + self.ncores * (self.QTC - 1 - g) for g in range(self.QTC)]


CFG = Cfg()


def build_kernel_v3(tc, outs, ins, cfg=CFG):
    nc = tc.nc
    C, H, HP, NJP = cfg.C, cfg.H, cfg.HP, cfg.NJP
    QW, NCH = cfg.QW, cfg.NCH
    Exp = mybir.ActivationFunctionType.Exp
    Ident = mybir.ActivationFunctionType.Identity

    xh, xl = ins["xh"], ins["xl"]
    xilh, xill = ins["xilh"], ins["xill"]
    xqh, xql = ins["xqh"], ins["xql"]
    wkilh, wkill = ins["wkilh"], ins["wkill"]
    wqilh, wqill = ins["wqilh"], ins["wqill"]
    wvrh, wvrl = ins["wvrh"], ins["wvrl"]
    wP = ins["wP"]
    bq_in, bk_in, bP_in = ins["bq"], ins["bk"], ins["bP"]
    maskq = ins["maskq"]
    y = outs["y"]

    stack = contextlib.ExitStack()
    with stack:
        persist = stack.enter_context(tc.tile_pool(name="persist", bufs=1))

        # rolling per-wave K^T (bf16, scaled 16x) and interleaved V (fp8 hi/lo)
        kt_roll = persist.tile([128, 2, HP, 1024], BF16, name="kt_roll")
        vh_roll = persist.tile([128, 2, 4, H, 256], E4, name="vh_roll")
        vl_roll = persist.tile([128, 2, 4, H, 256], E4, name="vl_roll")
        qt_t = persist.tile([128, HP, QW], BF16, name="qt_t")
        ytf = persist.tile([128, HP, QW], BF16, name="ytf")
        yacc = persist.tile([128, H, QW], F32, name="yacc")  # rows 0:65 used
        mask_sb = persist.tile([128, cfg.ncores * 128], BF16, name="mask_sb")
        wp_sb = persist.tile([128, cfg.NCT, C], BF16, name="wp_sb")
        wkh_sb = persist.tile([128, NJP, HP, 256], E4, name="wkh_sb")
        wkl_sb = persist.tile([128, NJP, HP, 256], E4, name="wkl_sb")
        wvh_sb = persist.tile([128, cfg.NCT, C], E4, name="wvh_sb")
        wvl_sb = persist.tile([128, cfg.NCT, C], E4, name="wvl_sb")
        bq_sb = persist.tile([128, HP], F32, name="bq_sb")
        bk_sb = persist.tile([128, HP], F32, name="bk_sb")
        bp_bc = persist.tile([128, C], F32, name="bp_bc")
        ones11 = persist.tile([1, 64], F32, name="ones11")
        ebias = persist.tile([128, 1], F32, name="ebias")

        nc.vector.memset(ebias, EXP_BIAS)
        nc.vector.memset(ones11, 1.0 / SW)
        # touch Exp early so the ACT table set loads during startup DMAs
        nc.scalar.activation(ones11, ones11, Exp, scale=0.0)
        nc.vector.memset(ones11, 1.0 / SW)
        # V stationaries: zero the pad region once (gpsimd memset); set the
        # ones column (logical col 64 of 128 -> interleaved positions 126-127).
        vh4 = vh_roll.rearrange("p w q h (t two) -> p w q h t two", two=2)
        vl4 = vl_roll.rearrange("p w q h (t two) -> p w q h t two", two=2)
        nc.gpsimd.memset(vh4[:, :, :, :, 0:63, :], 0.0)
        nc.gpsimd.memset(vl4[:, :, :, :, 0:64, :], 0.0)
        nc.vector.memset(vh4[:, :, :, :, 63:64, :], 1.0)

        with (
            tc.tile_pool(name="xpool", bufs=3) as xpool,
            tc.tile_pool(name="pkv", bufs=2, space="PSUM") as pkv,
            tc.tile_pool(name="pvp", bufs=2, space="PSUM") as pvp,
            tc.tile_pool(name="nrm", bufs=1) as nrm,
        ):
            ptp = tc.alloc_tile_pool(name="ptpA", bufs=6)
            qproj = tc.alloc_tile_pool(name="qproj", bufs=1)
            aps = tc.alloc_tile_pool(name="apsA", bufs=2, space="PSUM")

            xhr = xh.rearrange("(j p) t -> p j t", p=128)
            xlr = xl.rearrange("(j p) t -> p j t", p=128)

            def load_xch(ch, split=False):
                th = xpool.tile([128, cfg.NCT, 512], E4, name="xch_h", tag="xh")
                tl = xpool.tile([128, cfg.NCT, 512], E4, name="xch_l", tag="xl")
                tih = xpool.tile([128, NJP, 4, 256], E4, name="xil_h", tag="xih")
                til = xpool.tile([128, NJP, 4, 256], E4, name="xil_l", tag="xil")
                cs = 512 * ch
                if split:
                    # ct-pair pieces: the first projection group consumes
                    # pair j as soon as piece j lands
                    for j in range(NJP):
                        nc.sync.dma_start(
                            out=th[:, 2 * j : 2 * j + 2, :],
                            in_=xhr[:, 2 * j : 2 * j + 2, cs : cs + 512],
                        )
                        nc.sync.dma_start(
                            out=tl[:, 2 * j : 2 * j + 2, :],
                            in_=xlr[:, 2 * j : 2 * j + 2, cs : cs + 512],
                        )
                else:
                    nc.sync.dma_start(out=th, in_=xhr[:, :, cs : cs + 512])
                    nc.sync.dma_start(out=tl, in_=xlr[:, :, cs : cs + 512])
                nc.sync.dma_start(out=tih, in_=xilh[:, :, 4 * ch : 4 * ch + 4, :])
                nc.sync.dma_start(out=til, in_=xill[:, :, 4 * ch : 4 * ch + 4, :])
                return th, tl, tih, til

            # startup DMA order: first x chunk, K weights, V weights, masks,
            # Q inputs - so the PE never waits on a cold queue
            nc.sync.dma_start(out=wkh_sb, in_=wkilh)
            nc.sync.dma_start(out=wkl_sb, in_=wkill)
            # biases ride the idle ACT DMA queue so their issue+descgen
            # never sits ahead of the critical startup loads on SP
            nc.scalar.dma_start(out=bq_sb,
                                in_=bq_in.rearrange("(hp p) -> p hp", p=128))
            nc.scalar.dma_start(out=bk_sb,
                                in_=bk_in.rearrange("(hp p) -> p hp", p=128))
            xch_pre = {0: load_xch(0, split=True)}
            nc.sync.dma_start(
                out=wvh_sb, in_=wvrh.rearrange("(j p) t -> p j t", p=128)
            )
            nc.sync.dma_start(
                out=wvl_sb, in_=wvrl.rearrange("(j p) t -> p j t", p=128)
            )
            xch_pre[1] = load_xch(1)
            nc.sync.dma_start(out=mask_sb, in_=maskq)
            wqh_sb = qproj.tile([128, NJP, HP, 256], E4, name="wqh_sb")
            wql_sb = qproj.tile([128, NJP, HP, 256], E4, name="wql_sb")
            xqh_sb = qproj.tile([128, cfg.NCT, QW], E4, name="xqh_sb")
            xql_sb = qproj.tile([128, cfg.NCT, QW], E4, name="xql_sb")
            nc.sync.dma_start(out=wqh_sb, in_=wqilh)
            nc.sync.dma_start(out=wql_sb, in_=wqill)
            nc.sync.dma_start(out=xqh_sb, in_=xqh.rearrange("(j p) t -> p j t", p=128))
            nc.sync.dma_start(out=xql_sb, in_=xql.rearrange("(j p) t -> p j t", p=128))

            def comp_dri(ps, wil_h, wil_l, xp_h, xp_l, n0=None, n1=None):
                """9-term compensated DRI group into `ps`.

                wil_*: callables j -> stationary AP [128, 2*M interleaved]
                xp_*: callables j -> moving AP [128, 2, N]
                """
                terms = [(wil_h, xp_h), (wil_l, xp_h), (wil_h, xp_l)]
                nmm = 0
                for wf, xf in terms:
                    for j in range(NJP):
                        nc.tensor.matmul(
                            ps,
                            wf(j).rearrange("p (m two) -> p m two", two=2),
                            xf(j),
                            start=(nmm == 0),
                            stop=(nmm == 3 * NJP - 1),
                            perf_mode=DRI,
                        )
                        nmm += 1

            for cp in range(NCH // 2):
                par = cp % 2
                first, last = cp == 0, cp == NCH // 2 - 1
                chunks = (2 * cp, 2 * cp + 1)
                if cp == 2:
                    # small-n waves: swap the 2x2-bank score pool for a
                    # 4x1-bank pool (a pair fits one bank), doubling the
                    # QK->exp ping-pong depth; likewise swap the P pool to
                    # ten half-size tiles (a small pair is <= 512 B)
                    aps.release()
                    aps = tc.alloc_tile_pool(name="apsB", bufs=4,
                                             space="PSUM")
                    ptp.release()
                    ptp = tc.alloc_tile_pool(name="ptpB", bufs=10)
                smallw = cfg.nb(4 * chunks[0]) <= 256
                # ---- project K^T / V for this wave's two chunks ------------
                for half, ch in enumerate(chunks):
                    th, tl, tih, til = (
                        xch_pre.pop(ch) if ch in xch_pre else load_xch(ch)
                    )
                    sched = [("k", hp) for hp in range(HP)] + [
                        ("v", (tt, nn)) for tt in range(4) for nn in range(2)
                    ]
                    for kind, item in sched:
                      if kind == "k":
                        hp = item
                        ps_k = pkv.tile([128, 512], F32, name="ps_k", tag="pkv")
                        comp_dri(
                            ps_k,
                            lambda j, hp=hp: wkh_sb[:, j, hp, :],
                            lambda j, hp=hp: wkl_sb[:, j, hp, :],
                            lambda j: th[:, 2 * j : 2 * j + 2, :],
                            lambda j: tl[:, 2 * j : 2 * j + 2, :],
                        )
                        nc.vector.tensor_scalar_add(
                            kt_roll[:, par, hp, 512 * half : 512 * (half + 1)],
                            ps_k,
                            bk_sb[:, hp : hp + 1],
                        )
                      else:
                        tt, nn = item
                        pi = 2 * half + tt // 2  # pair index in wave
                        pb = tt % 2  # block within pair
                        for n0, n1 in (((0, 384),) if nn == 0 else ((384, 768),)):
                            h0, h1 = n0 // 64, n1 // 64
                            ps_v = pkv.tile([128, 384], F32, name="ps_v", tag="pkv")
                            nmm = 0
                            for xf, wf in (
                                (tih, wvh_sb),
                                (tih, wvl_sb),
                                (til, wvh_sb),
                            ):
                                for j in range(NJP):
                                    nc.tensor.matmul(
                                        ps_v,
                                        xf[:, j, tt, :].rearrange(
                                            "p (m two) -> p m two", two=2
                                        ),
                                        wf[:, 2 * j : 2 * j + 2, n0:n1],
                                        start=(nmm == 0),
                                        stop=(nmm == 3 * NJP - 1),
                                        perf_mode=DRI,
                                    )
                                    nmm += 1
                            # v_hi = e4m3(v); v_lo = v - v_hi (bias folded into
                            # the output projection host-side)
                            psr = ps_v.rearrange("p (h e) -> p h e", e=64)
                            vh4w = vh_roll.rearrange(
                                "p w q h (t two) -> p w q h t two", two=2
                            )[:, par, pi, h0:h1, 64:128, pb]
                            vl4w = vl_roll.rearrange(
                                "p w q h (t two) -> p w q h t two", two=2
                            )[:, par, pi, h0:h1, 64:128, pb]
                            nc.vector.tensor_copy(vh4w, psr)
                            nc.vector.tensor_sub(vl4w, psr, vh4w)

                if cp == min(1, NCH // 2 - 1):
                    # prefetch output-projection weights mid-loop
                    for ct in range(cfg.NCT):
                        nc.sync.dma_start(
                            out=wp_sb[:, ct, :],
                            in_=wP[128 * ct : 128 * (ct + 1), :],
                        )
                    bp_src = bass.AP(
                        tensor=bP_in.tensor, offset=bP_in.offset, ap=[[0, 128], [1, C]]
                    )
                    nc.gpsimd.dma_start(out=bp_bc, in_=bp_src)
                if cp == 0:
                    # Q^T projection - emitted here so the PE chews K/V
                    # projection first while the Q inputs stream in
                    for hp in range(HP):
                        ps_q = pvp.tile([128, QW], F32, name="ps_q", tag="ps_y")
                        comp_dri(
                            ps_q,
                            lambda j, hp=hp: wqh_sb[:, j, hp, :],
                            lambda j, hp=hp: wql_sb[:, j, hp, :],
                            lambda j: xqh_sb[:, 2 * j : 2 * j + 2, :],
                            lambda j: xql_sb[:, 2 * j : 2 * j + 2, :],
                        )
                        nc.scalar.activation(
                            qt_t[:, hp, :], ps_q, Ident, bias=bq_sb[:, hp : hp + 1]
                        )
                    qproj.release()

                # ---- attention for this wave's 8 key-blocks ----------------
                nA = cfg.nb(4 * chunks[0])  # widths per half-wave
                for hp in range(HP):
                    for h in range(2):
                        hd = 2 * hp + h
                        ps_y = pvp.tile([128, 512], F32, name="ps_y", tag="ps_y")
                        # sweep 1: QK + exp + mask for all four pairs (pt
                        # tiles held); sweep 2: all eight PV matmuls back to
                        # back - PV never waits on a freshly computed mask
                        ptl = []
                        if True:
                          for pi in range(4):
                            half = pi // 2
                            ch = chunks[half]
                            pl = pi % 2  # pair within the half-wave
                            ba = 4 * ch + 2 * pl
                            n = cfg.nb(ba)
                            pt = ptp.tile([128, 1024 if not smallw else 512],
                                          E4, name=f"pt{h}", tag=f"pt{h}")
                            # big waves: blocks at offsets 0/512 in a 2-bank
                            # tile; small waves: contiguous at 0/n in 1 bank
                            sw = 1024 if not smallw else 512
                            off = 512 if not smallw else n
                            sps = aps.tile([128, sw], F32, name="sps",
                                           tag="sps")
                            blkv = pt[:, 0 : 2 * off].rearrange(
                                "p (b n) -> p b n", n=off
                            )[:, :, 0:n]
                            for pb in (0, 1):
                                bw = 4 * half + 2 * pl + pb  # kt_roll block
                                nc.tensor.matmul(
                                    sps[:, off * pb : off * pb + n],
                                    kt_roll[64 * h : 64 * (h + 1), par, hp,
                                            128 * bw : 128 * (bw + 1)],
                                    qt_t[64 * h : 64 * (h + 1), hp, 0:n],
                                    start=True,
                                    stop=True,
                                )
                            nc.scalar.activation(
                                blkv,
                                sps[:, 0 : 2 * off].rearrange(
                                    "p (b n) -> p b n", n=off
                                )[:, :, 0:n],
                                Exp, scale=EXP_SCALE, bias=ebias,
                            )
                            # causal boundary: mask last 128 q-cols of each blk
                            r0 = ba % cfg.ncores
                            pts = blkv[:, :, n - 128 : n]
                            msk = mask_sb[:, 128 * r0 : 128 * (r0 + 2)].rearrange(
                                "p (b n) -> p b n", n=128
                            )
                            meng = nc.vector if pi == 0 else nc.gpsimd
                            meng.tensor_mul(pts, pts, msk)
                            ptl.append((pi, n, blkv))
                        for pi, n, blkv in ptl:
                            # PV: two DRI matmuls (v_hi, v_lo), contraction
                            # over both blocks of the pair
                            for vroll in (vh_roll, vl_roll):
                                nc.tensor.matmul(
                                    ps_y[:, 0:n],
                                    vroll[:, par, pi, hd, :].rearrange(
                                        "p (m two) -> p m two", two=2
                                    ),
                                    blkv,
                                    start=(pi == 0 and vroll is vh_roll),
                                    stop=(pi == 3 and vroll is vl_roll),
                                    perf_mode=DRI,
                                    skip_group_check=True,
                                )
                        if first:
                            nc.vector.tensor_copy(
                                yacc[0:65, hd, 0:nA], ps_y[0:65, 0:nA]
                            )
                        else:
                            nc.vector.tensor_add(
                                yacc[0:65, hd, 0:nA],
                                yacc[0:65, hd, 0:nA],
                                ps_y[0:65, 0:nA],
                            )
                        if last:
                            # normalize this head now - overlaps the
                            # remaining heads' attention
                            rec = nrm.tile([1, QW], F32, name="rec", tag="rec")
                            rc_ps = pkv.tile([64, QW], F32, name="rc_ps",
                                             tag="pkv")
                            nc.vector.reciprocal(rec, yacc[64:65, hd, :])
                            nc.tensor.matmul(
                                rc_ps, ones11[0:1, :], rec, start=True, stop=True
                            )
                            nc.vector.tensor_mul(
                                ytf[64 * h : 64 * (h + 1), hp, :],
                                yacc[0:64, hd, :], rc_ps,
                            )

            aps.release()
            ptp.release()

        # ---- output projection -------------------------------------------
        with (
            tc.tile_pool(name="ops", bufs=2, space="PSUM") as ops,
            tc.tile_pool(name="osb", bufs=2) as osb,
        ):
            for g in range(cfg.QTC):
                ps_o = ops.tile([128, C], F32, name="ps_o", tag="ps_o")
                for n0, n1 in ((0, 512), (512, C)):
                    for hp in range(HP):
                        nc.tensor.matmul(
                            ps_o[:, n0:n1],
                            ytf[:, hp, 128 * g : 128 * (g + 1)],
                            wp_sb[:, hp, n0:n1],
                            start=(hp == 0),
                            stop=(hp == HP - 1),
                        )
                yo = osb.tile([128, C], F32, name="yo", tag="yo")
                nc.vector.tensor_add(yo, ps_o, bp_bc)
                nc.sync.dma_start(out=y[128 * g : 128 * (g + 1), :], in_=yo)


# ---------------------------------------------------------------------------
# host side
# ---------------------------------------------------------------------------


def _hilo(a):
    hi = np.asarray(a, NPE4)
    lo = np.asarray(a - hi.astype(np.float32), NPE4)
    return hi, lo


def _ileave4(W4):
    """[NCT, 128, G, M] -> interleaved [128, NCT/2, G, 2M] walrus layout."""
    A = W4[0::2]  # [NJP, 128, G, M]
    B = W4[1::2]
    il = np.empty(A.shape[:3] + (2 * A.shape[3],), A.dtype)
    il[..., 0::2] = A[..., ::-1]
    il[..., 1::2] = B[..., ::-1]
    return np.ascontiguousarray(il.transpose(1, 0, 2, 3))


def make_in_maps(x, w_attn, b_attn, w_proj, b_proj, cfg=CFG):
    T, C, H, HP, NCT = cfg.T, cfg.C, cfg.H, cfg.HP, cfg.NCT
    xT = np.ascontiguousarray(x.reshape(T, C).T).astype(np.float32)  # [C,T]
    xh, xl = _hilo(xT)

    w16 = (np.asarray(w_attn, np.float32)) * SW
    wq16, wk16, wv16 = w16[:, 0:C], w16[:, C : 2 * C], w16[:, 2 * C :]

    def wil_pair(wsec):
        h, l = _hilo(wsec)
        W4h = h.reshape(NCT, 128, HP, 128)
        W4l = l.reshape(NCT, 128, HP, 128)
        return _ileave4(W4h), _ileave4(W4l)

    wqilh, wqill = wil_pair(wq16)
    wkilh, wkill = wil_pair(wk16)

    # V moving operand: per-head reversed d order (so the strided interleaved
    # SBUF write runs with a positive stride)
    wvr = np.ascontiguousarray(
        wv16.reshape(C, H, 64)[:, :, ::-1].reshape(C, C)
    )
    wvrh, wvrl = _hilo(wvr)

    # V stationary: x k-tile pairs interleaved per 128-key tile
    X4h = xh.astype(np.float32).reshape(NCT, 128, 32, 128)
    X4l = xl.astype(np.float32).reshape(NCT, 128, 32, 128)
    xilh = _ileave4(X4h.astype(NPE4))
    xill = _ileave4(X4l.astype(NPE4))

    wP = np.asarray(w_proj, np.float32).astype(NPBF16)
    bq = np.ascontiguousarray(np.asarray(b_attn[0:C], np.float32) * SW)
    bk = np.ascontiguousarray(np.asarray(b_attn[C : 2 * C], np.float32) * SW)
    # V bias folded into the output projection (exact)
    bP = np.ascontiguousarray(
        np.asarray(b_proj, np.float32)
        + np.asarray(b_attn[2 * C :], np.float32) @ np.asarray(w_proj, np.float32)
    )

    jl = np.arange(128)[:, None]
    ii = np.arange(128)[None, :]
    in_maps = []
    for c in range(cfg.ncores):
        colsh = np.concatenate(
            [xh[:, 128 * t : 128 * (t + 1)] for t in cfg.qtiles(c)], axis=1
        )
        colsl = np.concatenate(
            [xl[:, 128 * t : 128 * (t + 1)] for t in cfg.qtiles(c)], axis=1
        )
        # multiplicative {0,1} masks on the fp8 P slabs, per key-block residue
        masks = np.stack(
            [(jl - ii <= 128 * (c - r)) for r in range(cfg.ncores)]
        ).astype(np.float32)
        maskq = np.ascontiguousarray(
            masks.transpose(1, 0, 2).reshape(128, cfg.ncores * 128)
        ).astype(NPBF16)
        in_maps.append(
            {
                "xh": xh,
                "xl": xl,
                "xilh": xilh,
                "xill": xill,
                "xqh": np.ascontiguousarray(colsh),
                "xql": np.ascontiguousarray(colsl),
                "wqilh": wqilh,
                "wqill": wqill,
                "wkilh": wkilh,
                "wkill": wkill,
                "wvrh": wvrh,
                "wvrl": wvrl,
                "wP": wP,
                "bq": bq,
                "bk": bk,
                "bP": bP,
                "maskq": maskq,
            }
        )
    return in_maps


def declare_io(nc, cfg=CFG):
    C, T, HP, NJP, QW = cfg.C, cfg.T, cfg.HP, cfg.NJP, cfg.QW
    dt = nc.dram_tensor
    ins = {
        "xh": dt("xh", [C, T], E4, kind="ExternalInput").ap(),
        "xl": dt("xl", [C, T], E4, kind="ExternalInput").ap(),
        "xilh": dt("xilh", [128, NJP, 32, 256], E4, kind="ExternalInput").ap(),
        "xill": dt("xill", [128, NJP, 32, 256], E4, kind="ExternalInput").ap(),
        "xqh": dt("xqh", [C, QW], E4, kind="ExternalInput").ap(),
        "xql": dt("xql", [C, QW], E4, kind="ExternalInput").ap(),
        "wqilh": dt("wqilh", [128, NJP, HP, 256], E4, kind="ExternalInput").ap(),
        "wqill": dt("wqill", [128, NJP, HP, 256], E4, kind="ExternalInput").ap(),
        "wkilh": dt("wkilh", [128, NJP, HP, 256], E4, kind="ExternalInput").ap(),
        "wkill": dt("wkill", [128, NJP, HP, 256], E4, kind="ExternalInput").ap(),
        "wvrh": dt("wvrh", [C, C], E4, kind="ExternalInput").ap(),
        "wvrl": dt("wvrl", [C, C], E4, kind="ExternalInput").ap(),
        "wP": dt("wP", [C, C], BF16, kind="ExternalInput").ap(),
        "bq": dt("bq", [C], F32, kind="ExternalInput").ap(),
        "bk": dt("bk", [C], F32, kind="ExternalInput").ap(),
        "bP": dt("bP", [C], F32, kind="ExternalInput").ap(),
        "maskq": dt("maskq", [128, cfg.ncores * 128], BF16,
                    kind="ExternalInput").ap(),
    }
    outs = {
        "y": dt("y", [QW, C], F32, kind="ExternalOutput").ap()
    }
    return ins, outs


def build_program(cfg=CFG, repeat=1):
    nc = bacc.Bacc("TRN2", target_bir_lowering=False, debug=False,
                   num_devices=cfg.ncores)
    ins, outs = declare_io(nc, cfg)
    with tile.TileContext(nc) as tc:
        for _ in range(repeat):
            build_kernel_v3(tc, outs, ins, cfg)
    nc.compile()
    return nc


def assemble_output(results, cfg=CFG):
    y = np.empty((cfg.T, cfg.C), np.float32)
    for c in range(cfg.ncores):
        yc = results[c]["y"]
        for g, t in enumerate(cfg.qtiles(c)):
            y[128 * t : 128 * (t + 1)] = yc[128 * g : 128 * (g + 1)]
    return y.reshape(1, cfg.T, cfg.C)


_PROGRAM = None


def kernel(x, w_attn, b_attn, w_proj, b_proj):
    global _PROGRAM
    cfg = CFG
    x = np.asarray(x, np.float32)
    if _PROGRAM is None:
        _PROGRAM = build_program(cfg)
    in_maps = make_in_maps(
        x, np.asarray(w_attn), np.asarray(b_attn), np.asarray(w_proj),
        np.asarray(b_proj), cfg
    )
    res = run_bass_kernel_spmd(_PROGRAM, in_maps, core_ids=list(range(cfg.ncores)))
    return assemble_output(res.results, cfg)


if __name__ == "__main__":
    import reference

    inputs = {k: np.asarray(v) for k, v in reference.setup_inputs().items()}
    out = kernel(**inputs)
    print("kernel output", out.shape, out.dtype)


# revision 78
# speedup vs baseline: 1.1891x; 1.0009x over previous
"""Causal self-attention (B=1, T=4096, C=768, H=12) on 8 TRN2 NeuronCores.

Strategy (single SPMD NEFF, no collectives):
  - Sequence-parallel over queries: core c owns q-tiles {c, c+8, c+16, c+24}
    (128 rows each, descending-extent column order). Slot s of every core
    processes key-blocks 8s..8s+7 (uniform instruction stream across cores);
    the true causal boundary is enforced by a per-core binary mask library
    passed as input data, so ONE program serves all 8 cores.
  - K/V/Q projections run as error-compensated fp8 DoubleRowSwInterleave
    matmuls: host splits x and 16*w_attn into e4m3 (hi, lo) pairs and the
    kernel computes xh*wh + xh*wl + xl*wh (the lo*lo term is negligible).
    Each DRI matmul contracts TWO 128-row k-tiles per pass at 0.5 cyc/row,
    so the 9-matmul group costs 0.75x the bf16 equivalent with bf16-class
    accuracy (measured end-to-end rel err 3.4e-3 for the projections).
  - Attention scores stay transposed: S^T = K @ Q^T with keys on partitions;
    exp runs PSUM->SBUF on ScalarE with scale 1/2048 (the 16x weight
    prescale squares into S) and bias -2 so exp output fits fp8e4 range.
  - P^T is written as fp8e4; PV uses DRI pairing two CONSECUTIVE KEY BLOCKS
    per pass (keys are the contraction dim), with V stored as interleaved
    fp8 (hi, lo) stationaries: y = P*vh + P*vl keeps v at bf16-class
    precision while PV runs at 2x bf16 speed. The V bias is folded into an
    effective output-projection bias on the host (exact).
  - K/V live in small rolling per-wave buffers (each wave's blocks are only
    read by that wave's attention). The softmax denominator falls out of a
    65th all-ones column of the padded-to-128 interleaved V stationary.
  - Per (head, wave): sweep 1 computes QK + exp + mask for all four block
    pairs (pt tiles buffered), sweep 2 fires the eight PV matmuls back to
    back so the PE never waits on a freshly produced mask; masks run 1/4 on
    DVE and 3/4 on GPSIMD to balance the elementwise queues.
  - Measured end-to-end relative error vs the fp32 reference: 1.2e-2
    (matching a numpy emulation of the same quantization points).
"""

import contextlib
from dataclasses import dataclass

import ml_dtypes
import numpy as np

import concourse.bass as bass
import concourse.mybir as mybir
import concourse.tile as tile
from concourse import bacc
from concourse.bass_utils import run_bass_kernel_spmd

BF16 = mybir.dt.bfloat16
F32 = mybir.dt.float32
E4 = mybir.dt.float8e4
NPBF16 = ml_dtypes.bfloat16
NPE4 = ml_dtypes.float8_e4m3
DRI = mybir.MatmulPerfMode.DoubleRowSwInterleave

SW = 16.0  # weight prescale (power of two: commutes with rounding)
EXP_SCALE = 1.0 / (8.0 * SW * SW)  # 1/(sqrt(D) * SW^2)
EXP_BIAS = -2.0  # keeps exp output within fp8e4 range; cancels in softmax


@dataclass(frozen=True)
class Cfg:
    T: int = 4096
    H: int = 12
    D: int = 64
    ncores: int = 8

    @property
    def C(self):
        return self.H * self.D

    @property
    def HP(self):  # head pairs
        return self.H // 2

    @property
    def NKB(self):  # 128-row key blocks
        return self.T // 128

    @property
    def NCH(self):  # 512-row key chunks
        return self.T // 512

    @property
    def QTC(self):  # q-tiles per core
        return self.T // 128 // self.ncores

    @property
    def QW(self):  # q columns per core
        return 128 * self.QTC

    @property
    def NCT(self):  # 128-row contraction tiles over C
        return self.C // 128

    @property
    def NJP(self):  # contraction k-tile pairs
        return self.NCT // 2

    def nb(self, b):  # valid q-column prefix width for key-block b
        return 128 * (self.QTC - b // self.ncores)

    def qtiles(self, c):  # global q-tile indices for core c, descending extent
        return [c + self.ncores * (self.QTC - 1 - g) for g in range(self.QTC)]


CFG = Cfg()


def build_kernel_v3(tc, outs, ins, cfg=CFG):
    nc = tc.nc
    C, H, HP, NJP = cfg.C, cfg.H, cfg.HP, cfg.NJP
    QW, NCH = cfg.QW, cfg.NCH
    Exp = mybir.ActivationFunctionType.Exp
    Ident = mybir.ActivationFunctionType.Identity

    xh, xl = ins["xh"], ins["xl"]
    xilh, xill = ins["xilh"], ins["xill"]
    xqh, xql = ins["xqh"], ins["xql"]
    wkilh, wkill = ins["wkilh"], ins["wkill"]
    wqilh, wqill = ins["wqilh"], ins["wqill"]
    wvrh, wvrl = ins["wvrh"], ins["wvrl"]
    wP = ins["wP"]
    bq_in, bk_in, bP_in = ins["bq"], ins["bk"], ins["bP"]
    maskq = ins["maskq"]
    y = outs["y"]

    stack = contextlib.ExitStack()
    with stack:
        persist = stack.enter_context(tc.tile_pool(name="persist", bufs=1))

        # rolling per-wave K^T (bf16, scaled 16x) and interleaved V (fp8 hi/lo)
        kt_roll = persist.tile([128, 2, HP, 1024], BF16, name="kt_roll")
        vh_roll = persist.tile([128, 2, 4, H, 256], E4, name="vh_roll")
        vl_roll = persist.tile([128, 2, 4, H, 256], E4, name="vl_roll")
        qt_t = persist.tile([128, HP, QW], BF16, name="qt_t")
        ytf = persist.tile([128, HP, QW], BF16, name="ytf")
        yacc = persist.tile([128, H, QW], F32, name="yacc")  # rows 0:65 used
        mask_sb = persist.tile([128, cfg.ncores * 128], BF16, name="mask_sb")
        wp_sb = persist.tile([128, cfg.NCT, C], BF16, name="wp_sb")
        wkh_sb = persist.tile([128, NJP, HP, 256], E4, name="wkh_sb")
        wkl_sb = persist.tile([128, NJP, HP, 256], E4, name="wkl_sb")
        wvh_sb = persist.tile([128, cfg.NCT, C], E4, name="wvh_sb")
        wvl_sb = persist.tile([128, cfg.NCT, C], E4, name="wvl_sb")
        bq_sb = persist.tile([128, HP], F32, name="bq_sb")
        bk_sb = persist.tile([128, HP], F32, name="bk_sb")
        bp_bc = persist.tile([128, C], F32, name="bp_bc")
        ones11 = persist.tile([1, 64], F32, name="ones11")
        ebias = persist.tile([128, 1], F32, name="ebias")

        nc.vector.memset(ebias, EXP_BIAS)
        nc.vector.memset(ones11, 1.0 / SW)
        # touch Exp early so the ACT table set loads during startup DMAs
        nc.scalar.activation(ones11, ones11, Exp, scale=0.0)
        nc.vector.memset(ones11, 1.0 / SW)
        # V stationaries: zero the pad region once (gpsimd memset); set the
        # ones column (logical col 64 of 128 -> interleaved positions 126-127).
        vh4 = vh_roll.rearrange("p w q h (t two) -> p w q h t two", two=2)
        vl4 = vl_roll.rearrange("p w q h (t two) -> p w q h t two", two=2)
        nc.gpsimd.memset(vh4[:, :, :, :, 0:63, :], 0.0)
        nc.gpsimd.memset(vl4[:, :, :, :, 0:64, :], 0.0)
        nc.vector.memset(vh4[:, :, :, :, 63:64, :], 1.0)

        with (
            tc.tile_pool(name="xpool", bufs=3) as xpool,
            tc.tile_pool(name="pkv", bufs=2, space="PSUM") as pkv,
            tc.tile_pool(name="pvp", bufs=2, space="PSUM") as pvp,
            tc.tile_pool(name="nrm", bufs=1) as nrm,
        ):
            ptp = tc.alloc_tile_pool(name="ptpA", bufs=6)
            qproj = tc.alloc_tile_pool(name="qproj", bufs=1)
            aps = tc.alloc_tile_pool(name="apsA", bufs=2, space="PSUM")

            xhr = xh.rearrange("(j p) t -> p j t", p=128)
            xlr = xl.rearrange("(j p) t -> p j t", p=128)

            def load_xch(ch, split=False):
                th = xpool.tile([128, cfg.NCT, 512], E4, name="xch_h", tag="xh")
                tl = xpool.tile([128, cfg.NCT, 512], E4, name="xch_l", tag="xl")
                tih = xpool.tile([128, NJP, 4, 256], E4, name="xil_h", tag="xih")
                til = xpool.tile([128, NJP, 4, 256], E4, name="xil_l", tag="xil")
                cs = 512 * ch
                if split:
                    # ct-pair pieces: the first projection group consumes
                    # pair j as soon as piece j lands
                    for j in range(NJP):
                        nc.sync.dma_start(
                            out=th[:, 2 * j : 2 * j + 2, :],
                            in_=xhr[:, 2 * j : 2 * j + 2, cs : cs + 512],
                        )
                        nc.sync.dma_start(
                            out=tl[:, 2 * j : 2 * j + 2, :],
                            in_=xlr[:, 2 * j : 2 * j + 2, cs : cs + 512],
                        )
                else:
                    nc.sync.dma_start(out=th, in_=xhr[:, :, cs : cs + 512])
                    nc.sync.dma_start(out=tl, in_=xlr[:, :, cs : cs + 512])
                nc.sync.dma_start(out=tih, in_=xilh[:, :, 4 * ch : 4 * ch + 4, :])
                nc.sync.dma_start(out=til, in_=xill[:, :, 4 * ch : 4 * ch + 4, :])
                return th, tl, tih, til

            # startup DMA order: first x chunk, K weights, V weights, masks,
            # Q inputs - so the PE never waits on a cold queue
            nc.sync.dma_start(out=wkh_sb, in_=wkilh)
            nc.sync.dma_start(out=wkl_sb, in_=wkill)
            # biases ride the idle ACT DMA queue so their issue+descgen
            # never sits ahead of the critical startup loads on SP
            nc.scalar.dma_start(out=bq_sb,
                                in_=bq_in.rearrange("(hp p) -> p hp", p=128))
            nc.scalar.dma_start(out=bk_sb,
                                in_=bk_in.rearrange("(hp p) -> p hp", p=128))
            xch_pre = {0: load_xch(0, split=True)}
            nc.sync.dma_start(
                out=wvh_sb, in_=wvrh.rearrange("(j p) t -> p j t", p=128)
            )
            nc.sync.dma_start(
                out=wvl_sb, in_=wvrl.rearrange("(j p) t -> p j t", p=128)
            )
            xch_pre[1] = load_xch(1)
            nc.sync.dma_start(out=mask_sb, in_=maskq)
            wqh_sb = qproj.tile([128, NJP, HP, 256], E4, name="wqh_sb")
            wql_sb = qproj.tile([128, NJP, HP, 256], E4, name="wql_sb")
            xqh_sb = qproj.tile([128, cfg.NCT, QW], E4, name="xqh_sb")
            xql_sb = qproj.tile([128, cfg.NCT, QW], E4, name="xql_sb")
            nc.sync.dma_start(out=wqh_sb, in_=wqilh)
            nc.sync.dma_start(out=wql_sb, in_=wqill)
            nc.sync.dma_start(out=xqh_sb, in_=xqh.rearrange("(j p) t -> p j t", p=128))
            nc.sync.dma_start(out=xql_sb, in_=xql.rearrange("(j p) t -> p j t", p=128))

            def comp_dri(ps, wil_h, wil_l, xp_h, xp_l, n0=None, n1=None):
                """9-term compensated DRI group into `ps`.

                wil_*: callables j -> stationary AP [128, 2*M interleaved]
                xp_*: callables j -> moving AP [128, 2, N]
                """
                terms = [(wil_h, xp_h), (wil_l, xp_h), (wil_h, xp_l)]
                nmm = 0
                for wf, xf in terms:
                    for j in range(NJP):
                        nc.tensor.matmul(
                            ps,
                            wf(j).rearrange("p (m two) -> p m two", two=2),
                            xf(j),
                            start=(nmm == 0),
                            stop=(nmm == 3 * NJP - 1),
                            perf_mode=DRI,
                        )
                        nmm += 1

            for cp in range(NCH // 2):
                par = cp % 2
                first, last = cp == 0, cp == NCH // 2 - 1
                chunks = (2 * cp, 2 * cp + 1)
                if cp == 2:
                    # small-n waves: swap the 2x2-bank score pool for a
                    # 4x1-bank pool (a pair fits one bank), doubling the
                    # QK->exp ping-pong depth; likewise swap the P pool to
                    # ten half-size tiles (a small pair is <= 512 B)
                    aps.release()
                    aps = tc.alloc_tile_pool(name="apsB", bufs=4,
                                             space="PSUM")
                    ptp.release()
                    ptp = tc.alloc_tile_pool(name="ptpB", bufs=14)
                smallw = cfg.nb(4 * chunks[0]) <= 256
                # ---- project K^T / V for this wave's two chunks ------------
                for half, ch in enumerate(chunks):
                    th, tl, tih, til = (
                        xch_pre.pop(ch) if ch in xch_pre else load_xch(ch)
                    )
                    sched = [("k", hp) for hp in range(HP)] + [
                        ("v", (tt, nn)) for tt in range(4) for nn in range(2)
                    ]
                    for kind, item in sched:
                      if kind == "k":
                        hp = item
                        ps_k = pkv.tile([128, 512], F32, name="ps_k", tag="pkv")
                        comp_dri(
                            ps_k,
                            lambda j, hp=hp: wkh_sb[:, j, hp, :],
                            lambda j, hp=hp: wkl_sb[:, j, hp, :],
                            lambda j: th[:, 2 * j : 2 * j + 2, :],
                            lambda j: tl[:, 2 * j : 2 * j + 2, :],
                        )
                        nc.vector.tensor_scalar_add(
                            kt_roll[:, par, hp, 512 * half : 512 * (half + 1)],
                            ps_k,
                            bk_sb[:, hp : hp + 1],
                        )
                      else:
                        tt, nn = item
                        pi = 2 * half + tt // 2  # pair index in wave
                        pb = tt % 2  # block within pair
                        for n0, n1 in (((0, 384),) if nn == 0 else ((384, 768),)):
                            h0, h1 = n0 // 64, n1 // 64
                            ps_v = pkv.tile([128, 384], F32, name="ps_v", tag="pkv")
                            nmm = 0
                            for xf, wf in (
                                (tih, wvh_sb),
                                (tih, wvl_sb),
                                (til, wvh_sb),
                            ):
                                for j in range(NJP):
                                    nc.tensor.matmul(
                                        ps_v,
                                        xf[:, j, tt, :].rearrange(
                                            "p (m two) -> p m two", two=2
                                        ),
                                        wf[:, 2 * j : 2 * j + 2, n0:n1],
                                        start=(nmm == 0),
                                        stop=(nmm == 3 * NJP - 1),
                                        perf_mode=DRI,
                                    )
                                    nmm += 1
                            # v_hi = e4m3(v); v_lo = v - v_hi (bias folded into
                            # the output projection host-side)
                            psr = ps_v.rearrange("p (h e) -> p h e", e=64)
                            vh4w = vh_roll.rearrange(
                                "p w q h (t two) -> p w q h t two", two=2
                            )[:, par, pi, h0:h1, 64:128, pb]
                            vl4w = vl_roll.rearrange(
                                "p w q h (t two) -> p w q h t two", two=2
                            )[:, par, pi, h0:h1, 64:128, pb]
                            nc.vector.tensor_copy(vh4w, psr)
                            nc.vector.tensor_sub(vl4w, psr, vh4w)

                if cp == min(1, NCH // 2 - 1):
                    # prefetch output-projection weights mid-loop
                    for ct in range(cfg.NCT):
                        nc.sync.dma_start(
                            out=wp_sb[:, ct, :],
                            in_=wP[128 * ct : 128 * (ct + 1), :],
                        )
                    bp_src = bass.AP(
                        tensor=bP_in.tensor, offset=bP_in.offset, ap=[[0, 128], [1, C]]
                    )
                    nc.gpsimd.dma_start(out=bp_bc, in_=bp_src)
                if cp == 0:
                    # Q^T projection - emitted here so the PE chews K/V
                    # projection first while the Q inputs stream in
                    for hp in range(HP):
                        ps_q = pvp.tile([128, QW], F32, name="ps_q", tag="ps_y")
                        comp_dri(
                            ps_q,
                            lambda j, hp=hp: wqh_sb[:, j, hp, :],
                            lambda j, hp=hp: wql_sb[:, j, hp, :],
                            lambda j: xqh_sb[:, 2 * j : 2 * j + 2, :],
                            lambda j: xql_sb[:, 2 * j : 2 * j + 2, :],
                        )
                        nc.scalar.activation(
                            qt_t[:, hp, :], ps_q, Ident, bias=bq_sb[:, hp : hp + 1]
                        )
                    qproj.release()

                # ---- attention for this wave's 8 key-blocks ----------------
                nA = cfg.nb(4 * chunks[0])  # widths per half-wave
                for hp in range(HP):
                    for h in range(2):
                        hd = 2 * hp + h
                        ps_y = pvp.tile([128, 512], F32, name="ps_y", tag="ps_y")
                        # sweep 1: QK + exp + mask for all four pairs (pt
                        # tiles held); sweep 2: all eight PV matmuls back to
                        # back - PV never waits on a freshly computed mask
                        ptl = []
                        if True:
                          for pi in range(4):
                            half = pi // 2
                            ch = chunks[half]
                            pl = pi % 2  # pair within the half-wave
                            ba = 4 * ch + 2 * pl
                            n = cfg.nb(ba)
                            pt = ptp.tile([128, 1024 if not smallw else 512],
                                          E4, name=f"pt{h}", tag=f"pt{h}")
                            # big waves: blocks at offsets 0/512 in a 2-bank
                            # tile; small waves: contiguous at 0/n in 1 bank
                            sw = 1024 if not smallw else 512
                            off = 512 if not smallw else n
                            sps = aps.tile([128, sw], F32, name="sps",
                                           tag="sps")
                            blkv = pt[:, 0 : 2 * off].rearrange(
                                "p (b n) -> p b n", n=off
                            )[:, :, 0:n]
                            for pb in (0, 1):
                                bw = 4 * half + 2 * pl + pb  # kt_roll block
                                nc.tensor.matmul(
                                    sps[:, off * pb : off * pb + n],
                                    kt_roll[64 * h : 64 * (h + 1), par, hp,
                                            128 * bw : 128 * (bw + 1)],
                                    qt_t[64 * h : 64 * (h + 1), hp, 0:n],
                                    start=True,
                                    stop=True,
                                )
                            nc.scalar.activation(
                                blkv,
                                sps[:, 0 : 2 * off].rearrange(
                                    "p (b n) -> p b n", n=off
                                )[:, :, 0:n],
                                Exp, scale=EXP_SCALE, bias=ebias,
                            )
                            # causal boundary: mask last 128 q-cols of each blk
                            r0 = ba % cfg.ncores
                            pts = blkv[:, :, n - 128 : n]
                            msk = mask_sb[:, 128 * r0 : 128 * (r0 + 2)].rearrange(
                                "p (b n) -> p b n", n=128
                            )
                            meng = nc.vector if pi == 0 else nc.gpsimd
                            meng.tensor_mul(pts, pts, msk)
                            ptl.append((pi, n, blkv))
                        for pi, n, blkv in ptl:
                            # PV: two DRI matmuls (v_hi, v_lo), contraction
                            # over both blocks of the pair
                            for vroll in (vh_roll, vl_roll):
                                nc.tensor.matmul(
                                    ps_y[:, 0:n],
                                    vroll[:, par, pi, hd, :].rearrange(
                                        "p (m two) -> p m two", two=2
                                    ),
                                    blkv,
                                    start=(pi == 0 and vroll is vh_roll),
                                    stop=(pi == 3 and vroll is vl_roll),
                                    perf_mode=DRI,
                                    skip_group_check=True,
                                )
                        if first:
                            nc.vector.tensor_copy(
                                yacc[0:65, hd, 0:nA], ps_y[0:65, 0:nA]
                            )
                        else:
                            nc.vector.tensor_add(
                                yacc[0:65, hd, 0:nA],
                                yacc[0:65, hd, 0:nA],
                                ps_y[0:65, 0:nA],
                            )
                        if last:
                            # normalize this head now - overlaps the
                            # remaining heads' attention
                            rec = nrm.tile([1, QW], F32, name="rec", tag="rec")
                            rc_ps = pkv.tile([64, QW], F32, name="rc_ps",
                                             tag="pkv")
                            nc.vector.reciprocal(rec, yacc[64:65, hd, :])
                            nc.tensor.matmul(
                                rc_ps, ones11[0:1, :], rec, start=True, stop=True
                            )
                            nc.vector.tensor_mul(
                                ytf[64 * h : 64 * (h + 1), hp, :],
                                yacc[0:64, hd, :], rc_ps,
                            )

            aps.release()
            ptp.release()

        # ---- output projection -------------------------------------------
        with (
            tc.tile_pool(name="ops", bufs=2, space="PSUM") as ops,
            tc.tile_pool(name="osb", bufs=2) as osb,
        ):
            for g in range(cfg.QTC):
                ps_o = ops.tile([128, C], F32, name="ps_o", tag="ps_o")
                for n0, n1 in ((0, 512), (512, C)):
                    for hp in range(HP):
                        nc.tensor.matmul(
                            ps_o[:, n0:n1],
                            ytf[:, hp, 128 * g : 128 * (g + 1)],
                            wp_sb[:, hp, n0:n1],
                            start=(hp == 0),
                            stop=(hp == HP - 1),
                        )
                yo = osb.tile([128, C], F32, name="yo", tag="yo")
                nc.vector.tensor_add(yo, ps_o, bp_bc)
                nc.sync.dma_start(out=y[128 * g : 128 * (g + 1), :], in_=yo)


# ---------------------------------------------------------------------------
# host side
# ---------------------------------------------------------------------------


def _hilo(a):
    hi = np.asarray(a, NPE4)
    lo = np.asarray(a - hi.astype(np.float32), NPE4)
    return hi, lo


def _ileave4(W4):
    """[NCT, 128, G, M] -> interleaved [128, NCT/2, G, 2M] walrus layout."""
    A = W4[0::2]  # [NJP, 128, G, M]
    B = W4[1::2]
    il = np.empty(A.shape[:3] + (2 * A.shape[3],), A.dtype)
    il[..., 0::2] = A[..., ::-1]
    il[..., 1::2] = B[..., ::-1]
    return np.ascontiguousarray(il.transpose(1, 0, 2, 3))


def make_in_maps(x, w_attn, b_attn, w_proj, b_proj, cfg=CFG):
    T, C, H, HP, NCT = cfg.T, cfg.C, cfg.H, cfg.HP, cfg.NCT
    xT = np.ascontiguousarray(x.reshape(T, C).T).astype(np.float32)  # [C,T]
    xh, xl = _hilo(xT)

    w16 = (np.asarray(w_attn, np.float32)) * SW
    wq16, wk16, wv16 = w16[:, 0:C], w16[:, C : 2 * C], w16[:, 2 * C :]

    def wil_pair(wsec):
        h, l = _hilo(wsec)
        W4h = h.reshape(NCT, 128, HP, 128)
        W4l = l.reshape(NCT, 128, HP, 128)
        return _ileave4(W4h), _ileave4(W4l)

    wqilh, wqill = wil_pair(wq16)
    wkilh, wkill = wil_pair(wk16)

    # V moving operand: per-head reversed d order (so the strided interleaved
    # SBUF write runs with a positive stride)
    wvr = np.ascontiguousarray(
        wv16.reshape(C, H, 64)[:, :, ::-1].reshape(C, C)
    )
    wvrh, wvrl = _hilo(wvr)

    # V stationary: x k-tile pairs interleaved per 128-key tile
    X4h = xh.astype(np.float32).reshape(NCT, 128, 32, 128)
    X4l = xl.astype(np.float32).reshape(NCT, 128, 32, 128)
    xilh = _ileave4(X4h.astype(NPE4))
    xill = _ileave4(X4l.astype(NPE4))

    wP = np.asarray(w_proj, np.float32).astype(NPBF16)
    bq = np.ascontiguousarray(np.asarray(b_attn[0:C], np.float32) * SW)
    bk = np.ascontiguousarray(np.asarray(b_attn[C : 2 * C], np.float32) * SW)
    # V bias folded into the output projection (exact)
    bP = np.ascontiguousarray(
        np.asarray(b_proj, np.float32)
        + np.asarray(b_attn[2 * C :], np.float32) @ np.asarray(w_proj, np.float32)
    )

    jl = np.arange(128)[:, None]
    ii = np.arange(128)[None, :]
    in_maps = []
    for c in range(cfg.ncores):
        colsh = np.concatenate(
            [xh[:, 128 * t : 128 * (t + 1)] for t in cfg.qtiles(c)], axis=1
        )
        colsl = np.concatenate(
            [xl[:, 128 * t : 128 * (t + 1)] for t in cfg.qtiles(c)], axis=1
        )
        # multiplicative {0,1} masks on the fp8 P slabs, per key-block residue
        masks = np.stack(
            [(jl - ii <= 128 * (c - r)) for r in range(cfg.ncores)]
        ).astype(np.float32)
        maskq = np.ascontiguousarray(
            masks.transpose(1, 0, 2).reshape(128, cfg.ncores * 128)
        ).astype(NPBF16)
        in_maps.append(
            {
                "xh": xh,
                "xl": xl,
                "xilh": xilh,
                "xill": xill,
                "xqh": np.ascontiguousarray(colsh),
                "xql": np.ascontiguousarray(colsl),
                "wqilh": wqilh,
                "wqill": wqill,
                "wkilh": wkilh,
                "wkill": wkill,
                "wvrh": wvrh,
                "wvrl": wvrl,
                "wP": wP,
                "bq": bq,
                "bk": bk,
                "bP": bP,
                "maskq": maskq,
            }
        )
    return in_maps


def declare_io(nc, cfg=CFG):
    C, T, HP, NJP, QW = cfg.C, cfg.T, cfg.HP, cfg.NJP, cfg.QW
    dt = nc.dram_tensor
    ins = {
        "xh": dt("xh", [C, T], E4, kind="ExternalInput").ap(),
        "xl": dt("xl", [C, T], E4, kind="ExternalInput").ap(),
        "xilh": dt("xilh", [128, NJP, 32, 256], E4, kind="ExternalInput").ap(),
        "xill": dt("xill", [128, NJP, 32, 256], E4, kind="ExternalInput").ap(),
        "xqh": dt("xqh", [C, QW], E4, kind="ExternalInput").ap(),
        "xql": dt("xql", [C, QW], E4, kind="ExternalInput").ap(),
        "wqilh": dt("wqilh", [128, NJP, HP, 256], E4, kind="ExternalInput").ap(),
        "wqill": dt("wqill", [128, NJP, HP, 256], E4, kind="ExternalInput").ap(),
        "wkilh": dt("wkilh", [128, NJP, HP, 256], E4, kind="ExternalInput").ap(),
        "wkill": dt("wkill", [128, NJP, HP, 256], E4, kind="ExternalInput").ap(),
        "wvrh": dt("wvrh", [C, C], E4, kind="ExternalInput").ap(),
        "wvrl": dt("wvrl", [C, C], E4, kind="ExternalInput").ap(),
        "wP": dt("wP", [C, C], BF16, kind="ExternalInput").ap(),
        "bq": dt("bq", [C], F32, kind="ExternalInput").ap(),
        "bk": dt("bk", [C], F32, kind="ExternalInput").ap(),
        "bP": dt("bP", [C], F32, kind="ExternalInput").ap(),
        "maskq": dt("maskq", [128, cfg.ncores * 128], BF16,
                    kind="ExternalInput").ap(),
    }
    outs = {
        "y": dt("y", [QW, C], F32, kind="ExternalOutput").ap()
    }
    return ins, outs


def build_program(cfg=CFG, repeat=1):
    nc = bacc.Bacc("TRN2", target_bir_lowering=False, debug=False,
                   num_devices=cfg.ncores)
    ins, outs = declare_io(nc, cfg)
    with tile.TileContext(nc) as tc:
        for _ in range(repeat):
            build_kernel_v3(tc, outs, ins, cfg)
    nc.compile()
    return nc


def assemble_output(results, cfg=CFG):
    y = np.empty((cfg.T, cfg.C), np.float32)
    for c in range(cfg.ncores):
        yc = results[c]["y"]
        for g, t in enumerate(cfg.qtiles(c)):
            y[128 * t : 128 * (t + 1)] = yc[128 * g : 128 * (g + 1)]
    return y.reshape(1, cfg.T, cfg.C)


_PROGRAM = None


def kernel(x, w_attn, b_attn, w_proj, b_proj):
    global _PROGRAM
    cfg = CFG
    x = np.asarray(x, np.float32)
    if _PROGRAM is None:
        _PROGRAM = build_program(cfg)
    in_maps = make_in_maps(
        x, np.asarray(w_attn), np.asarray(b_attn), np.asarray(w_proj),
        np.asarray(b_proj), cfg
    )
    res = run_bass_kernel_spmd(_PROGRAM, in_maps, core_ids=list(range(cfg.ncores)))
    return assemble_output(res.results, cfg)


if __name__ == "__main__":
    import reference

    inputs = {k: np.asarray(v) for k, v in reference.setup_inputs().items()}
    out = kernel(**inputs)
    print("kernel output", out.shape, out.dtype)


# revision 79
# speedup vs baseline: 1.2285x; 1.0331x over previous
"""Causal self-attention (B=1, T=4096, C=768, H=12) on 8 TRN2 NeuronCores.

Strategy (single SPMD NEFF, no collectives):
  - Sequence-parallel over queries: core c owns q-tiles {c, c+8, c+16, c+24}
    (128 rows each, descending-extent column order). Slot s of every core
    processes key-blocks 8s..8s+7 (uniform instruction stream across cores);
    the true causal boundary is enforced by a per-core binary mask library
    passed as input data, so ONE program serves all 8 cores.
  - K/V/Q projections run as error-compensated fp8 DoubleRowSwInterleave
    matmuls: host splits x and 16*w_attn into e4m3 (hi, lo) pairs and the
    kernel computes xh*wh + xh*wl + xl*wh (the lo*lo term is negligible).
    Each DRI matmul contracts TWO 128-row k-tiles per pass at 0.5 cyc/row,
    so the 9-matmul group costs 0.75x the bf16 equivalent with bf16-class
    accuracy (measured end-to-end rel err 3.4e-3 for the projections).
  - Attention scores stay transposed: S^T = K @ Q^T with keys on partitions;
    exp runs PSUM->SBUF on ScalarE with scale 1/2048 (the 16x weight
    prescale squares into S) and bias -2 so exp output fits fp8e4 range.
  - P^T is written as fp8e4; PV uses DRI pairing two CONSECUTIVE KEY BLOCKS
    per pass (keys are the contraction dim), with V stored as interleaved
    fp8 (hi, lo) stationaries: y = P*vh + P*vl keeps v at bf16-class
    precision while PV runs at 2x bf16 speed. The V bias is folded into an
    effective output-projection bias on the host (exact).
  - K/V live in small rolling per-wave buffers (each wave's blocks are only
    read by that wave's attention). The softmax denominator falls out of a
    65th all-ones column of the padded-to-128 interleaved V stationary.
  - Per (head, wave): sweep 1 computes QK + exp + mask for all four block
    pairs (pt tiles buffered), sweep 2 fires the eight PV matmuls back to
    back so the PE never waits on a freshly produced mask; masks run 1/4 on
    DVE and 3/4 on GPSIMD to balance the elementwise queues.
  - Measured end-to-end relative error vs the fp32 reference: 1.2e-2
    (matching a numpy emulation of the same quantization points).
"""

import contextlib
from dataclasses import dataclass

import ml_dtypes
import numpy as np

import concourse.bass as bass
import concourse.mybir as mybir
import concourse.tile as tile
from concourse import bacc
from concourse.bass_utils import run_bass_kernel_spmd

BF16 = mybir.dt.bfloat16
F32 = mybir.dt.float32
E4 = mybir.dt.float8e4
NPBF16 = ml_dtypes.bfloat16
NPE4 = ml_dtypes.float8_e4m3
DRI = mybir.MatmulPerfMode.DoubleRowSwInterleave

SW = 16.0  # weight prescale (power of two: commutes with rounding)
EXP_SCALE = 1.0 / (8.0 * SW * SW)  # 1/(sqrt(D) * SW^2)
EXP_BIAS = -2.0  # keeps exp output within fp8e4 range; cancels in softmax


@dataclass(frozen=True)
class Cfg:
    T: int = 4096
    H: int = 12
    D: int = 64
    ncores: int = 8

    @property
    def C(self):
        return self.H * self.D

    @property
    def HP(self):  # head pairs
        return self.H // 2

    @property
    def NKB(self):  # 128-row key blocks
        return self.T // 128

    @property
    def NCH(self):  # 512-row key chunks
        return self.T // 512

    @property
    def QTC(self):  # q-tiles per core
        return self.T // 128 // self.ncores

    @property
    def QW(self):  # q columns per core
        return 128 * self.QTC

    @property
    def NCT(self):  # 128-row contraction tiles over C
        return self.C // 128

    @property
    def NJP(self):  # contraction k-tile pairs
        return self.NCT // 2

    def nb(self, b):  # valid q-column prefix width for key-block b
        return 128 * (self.QTC - b // self.ncores)

    def qtiles(self, c):  # global q-tile indices for core c, descending extent
        return [c + self.ncores * (self.QTC - 1 - g) for g in range(self.QTC)]


CFG = Cfg()


def build_kernel_v3(tc, outs, ins, cfg=CFG):
    nc = tc.nc
    C, H, HP, NJP = cfg.C, cfg.H, cfg.HP, cfg.NJP
    QW, NCH = cfg.QW, cfg.NCH
    Exp = mybir.ActivationFunctionType.Exp
    Ident = mybir.ActivationFunctionType.Identity

    xh, xl = ins["xh"], ins["xl"]
    xilh, xill = ins["xilh"], ins["xill"]
    xqh, xql = ins["xqh"], ins["xql"]
    wkilh, wkill = ins["wkilh"], ins["wkill"]
    wqilh, wqill = ins["wqilh"], ins["wqill"]
    wvrh, wvrl = ins["wvrh"], ins["wvrl"]
    wP = ins["wP"]
    bq_in, bk_in, bP_in = ins["bq"], ins["bk"], ins["bP"]
    maskq = ins["maskq"]
    y = outs["y"]

    stack = contextlib.ExitStack()
    with stack:
        persist = stack.enter_context(tc.tile_pool(name="persist", bufs=1))

        # rolling per-wave K^T (bf16, scaled 16x) and interleaved V (fp8 hi/lo)
        kt_roll = persist.tile([128, 2, HP, 1024], BF16, name="kt_roll")
        vh_roll = persist.tile([128, 2, 4, H, 256], E4, name="vh_roll")
        vl_roll = persist.tile([128, 2, 4, H, 256], E4, name="vl_roll")
        qt_t = persist.tile([128, HP, QW], BF16, name="qt_t")
        ytf = persist.tile([128, HP, QW], BF16, name="ytf")
        yacc = persist.tile([128, H, QW], F32, name="yacc")  # rows 0:65 used
        mask_sb = persist.tile([128, cfg.ncores * 128], BF16, name="mask_sb")
        wp_sb = persist.tile([128, cfg.NCT, C], BF16, name="wp_sb")
        wkh_sb = persist.tile([128, NJP, HP, 256], E4, name="wkh_sb")
        wkl_sb = persist.tile([128, NJP, HP, 256], E4, name="wkl_sb")
        wvh_sb = persist.tile([128, cfg.NCT, C], E4, name="wvh_sb")
        wvl_sb = persist.tile([128, cfg.NCT, C], E4, name="wvl_sb")
        bq_sb = persist.tile([128, HP], F32, name="bq_sb")
        bk_sb = persist.tile([128, HP], F32, name="bk_sb")
        bp_bc = persist.tile([128, C], F32, name="bp_bc")
        ones11 = persist.tile([1, 64], F32, name="ones11")
        ebias = persist.tile([128, 1], F32, name="ebias")

        nc.vector.memset(ebias, EXP_BIAS)
        nc.vector.memset(ones11, 1.0 / SW)
        # touch Exp early so the ACT table set loads during startup DMAs
        nc.scalar.activation(ones11, ones11, Exp, scale=0.0)
        nc.vector.memset(ones11, 1.0 / SW)
        # V stationaries: zero the pad region once (gpsimd memset); set the
        # ones column (logical col 64 of 128 -> interleaved positions 126-127).
        vh4 = vh_roll.rearrange("p w q h (t two) -> p w q h t two", two=2)
        vl4 = vl_roll.rearrange("p w q h (t two) -> p w q h t two", two=2)
        nc.gpsimd.memset(vh4[:, :, :, :, 0:63, :], 0.0)
        nc.gpsimd.memset(vl4[:, :, :, :, 0:64, :], 0.0)
        nc.vector.memset(vh4[:, :, :, :, 63:64, :], 1.0)

        with (
            tc.tile_pool(name="xpool", bufs=3) as xpool,
            tc.tile_pool(name="pkv", bufs=2, space="PSUM") as pkv,
            tc.tile_pool(name="pvp", bufs=2, space="PSUM") as pvp,
            tc.tile_pool(name="nrm", bufs=2) as nrm,
        ):
            ptp = tc.alloc_tile_pool(name="ptpA", bufs=6)
            qproj = tc.alloc_tile_pool(name="qproj", bufs=1)
            aps = tc.alloc_tile_pool(name="apsA", bufs=2, space="PSUM")

            xhr = xh.rearrange("(j p) t -> p j t", p=128)
            xlr = xl.rearrange("(j p) t -> p j t", p=128)

            def load_xch(ch, split=False):
                th = xpool.tile([128, cfg.NCT, 512], E4, name="xch_h", tag="xh")
                tl = xpool.tile([128, cfg.NCT, 512], E4, name="xch_l", tag="xl")
                tih = xpool.tile([128, NJP, 4, 256], E4, name="xil_h", tag="xih")
                til = xpool.tile([128, NJP, 4, 256], E4, name="xil_l", tag="xil")
                cs = 512 * ch
                if split:
                    # ct-pair pieces: the first projection group consumes
                    # pair j as soon as piece j lands
                    for j in range(NJP):
                        nc.sync.dma_start(
                            out=th[:, 2 * j : 2 * j + 2, :],
                            in_=xhr[:, 2 * j : 2 * j + 2, cs : cs + 512],
                        )
                        nc.sync.dma_start(
                            out=tl[:, 2 * j : 2 * j + 2, :],
                            in_=xlr[:, 2 * j : 2 * j + 2, cs : cs + 512],
                        )
                else:
                    nc.sync.dma_start(out=th, in_=xhr[:, :, cs : cs + 512])
                    nc.sync.dma_start(out=tl, in_=xlr[:, :, cs : cs + 512])
                nc.sync.dma_start(out=tih, in_=xilh[:, :, 4 * ch : 4 * ch + 4, :])
                nc.sync.dma_start(out=til, in_=xill[:, :, 4 * ch : 4 * ch + 4, :])
                return th, tl, tih, til

            # startup DMA order: first x chunk, K weights, V weights, masks,
            # Q inputs - so the PE never waits on a cold queue
            nc.sync.dma_start(out=wkh_sb, in_=wkilh)
            nc.sync.dma_start(out=wkl_sb, in_=wkill)
            # biases ride the idle ACT DMA queue so their issue+descgen
            # never sits ahead of the critical startup loads on SP
            nc.scalar.dma_start(out=bq_sb,
                                in_=bq_in.rearrange("(hp p) -> p hp", p=128))
            nc.scalar.dma_start(out=bk_sb,
                                in_=bk_in.rearrange("(hp p) -> p hp", p=128))
            xch_pre = {0: load_xch(0, split=True)}
            nc.sync.dma_start(
                out=wvh_sb, in_=wvrh.rearrange("(j p) t -> p j t", p=128)
            )
            nc.sync.dma_start(
                out=wvl_sb, in_=wvrl.rearrange("(j p) t -> p j t", p=128)
            )
            xch_pre[1] = load_xch(1)
            nc.sync.dma_start(out=mask_sb, in_=maskq)
            wqh_sb = qproj.tile([128, NJP, HP, 256], E4, name="wqh_sb")
            wql_sb = qproj.tile([128, NJP, HP, 256], E4, name="wql_sb")
            xqh_sb = qproj.tile([128, cfg.NCT, QW], E4, name="xqh_sb")
            xql_sb = qproj.tile([128, cfg.NCT, QW], E4, name="xql_sb")
            nc.sync.dma_start(out=wqh_sb, in_=wqilh)
            nc.sync.dma_start(out=wql_sb, in_=wqill)
            nc.sync.dma_start(out=xqh_sb, in_=xqh.rearrange("(j p) t -> p j t", p=128))
            nc.sync.dma_start(out=xql_sb, in_=xql.rearrange("(j p) t -> p j t", p=128))

            def comp_dri(ps, wil_h, wil_l, xp_h, xp_l, n0=None, n1=None):
                """9-term compensated DRI group into `ps`.

                wil_*: callables j -> stationary AP [128, 2*M interleaved]
                xp_*: callables j -> moving AP [128, 2, N]
                """
                terms = [(wil_h, xp_h), (wil_l, xp_h), (wil_h, xp_l)]
                nmm = 0
                for wf, xf in terms:
                    for j in range(NJP):
                        nc.tensor.matmul(
                            ps,
                            wf(j).rearrange("p (m two) -> p m two", two=2),
                            xf(j),
                            start=(nmm == 0),
                            stop=(nmm == 3 * NJP - 1),
                            perf_mode=DRI,
                        )
                        nmm += 1

            for cp in range(NCH // 2):
                par = cp % 2
                first, last = cp == 0, cp == NCH // 2 - 1
                chunks = (2 * cp, 2 * cp + 1)
                if cp == 2:
                    # small-n waves: swap the 2x2-bank score pool for a
                    # 4x1-bank pool (a pair fits one bank), doubling the
                    # QK->exp ping-pong depth; likewise swap the P pool to
                    # ten half-size tiles (a small pair is <= 512 B)
                    aps.release()
                    aps = tc.alloc_tile_pool(name="apsB", bufs=4,
                                             space="PSUM")
                    ptp.release()
                    ptp = tc.alloc_tile_pool(name="ptpB", bufs=14)
                smallw = cfg.nb(4 * chunks[0]) <= 256
                # ---- project K^T / V for this wave's two chunks ------------
                for half, ch in enumerate(chunks):
                    th, tl, tih, til = (
                        xch_pre.pop(ch) if ch in xch_pre else load_xch(ch)
                    )
                    sched = [("k", hp) for hp in range(HP)] + [
                        ("v", (tt, nn)) for tt in range(4) for nn in range(2)
                    ]
                    for kind, item in sched:
                      if kind == "k":
                        hp = item
                        ps_k = pkv.tile([128, 512], F32, name="ps_k", tag="pkv")
                        comp_dri(
                            ps_k,
                            lambda j, hp=hp: wkh_sb[:, j, hp, :],
                            lambda j, hp=hp: wkl_sb[:, j, hp, :],
                            lambda j: th[:, 2 * j : 2 * j + 2, :],
                            lambda j: tl[:, 2 * j : 2 * j + 2, :],
                        )
                        nc.vector.tensor_scalar_add(
                            kt_roll[:, par, hp, 512 * half : 512 * (half + 1)],
                            ps_k,
                            bk_sb[:, hp : hp + 1],
                        )
                      else:
                        tt, nn = item
                        pi = 2 * half + tt // 2  # pair index in wave
                        pb = tt % 2  # block within pair
                        for n0, n1 in (((0, 384),) if nn == 0 else ((384, 768),)):
                            h0, h1 = n0 // 64, n1 // 64
                            ps_v = pkv.tile([128, 384], F32, name="ps_v", tag="pkv")
                            nmm = 0
                            for xf, wf in (
                                (tih, wvh_sb),
                                (tih, wvl_sb),
                                (til, wvh_sb),
                            ):
                                for j in range(NJP):
                                    nc.tensor.matmul(
                                        ps_v,
                                        xf[:, j, tt, :].rearrange(
                                            "p (m two) -> p m two", two=2
                                        ),
                                        wf[:, 2 * j : 2 * j + 2, n0:n1],
                                        start=(nmm == 0),
                                        stop=(nmm == 3 * NJP - 1),
                                        perf_mode=DRI,
                                    )
                                    nmm += 1
                            # v_hi = e4m3(v); v_lo = v - v_hi (bias folded into
                            # the output projection host-side)
                            psr = ps_v.rearrange("p (h e) -> p h e", e=64)
                            vh4w = vh_roll.rearrange(
                                "p w q h (t two) -> p w q h t two", two=2
                            )[:, par, pi, h0:h1, 64:128, pb]
                            vl4w = vl_roll.rearrange(
                                "p w q h (t two) -> p w q h t two", two=2
                            )[:, par, pi, h0:h1, 64:128, pb]
                            nc.vector.tensor_copy(vh4w, psr)
                            nc.vector.tensor_sub(vl4w, psr, vh4w)

                if cp == min(1, NCH // 2 - 1):
                    # prefetch output-projection weights mid-loop
                    for ct in range(cfg.NCT):
                        nc.sync.dma_start(
                            out=wp_sb[:, ct, :],
                            in_=wP[128 * ct : 128 * (ct + 1), :],
                        )
                    bp_src = bass.AP(
                        tensor=bP_in.tensor, offset=bP_in.offset, ap=[[0, 128], [1, C]]
                    )
                    nc.gpsimd.dma_start(out=bp_bc, in_=bp_src)
                if cp == 0:
                    # Q^T projection - emitted here so the PE chews K/V
                    # projection first while the Q inputs stream in
                    for hp in range(HP):
                        ps_q = pvp.tile([128, QW], F32, name="ps_q", tag="ps_y")
                        comp_dri(
                            ps_q,
                            lambda j, hp=hp: wqh_sb[:, j, hp, :],
                            lambda j, hp=hp: wql_sb[:, j, hp, :],
                            lambda j: xqh_sb[:, 2 * j : 2 * j + 2, :],
                            lambda j: xql_sb[:, 2 * j : 2 * j + 2, :],
                        )
                        nc.scalar.activation(
                            qt_t[:, hp, :], ps_q, Ident, bias=bq_sb[:, hp : hp + 1]
                        )
                    qproj.release()

                # ---- attention for this wave's 8 key-blocks ----------------
                nA = cfg.nb(4 * chunks[0])  # widths per half-wave
                for hp in range(HP):
                    for h in range(2):
                        hd = 2 * hp + h
                        ps_y = pvp.tile([128, 512], F32, name="ps_y", tag="ps_y")
                        # sweep 1: QK + exp + mask for all four pairs (pt
                        # tiles held); sweep 2: all eight PV matmuls back to
                        # back - PV never waits on a freshly computed mask
                        ptl = []
                        if True:
                          for pi in range(4):
                            half = pi // 2
                            ch = chunks[half]
                            pl = pi % 2  # pair within the half-wave
                            ba = 4 * ch + 2 * pl
                            n = cfg.nb(ba)
                            pt = ptp.tile([128, 1024 if not smallw else 512],
                                          E4, name=f"pt{h}", tag=f"pt{h}")
                            # big waves: blocks at offsets 0/512 in a 2-bank
                            # tile; small waves: contiguous at 0/n in 1 bank
                            sw = 1024 if not smallw else 512
                            off = 512 if not smallw else n
                            sps = aps.tile([128, sw], F32, name="sps",
                                           tag="sps")
                            blkv = pt[:, 0 : 2 * off].rearrange(
                                "p (b n) -> p b n", n=off
                            )[:, :, 0:n]
                            for pb in (0, 1):
                                bw = 4 * half + 2 * pl + pb  # kt_roll block
                                nc.tensor.matmul(
                                    sps[:, off * pb : off * pb + n],
                                    kt_roll[64 * h : 64 * (h + 1), par, hp,
                                            128 * bw : 128 * (bw + 1)],
                                    qt_t[64 * h : 64 * (h + 1), hp, 0:n],
                                    start=True,
                                    stop=True,
                                )
                            nc.scalar.activation(
                                blkv,
                                sps[:, 0 : 2 * off].rearrange(
                                    "p (b n) -> p b n", n=off
                                )[:, :, 0:n],
                                Exp, scale=EXP_SCALE, bias=ebias,
                            )
                            # causal boundary: mask last 128 q-cols of each blk
                            r0 = ba % cfg.ncores
                            pts = blkv[:, :, n - 128 : n]
                            msk = mask_sb[:, 128 * r0 : 128 * (r0 + 2)].rearrange(
                                "p (b n) -> p b n", n=128
                            )
                            meng = nc.vector if pi == 0 else nc.gpsimd
                            meng.tensor_mul(pts, pts, msk)
                            ptl.append((pi, n, blkv))
                        for pi, n, blkv in ptl:
                            # PV: two DRI matmuls (v_hi, v_lo), contraction
                            # over both blocks of the pair
                            for vroll in (vh_roll, vl_roll):
                                nc.tensor.matmul(
                                    ps_y[:, 0:n],
                                    vroll[:, par, pi, hd, :].rearrange(
                                        "p (m two) -> p m two", two=2
                                    ),
                                    blkv,
                                    start=(pi == 0 and vroll is vh_roll),
                                    stop=(pi == 3 and vroll is vl_roll),
                                    perf_mode=DRI,
                                    skip_group_check=True,
                                )
                        if first:
                            nc.vector.tensor_copy(
                                yacc[0:65, hd, 0:nA], ps_y[0:65, 0:nA]
                            )
                        else:
                            nc.vector.tensor_add(
                                yacc[0:65, hd, 0:nA],
                                yacc[0:65, hd, 0:nA],
                                ps_y[0:65, 0:nA],
                            )
                        if last:
                            # normalize this head now - overlaps the
                            # remaining heads' attention
                            rec = nrm.tile([1, QW], F32, name="rec", tag="rec")
                            rc_ps = pkv.tile([64, QW], F32, name="rc_ps",
                                             tag="pkv")
                            nc.vector.reciprocal(rec, yacc[64:65, hd, :])
                            nc.tensor.matmul(
                                rc_ps, ones11[0:1, :], rec, start=True, stop=True
                            )
                            nc.vector.tensor_mul(
                                ytf[64 * h : 64 * (h + 1), hp, :],
                                yacc[0:64, hd, :], rc_ps,
                            )

            aps.release()
            ptp.release()

        # ---- output projection -------------------------------------------
        with (
            tc.tile_pool(name="ops", bufs=2, space="PSUM") as ops,
            tc.tile_pool(name="osb", bufs=2) as osb,
        ):
            for g in range(cfg.QTC):
                ps_o = ops.tile([128, C], F32, name="ps_o", tag="ps_o")
                for n0, n1 in ((0, 512), (512, C)):
                    for hp in range(HP):
                        nc.tensor.matmul(
                            ps_o[:, n0:n1],
                            ytf[:, hp, 128 * g : 128 * (g + 1)],
                            wp_sb[:, hp, n0:n1],
                            start=(hp == 0),
                            stop=(hp == HP - 1),
                        )
                yo = osb.tile([128, C], F32, name="yo", tag="yo")
                nc.vector.tensor_add(yo, ps_o, bp_bc)
                nc.sync.dma_start(out=y[128 * g : 128 * (g + 1), :], in_=yo)


# ---------------------------------------------------------------------------
# host side
# ---------------------------------------------------------------------------


def _hilo(a):
    hi = np.asarray(a, NPE4)
    lo = np.asarray(a - hi.astype(np.float32), NPE4)
    return hi, lo


def _ileave4(W4):
    """[NCT, 128, G, M] -> interleaved [128, NCT/2, G, 2M] walrus layout."""
    A = W4[0::2]  # [NJP, 128, G, M]
    B = W4[1::2]
    il = np.empty(A.shape[:3] + (2 * A.shape[3],), A.dtype)
    il[..., 0::2] = A[..., ::-1]
    il[..., 1::2] = B[..., ::-1]
    return np.ascontiguousarray(il.transpose(1, 0, 2, 3))


def make_in_maps(x, w_attn, b_attn, w_proj, b_proj, cfg=CFG):
    T, C, H, HP, NCT = cfg.T, cfg.C, cfg.H, cfg.HP, cfg.NCT
    xT = np.ascontiguousarray(x.reshape(T, C).T).astype(np.float32)  # [C,T]
    xh, xl = _hilo(xT)

    w16 = (np.asarray(w_attn, np.float32)) * SW
    wq16, wk16, wv16 = w16[:, 0:C], w16[:, C : 2 * C], w16[:, 2 * C :]

    def wil_pair(wsec):
        h, l = _hilo(wsec)
        W4h = h.reshape(NCT, 128, HP, 128)
        W4l = l.reshape(NCT, 128, HP, 128)
        return _ileave4(W4h), _ileave4(W4l)

    wqilh, wqill = wil_pair(wq16)
    wkilh, wkill = wil_pair(wk16)

    # V moving operand: per-head reversed d order (so the strided interleaved
    # SBUF write runs with a positive stride)
    wvr = np.ascontiguousarray(
        wv16.reshape(C, H, 64)[:, :, ::-1].reshape(C, C)
    )
    wvrh, wvrl = _hilo(wvr)

    # V stationary: x k-tile pairs interleaved per 128-key tile
    X4h = xh.astype(np.float32).reshape(NCT, 128, 32, 128)
    X4l = xl.astype(np.float32).reshape(NCT, 128, 32, 128)
    xilh = _ileave4(X4h.astype(NPE4))
    xill = _ileave4(X4l.astype(NPE4))

    wP = np.asarray(w_proj, np.float32).astype(NPBF16)
    bq = np.ascontiguousarray(np.asarray(b_attn[0:C], np.float32) * SW)
    bk = np.ascontiguousarray(np.asarray(b_attn[C : 2 * C], np.float32) * SW)
    # V bias folded into the output projection (exact)
    bP = np.ascontiguousarray(
        np.asarray(b_proj, np.float32)
        + np.asarray(b_attn[2 * C :], np.float32) @ np.asarray(w_proj, np.float32)
    )

    jl = np.arange(128)[:, None]
    ii = np.arange(128)[None, :]
    in_maps = []
    for c in range(cfg.ncores):
        colsh = np.concatenate(
            [xh[:, 128 * t : 128 * (t + 1)] for t in cfg.qtiles(c)], axis=1
        )
        colsl = np.concatenate(
            [xl[:, 128 * t : 128 * (t + 1)] for t in cfg.qtiles(c)], axis=1
        )
        # multiplicative {0,1} masks on the fp8 P slabs, per key-block residue
        masks = np.stack(
            [(jl - ii <= 128 * (c - r)) for r in range(cfg.ncores)]
        ).astype(np.float32)
        maskq = np.ascontiguousarray(
            masks.transpose(1, 0, 2).reshape(128, cfg.ncores * 128)
        ).astype(NPBF16)
        in_maps.append(
            {
                "xh": xh,
                "xl": xl,
                "xilh": xilh,
                "xill": xill,
                "xqh": np.ascontiguousarray(colsh),
                "xql": np.ascontiguousarray(colsl),
                "wqilh": wqilh,
                "wqill": wqill,
                "wkilh": wkilh,
                "wkill": wkill,
                "wvrh": wvrh,
                "wvrl": wvrl,
                "wP": wP,
                "bq": bq,
                "bk": bk,
                "bP": bP,
                "maskq": maskq,
            }
        )
    return in_maps


def declare_io(nc, cfg=CFG):
    C, T, HP, NJP, QW = cfg.C, cfg.T, cfg.HP, cfg.NJP, cfg.QW
    dt = nc.dram_tensor
    ins = {
        "xh": dt("xh", [C, T], E4, kind="ExternalInput").ap(),
        "xl": dt("xl", [C, T], E4, kind="ExternalInput").ap(),
        "xilh": dt("xilh", [128, NJP, 32, 256], E4, kind="ExternalInput").ap(),
        "xill": dt("xill", [128, NJP, 32, 256], E4, kind="ExternalInput").ap(),
        "xqh": dt("xqh", [C, QW], E4, kind="ExternalInput").ap(),
        "xql": dt("xql", [C, QW], E4, kind="ExternalInput").ap(),
        "wqilh": dt("wqilh", [128, NJP, HP, 256], E4, kind="ExternalInput").ap(),
        "wqill": dt("wqill", [128, NJP, HP, 256], E4, kind="ExternalInput").ap(),
        "wkilh": dt("wkilh", [128, NJP, HP, 256], E4, kind="ExternalInput").ap(),
        "wkill": dt("wkill", [128, NJP, HP, 256], E4, kind="ExternalInput").ap(),
        "wvrh": dt("wvrh", [C, C], E4, kind="ExternalInput").ap(),
        "wvrl": dt("wvrl", [C, C], E4, kind="ExternalInput").ap(),
        "wP": dt("wP", [C, C], BF16, kind="ExternalInput").ap(),
        "bq": dt("bq", [C], F32, kind="ExternalInput").ap(),
        "bk": dt("bk", [C], F32, kind="ExternalInput").ap(),
        "bP": dt("bP", [C], F32, kind="ExternalInput").ap(),
        "maskq": dt("maskq", [128, cfg.ncores * 128], BF16,
                    kind="ExternalInput").ap(),
    }
    outs = {
        "y": dt("y", [QW, C], F32, kind="ExternalOutput").ap()
    }
    return ins, outs


def build_program(cfg=CFG, repeat=1):
    nc = bacc.Bacc("TRN2", target_bir_lowering=False, debug=False,
                   num_devices=cfg.ncores)
    ins, outs = declare_io(nc, cfg)
    with tile.TileContext(nc) as tc:
        for _ in range(repeat):
            build_kernel_v3(tc, outs, ins, cfg)
    nc.compile()
    return nc


def assemble_output(results, cfg=CFG):
    y = np.empty((cfg.T, cfg.C), np.float32)
    for c in range(cfg.ncores):
        yc = results[c]["y"]
        for g, t in enumerate(cfg.qtiles(c)):
            y[128 * t : 128 * (t + 1)] = yc[128 * g : 128 * (g + 1)]
    return y.reshape(1, cfg.T, cfg.C)


_PROGRAM = None


def kernel(x, w_attn, b_attn, w_proj, b_proj):
    global _PROGRAM
    cfg = CFG
    x = np.asarray(x, np.float32)
    if _PROGRAM is None:
        _PROGRAM = build_program(cfg)
    in_maps = make_in_maps(
        x, np.asarray(w_attn), np.asarray(b_attn), np.asarray(w_proj),
        np.asarray(b_proj), cfg
    )
    res = run_bass_kernel_spmd(_PROGRAM, in_maps, core_ids=list(range(cfg.ncores)))
    return assemble_output(res.results, cfg)


if __name__ == "__main__":
    import reference

    inputs = {k: np.asarray(v) for k, v in reference.setup_inputs().items()}
    out = kernel(**inputs)
    print("kernel output", out.shape, out.dtype)


# revision 84
# speedup vs baseline: 1.2317x; 1.0026x over previous
"""Causal self-attention (B=1, T=4096, C=768, H=12) on 8 TRN2 NeuronCores.

Strategy (single SPMD NEFF, no collectives):
  - Sequence-parallel over queries: core c owns q-tiles {c, c+8, c+16, c+24}
    (128 rows each, descending-extent column order). Slot s of every core
    processes key-blocks 8s..8s+7 (uniform instruction stream across cores);
    the true causal boundary is enforced by a per-core binary mask library
    passed as input data, so ONE program serves all 8 cores.
  - K/V/Q projections run as error-compensated fp8 DoubleRowSwInterleave
    matmuls: host splits x and 16*w_attn into e4m3 (hi, lo) pairs and the
    kernel computes xh*wh + xh*wl + xl*wh (the lo*lo term is negligible).
    Each DRI matmul contracts TWO 128-row k-tiles per pass at 0.5 cyc/row,
    so the 9-matmul group costs 0.75x the bf16 equivalent with bf16-class
    accuracy (measured end-to-end rel err 3.4e-3 for the projections).
  - Attention scores stay transposed: S^T = K @ Q^T with keys on partitions;
    exp runs PSUM->SBUF on ScalarE with scale 1/2048 (the 16x weight
    prescale squares into S) and bias -2 so exp output fits fp8e4 range.
  - P^T is written as fp8e4; PV uses DRI pairing two CONSECUTIVE KEY BLOCKS
    per pass (keys are the contraction dim), with V stored as interleaved
    fp8 (hi, lo) stationaries: y = P*vh + P*vl keeps v at bf16-class
    precision while PV runs at 2x bf16 speed. The V bias is folded into an
    effective output-projection bias on the host (exact).
  - K/V live in small rolling per-wave buffers (each wave's blocks are only
    read by that wave's attention). The softmax denominator falls out of a
    65th all-ones column of the padded-to-128 interleaved V stationary.
  - Per (head, wave): sweep 1 computes QK + exp + mask for all four block
    pairs (pt tiles buffered), sweep 2 fires the eight PV matmuls back to
    back so the PE never waits on a freshly produced mask; masks run 1/4 on
    DVE and 3/4 on GPSIMD to balance the elementwise queues.
  - Measured end-to-end relative error vs the fp32 reference: 1.2e-2
    (matching a numpy emulation of the same quantization points).
"""

import contextlib
from dataclasses import dataclass

import ml_dtypes
import numpy as np

import concourse.bass as bass
import concourse.mybir as mybir
import concourse.tile as tile
from concourse import bacc
from concourse.bass_utils import run_bass_kernel_spmd

BF16 = mybir.dt.bfloat16
F32 = mybir.dt.float32
E4 = mybir.dt.float8e4
NPBF16 = ml_dtypes.bfloat16
NPE4 = ml_dtypes.float8_e4m3
DRI = mybir.MatmulPerfMode.DoubleRowSwInterleave

SW = 16.0  # weight prescale (power of two: commutes with rounding)
EXP_SCALE = 1.0 / (8.0 * SW * SW)  # 1/(sqrt(D) * SW^2)
EXP_BIAS = -2.0  # keeps exp output within fp8e4 range; cancels in softmax


@dataclass(frozen=True)
class Cfg:
    T: int = 4096
    H: int = 12
    D: int = 64
    ncores: int = 8

    @property
    def C(self):
        return self.H * self.D

    @property
    def HP(self):  # head pairs
        return self.H // 2

    @property
    def NKB(self):  # 128-row key blocks
        return self.T // 128

    @property
    def NCH(self):  # 512-row key chunks
        return self.T // 512

    @property
    def QTC(self):  # q-tiles per core
        return self.T // 128 // self.ncores

    @property
    def QW(self):  # q columns per core
        return 128 * self.QTC

    @property
    def NCT(self):  # 128-row contraction tiles over C
        return self.C // 128

    @property
    def NJP(self):  # contraction k-tile pairs
        return self.NCT // 2

    def nb(self, b):  # valid q-column prefix width for key-block b
        return 128 * (self.QTC - b // self.ncores)

    def qtiles(self, c):  # global q-tile indices for core c, descending extent
        return [c + self.ncores * (self.QTC - 1 - g) for g in range(self.QTC)]


CFG = Cfg()


def build_kernel_v3(tc, outs, ins, cfg=CFG):
    nc = tc.nc
    C, H, HP, NJP = cfg.C, cfg.H, cfg.HP, cfg.NJP
    QW, NCH = cfg.QW, cfg.NCH
    Exp = mybir.ActivationFunctionType.Exp
    Ident = mybir.ActivationFunctionType.Identity

    xh, xl = ins["xh"], ins["xl"]
    xilh, xill = ins["xilh"], ins["xill"]
    xqh, xql = ins["xqh"], ins["xql"]
    wkilh, wkill = ins["wkilh"], ins["wkill"]
    wqilh, wqill = ins["wqilh"], ins["wqill"]
    wvrh, wvrl = ins["wvrh"], ins["wvrl"]
    wP = ins["wP"]
    bq_in, bk_in, bP_in = ins["bq"], ins["bk"], ins["bP"]
    maskq = ins["maskq"]
    y = outs["y"]

    stack = contextlib.ExitStack()
    with stack:
        persist = stack.enter_context(tc.tile_pool(name="persist", bufs=1))

        # rolling per-wave K^T (bf16, scaled 16x) and interleaved V (fp8 hi/lo)
        kt_roll = persist.tile([128, 2, HP, 1024], BF16, name="kt_roll")
        vh_roll = persist.tile([128, 2, 4, H, 256], E4, name="vh_roll")
        vl_roll = persist.tile([128, 2, 4, H, 256], E4, name="vl_roll")
        qt_t = persist.tile([128, HP, QW], BF16, name="qt_t")
        ytf = persist.tile([128, HP, QW], BF16, name="ytf")
        yacc = persist.tile([128, H, QW], F32, name="yacc")  # rows 0:65 used
        mask_sb = persist.tile([128, cfg.ncores * 128], BF16, name="mask_sb")
        wp_sb = persist.tile([128, cfg.NCT, C], BF16, name="wp_sb")
        wkh_sb = persist.tile([128, NJP, HP, 256], E4, name="wkh_sb")
        wkl_sb = persist.tile([128, NJP, HP, 256], E4, name="wkl_sb")
        wvh_sb = persist.tile([128, cfg.NCT, C], E4, name="wvh_sb")
        wvl_sb = persist.tile([128, cfg.NCT, C], E4, name="wvl_sb")
        bq_sb = persist.tile([128, HP], F32, name="bq_sb")
        bk_sb = persist.tile([128, HP], F32, name="bk_sb")
        bp_bc = persist.tile([128, C], F32, name="bp_bc")
        ones11 = persist.tile([1, 64], F32, name="ones11")
        ebias = persist.tile([128, 1], F32, name="ebias")

        nc.vector.memset(ebias, EXP_BIAS)
        nc.vector.memset(ones11, 1.0 / SW)
        # touch Exp early so the ACT table set loads during startup DMAs
        nc.scalar.activation(ones11, ones11, Exp, scale=0.0)
        nc.vector.memset(ones11, 1.0 / SW)
        # V stationaries: zero the pad region once (gpsimd memset); set the
        # ones column (logical col 64 of 128 -> interleaved positions 126-127).
        vh4 = vh_roll.rearrange("p w q h (t two) -> p w q h t two", two=2)
        vl4 = vl_roll.rearrange("p w q h (t two) -> p w q h t two", two=2)
        nc.gpsimd.memset(vh4[:, :, :, :, 0:63, :], 0.0)
        nc.gpsimd.memset(vl4[:, :, :, :, 0:64, :], 0.0)
        nc.vector.memset(vh4[:, :, :, :, 63:64, :], 1.0)

        with (
            tc.tile_pool(name="xpool", bufs=3) as xpool,
            tc.tile_pool(name="pkv", bufs=2, space="PSUM") as pkv,
            tc.tile_pool(name="pvp", bufs=2, space="PSUM") as pvp,
            tc.tile_pool(name="nrm", bufs=2) as nrm,
        ):
            ptp = tc.alloc_tile_pool(name="ptpA", bufs=6)
            qproj = tc.alloc_tile_pool(name="qproj", bufs=1)
            aps = tc.alloc_tile_pool(name="apsA", bufs=2, space="PSUM")

            xhr = xh.rearrange("(j p) t -> p j t", p=128)
            xlr = xl.rearrange("(j p) t -> p j t", p=128)

            def load_xch(ch, split=False):
                th = xpool.tile([128, cfg.NCT, 512], E4, name="xch_h", tag="xh")
                tl = xpool.tile([128, cfg.NCT, 512], E4, name="xch_l", tag="xl")
                tih = xpool.tile([128, NJP, 4, 256], E4, name="xil_h", tag="xih")
                til = xpool.tile([128, NJP, 4, 256], E4, name="xil_l", tag="xil")
                cs = 512 * ch
                if split:
                    # ct-pair pieces: the first projection group consumes
                    # pair j as soon as piece j lands
                    for j in range(NJP):
                        nc.sync.dma_start(
                            out=th[:, 2 * j : 2 * j + 2, :],
                            in_=xhr[:, 2 * j : 2 * j + 2, cs : cs + 512],
                        )
                        nc.sync.dma_start(
                            out=tl[:, 2 * j : 2 * j + 2, :],
                            in_=xlr[:, 2 * j : 2 * j + 2, cs : cs + 512],
                        )
                else:
                    nc.sync.dma_start(out=th, in_=xhr[:, :, cs : cs + 512])
                    nc.sync.dma_start(out=tl, in_=xlr[:, :, cs : cs + 512])
                nc.sync.dma_start(out=tih, in_=xilh[:, :, 4 * ch : 4 * ch + 4, :])
                nc.sync.dma_start(out=til, in_=xill[:, :, 4 * ch : 4 * ch + 4, :])
                return th, tl, tih, til

            # startup DMA order: first x chunk, K weights, V weights, masks,
            # Q inputs - so the PE never waits on a cold queue
            nc.sync.dma_start(out=wkh_sb, in_=wkilh)
            nc.sync.dma_start(out=wkl_sb, in_=wkill)
            # biases ride the idle ACT DMA queue so their issue+descgen
            # never sits ahead of the critical startup loads on SP
            nc.scalar.dma_start(out=bq_sb,
                                in_=bq_in.rearrange("(hp p) -> p hp", p=128))
            nc.scalar.dma_start(out=bk_sb,
                                in_=bk_in.rearrange("(hp p) -> p hp", p=128))
            xch_pre = {0: load_xch(0, split=True)}
            nc.sync.dma_start(
                out=wvh_sb, in_=wvrh.rearrange("(j p) t -> p j t", p=128)
            )
            nc.sync.dma_start(
                out=wvl_sb, in_=wvrl.rearrange("(j p) t -> p j t", p=128)
            )
            xch_pre[1] = load_xch(1)
            nc.sync.dma_start(out=mask_sb, in_=maskq)
            wqh_sb = qproj.tile([128, NJP, HP, 256], E4, name="wqh_sb")
            wql_sb = qproj.tile([128, NJP, HP, 256], E4, name="wql_sb")
            xqh_sb = qproj.tile([128, cfg.NCT, QW], E4, name="xqh_sb")
            xql_sb = qproj.tile([128, cfg.NCT, QW], E4, name="xql_sb")
            nc.sync.dma_start(out=wqh_sb, in_=wqilh)
            nc.sync.dma_start(out=wql_sb, in_=wqill)
            nc.sync.dma_start(out=xqh_sb, in_=xqh.rearrange("(j p) t -> p j t", p=128))
            nc.sync.dma_start(out=xql_sb, in_=xql.rearrange("(j p) t -> p j t", p=128))

            def comp_dri(ps, wil_h, wil_l, xp_h, xp_l, n0=None, n1=None):
                """9-term compensated DRI group into `ps`.

                wil_*: callables j -> stationary AP [128, 2*M interleaved]
                xp_*: callables j -> moving AP [128, 2, N]
                """
                terms = [(wil_h, xp_h), (wil_l, xp_h), (wil_h, xp_l)]
                nmm = 0
                for wf, xf in terms:
                    for j in range(NJP):
                        nc.tensor.matmul(
                            ps,
                            wf(j).rearrange("p (m two) -> p m two", two=2),
                            xf(j),
                            start=(nmm == 0),
                            stop=(nmm == 3 * NJP - 1),
                            perf_mode=DRI,
                        )
                        nmm += 1

            for cp in range(NCH // 2):
                par = cp % 2
                first, last = cp == 0, cp == NCH // 2 - 1
                chunks = (2 * cp, 2 * cp + 1)
                if cp == 2:
                    # small-n waves: swap the 2x2-bank score pool for a
                    # 4x1-bank pool (a pair fits one bank), doubling the
                    # QK->exp ping-pong depth; likewise swap the P pool to
                    # ten half-size tiles (a small pair is <= 512 B)
                    aps.release()
                    aps = tc.alloc_tile_pool(name="apsB", bufs=4,
                                             space="PSUM")
                    ptp.release()
                    ptp = tc.alloc_tile_pool(name="ptpB", bufs=14)
                smallw = cfg.nb(4 * chunks[0]) <= 256
                # ---- project K^T / V for this wave's two chunks ------------
                for half, ch in enumerate(chunks):
                    th, tl, tih, til = (
                        xch_pre.pop(ch) if ch in xch_pre else load_xch(ch)
                    )
                    sched = [("k", hp) for hp in range(HP)] + [
                        ("v", (tt, nn)) for tt in range(4) for nn in range(2)
                    ]
                    for kind, item in sched:
                      if kind == "k":
                        hp = item
                        ps_k = pkv.tile([128, 512], F32, name="ps_k", tag="pkv")
                        comp_dri(
                            ps_k,
                            lambda j, hp=hp: wkh_sb[:, j, hp, :],
                            lambda j, hp=hp: wkl_sb[:, j, hp, :],
                            lambda j: th[:, 2 * j : 2 * j + 2, :],
                            lambda j: tl[:, 2 * j : 2 * j + 2, :],
                        )
                        nc.vector.tensor_scalar_add(
                            kt_roll[:, par, hp, 512 * half : 512 * (half + 1)],
                            ps_k,
                            bk_sb[:, hp : hp + 1],
                        )
                      else:
                        tt, nn = item
                        pi = 2 * half + tt // 2  # pair index in wave
                        pb = tt % 2  # block within pair
                        for n0, n1 in (((0, 384),) if nn == 0 else ((384, 768),)):
                            h0, h1 = n0 // 64, n1 // 64
                            ps_v = pkv.tile([128, 384], F32, name="ps_v", tag="pkv")
                            nmm = 0
                            for xf, wf in (
                                (tih, wvh_sb),
                                (tih, wvl_sb),
                                (til, wvh_sb),
                            ):
                                for j in range(NJP):
                                    nc.tensor.matmul(
                                        ps_v,
                                        xf[:, j, tt, :].rearrange(
                                            "p (m two) -> p m two", two=2
                                        ),
                                        wf[:, 2 * j : 2 * j + 2, n0:n1],
                                        start=(nmm == 0),
                                        stop=(nmm == 3 * NJP - 1),
                                        perf_mode=DRI,
                                    )
                                    nmm += 1
                            # v_hi = e4m3(v); v_lo = v - v_hi (bias folded into
                            # the output projection host-side)
                            psr = ps_v.rearrange("p (h e) -> p h e", e=64)
                            vh4w = vh_roll.rearrange(
                                "p w q h (t two) -> p w q h t two", two=2
                            )[:, par, pi, h0:h1, 64:128, pb]
                            vl4w = vl_roll.rearrange(
                                "p w q h (t two) -> p w q h t two", two=2
                            )[:, par, pi, h0:h1, 64:128, pb]
                            nc.vector.tensor_copy(vh4w, psr)
                            nc.vector.tensor_sub(vl4w, psr, vh4w)

                if cp == min(1, NCH // 2 - 1):
                    # prefetch output-projection weights mid-loop
                    for ct in range(cfg.NCT):
                        nc.sync.dma_start(
                            out=wp_sb[:, ct, :],
                            in_=wP[128 * ct : 128 * (ct + 1), :],
                        )
                    bp_src = bass.AP(
                        tensor=bP_in.tensor, offset=bP_in.offset, ap=[[0, 128], [1, C]]
                    )
                    nc.gpsimd.dma_start(out=bp_bc, in_=bp_src)
                if cp == 0:
                    # Q^T projection - emitted here so the PE chews K/V
                    # projection first while the Q inputs stream in
                    for hp in range(HP):
                        ps_q = pvp.tile([128, QW], F32, name="ps_q", tag="ps_y")
                        comp_dri(
                            ps_q,
                            lambda j, hp=hp: wqh_sb[:, j, hp, :],
                            lambda j, hp=hp: wql_sb[:, j, hp, :],
                            lambda j: xqh_sb[:, 2 * j : 2 * j + 2, :],
                            lambda j: xql_sb[:, 2 * j : 2 * j + 2, :],
                        )
                        nc.scalar.activation(
                            qt_t[:, hp, :], ps_q, Ident, bias=bq_sb[:, hp : hp + 1]
                        )
                    qproj.release()

                # ---- attention for this wave's 8 key-blocks ----------------
                nA = cfg.nb(4 * chunks[0])  # widths per half-wave
                for hp in range(HP):
                    for h in range(2):
                        hd = 2 * hp + h
                        ps_y = pvp.tile([128, 512], F32, name="ps_y", tag="ps_y")
                        # sweep 1: QK + exp + mask for all four pairs (pt
                        # tiles held); sweep 2: all eight PV matmuls back to
                        # back - PV never waits on a freshly computed mask
                        ptl = []
                        if True:
                          for pi in range(4):
                            half = pi // 2
                            ch = chunks[half]
                            pl = pi % 2  # pair within the half-wave
                            ba = 4 * ch + 2 * pl
                            n = cfg.nb(ba)
                            pt = ptp.tile([128, 1024 if not smallw else 512],
                                          E4, name=f"pt{h}", tag=f"pt{h}")
                            # big waves: blocks at offsets 0/512 in a 2-bank
                            # tile; small waves: contiguous at 0/n in 1 bank
                            sw = 1024 if not smallw else 512
                            off = 512 if not smallw else n
                            sps = aps.tile([128, sw], F32, name="sps",
                                           tag="sps")
                            blkv = pt[:, 0 : 2 * off].rearrange(
                                "p (b n) -> p b n", n=off
                            )[:, :, 0:n]
                            for pb in (0, 1):
                                bw = 4 * half + 2 * pl + pb  # kt_roll block
                                nc.tensor.matmul(
                                    sps[:, off * pb : off * pb + n],
                                    kt_roll[64 * h : 64 * (h + 1), par, hp,
                                            128 * bw : 128 * (bw + 1)],
                                    qt_t[64 * h : 64 * (h + 1), hp, 0:n],
                                    start=True,
                                    stop=True,
                                )
                            nc.scalar.activation(
                                blkv,
                                sps[:, 0 : 2 * off].rearrange(
                                    "p (b n) -> p b n", n=off
                                )[:, :, 0:n],
                                Exp, scale=EXP_SCALE, bias=ebias,
                            )
                            # causal boundary: mask last 128 q-cols of each blk
                            r0 = ba % cfg.ncores
                            pts = blkv[:, :, n - 128 : n]
                            msk = mask_sb[:, 128 * r0 : 128 * (r0 + 2)].rearrange(
                                "p (b n) -> p b n", n=128
                            )
                            meng = nc.vector if pi == 0 else nc.gpsimd
                            meng.tensor_mul(pts, pts, msk)
                            ptl.append((pi, n, blkv))
                        for pi, n, blkv in ptl:
                            # PV: two DRI matmuls (v_hi, v_lo), contraction
                            # over both blocks of the pair
                            for vroll in (vh_roll, vl_roll):
                                nc.tensor.matmul(
                                    ps_y[:, 0:n],
                                    vroll[:, par, pi, hd, :].rearrange(
                                        "p (m two) -> p m two", two=2
                                    ),
                                    blkv,
                                    start=(pi == 0 and vroll is vh_roll),
                                    stop=(pi == 3 and vroll is vl_roll),
                                    perf_mode=DRI,
                                    skip_group_check=True,
                                )
                        if first:
                            nc.vector.tensor_copy(
                                yacc[0:65, hd, 0:nA], ps_y[0:65, 0:nA]
                            )
                        else:
                            nc.vector.tensor_add(
                                yacc[0:65, hd, 0:nA],
                                yacc[0:65, hd, 0:nA],
                                ps_y[0:65, 0:nA],
                            )
                        if last:
                            # normalize this head now - overlaps the
                            # remaining heads' attention
                            rec = nrm.tile([1, QW], F32, name="rec", tag="rec")
                            rc_ps = pkv.tile([64, QW], F32, name="rc_ps",
                                             tag="pkv")
                            nc.vector.reciprocal(rec, yacc[64:65, hd, :])
                            nc.tensor.matmul(
                                rc_ps, ones11[0:1, :], rec, start=True, stop=True
                            )
                            nc.vector.tensor_mul(
                                ytf[64 * h : 64 * (h + 1), hp, :],
                                yacc[0:64, hd, :], rc_ps,
                            )

            aps.release()
            ptp.release()

        # ---- output projection -------------------------------------------
        with (
            tc.tile_pool(name="ops", bufs=3, space="PSUM") as ops,
            tc.tile_pool(name="osb", bufs=3) as osb,
        ):
            for g in range(cfg.QTC):
                ps_o = ops.tile([128, C], F32, name="ps_o", tag="ps_o")
                for n0, n1 in ((0, 512), (512, C)):
                    for hp in range(HP):
                        nc.tensor.matmul(
                            ps_o[:, n0:n1],
                            ytf[:, hp, 128 * g : 128 * (g + 1)],
                            wp_sb[:, hp, n0:n1],
                            start=(hp == 0),
                            stop=(hp == HP - 1),
                        )
                yo = osb.tile([128, C], F32, name="yo", tag="yo")
                nc.vector.tensor_add(yo, ps_o, bp_bc)
                nc.sync.dma_start(out=y[128 * g : 128 * (g + 1), :], in_=yo)


# ---------------------------------------------------------------------------
# host side
# ---------------------------------------------------------------------------


def _hilo(a):
    hi = np.asarray(a, NPE4)
    lo = np.asarray(a - hi.astype(np.float32), NPE4)
    return hi, lo


def _ileave4(W4):
    """[NCT, 128, G, M] -> interleaved [128, NCT/2, G, 2M] walrus layout."""
    A = W4[0::2]  # [NJP, 128, G, M]
    B = W4[1::2]
    il = np.empty(A.shape[:3] + (2 * A.shape[3],), A.dtype)
    il[..., 0::2] = A[..., ::-1]
    il[..., 1::2] = B[..., ::-1]
    return np.ascontiguousarray(il.transpose(1, 0, 2, 3))


def make_in_maps(x, w_attn, b_attn, w_proj, b_proj, cfg=CFG):
    T, C, H, HP, NCT = cfg.T, cfg.C, cfg.H, cfg.HP, cfg.NCT
    xT = np.ascontiguousarray(x.reshape(T, C).T).astype(np.float32)  # [C,T]
    xh, xl = _hilo(xT)

    w16 = (np.asarray(w_attn, np.float32)) * SW
    wq16, wk16, wv16 = w16[:, 0:C], w16[:, C : 2 * C], w16[:, 2 * C :]

    def wil_pair(wsec):
        h, l = _hilo(wsec)
        W4h = h.reshape(NCT, 128, HP, 128)
        W4l = l.reshape(NCT, 128, HP, 128)
        return _ileave4(W4h), _ileave4(W4l)

    wqilh, wqill = wil_pair(wq16)
    wkilh, wkill = wil_pair(wk16)

    # V moving operand: per-head reversed d order (so the strided interleaved
    # SBUF write runs with a positive stride)
    wvr = np.ascontiguousarray(
        wv16.reshape(C, H, 64)[:, :, ::-1].reshape(C, C)
    )
    wvrh, wvrl = _hilo(wvr)

    # V stationary: x k-tile pairs interleaved per 128-key tile
    X4h = xh.astype(np.float32).reshape(NCT, 128, 32, 128)
    X4l = xl.astype(np.float32).reshape(NCT, 128, 32, 128)
    xilh = _ileave4(X4h.astype(NPE4))
    xill = _ileave4(X4l.astype(NPE4))

    wP = np.asarray(w_proj, np.float32).astype(NPBF16)
    bq = np.ascontiguousarray(np.asarray(b_attn[0:C], np.float32) * SW)
    bk = np.ascontiguousarray(np.asarray(b_attn[C : 2 * C], np.float32) * SW)
    # V bias folded into the output projection (exact)
    bP = np.ascontiguousarray(
        np.asarray(b_proj, np.float32)
        + np.asarray(b_attn[2 * C :], np.float32) @ np.asarray(w_proj, np.float32)
    )

    jl = np.arange(128)[:, None]
    ii = np.arange(128)[None, :]
    in_maps = []
    for c in range(cfg.ncores):
        colsh = np.concatenate(
            [xh[:, 128 * t : 128 * (t + 1)] for t in cfg.qtiles(c)], axis=1
        )
        colsl = np.concatenate(
            [xl[:, 128 * t : 128 * (t + 1)] for t in cfg.qtiles(c)], axis=1
        )
        # multiplicative {0,1} masks on the fp8 P slabs, per key-block residue
        masks = np.stack(
            [(jl - ii <= 128 * (c - r)) for r in range(cfg.ncores)]
        ).astype(np.float32)
        maskq = np.ascontiguousarray(
            masks.transpose(1, 0, 2).reshape(128, cfg.ncores * 128)
        ).astype(NPBF16)
        in_maps.append(
            {
                "xh": xh,
                "xl": xl,
                "xilh": xilh,
                "xill": xill,
                "xqh": np.ascontiguousarray(colsh),
                "xql": np.ascontiguousarray(colsl),
                "wqilh": wqilh,
                "wqill": wqill,
                "wkilh": wkilh,
                "wkill": wkill,
                "wvrh": wvrh,
                "wvrl": wvrl,
                "wP": wP,
                "bq": bq,
                "bk": bk,
                "bP": bP,
                "maskq": maskq,
            }
        )
    return in_maps


def declare_io(nc, cfg=CFG):
    C, T, HP, NJP, QW = cfg.C, cfg.T, cfg.HP, cfg.NJP, cfg.QW
    dt = nc.dram_tensor
    ins = {
        "xh": dt("xh", [C, T], E4, kind="ExternalInput").ap(),
        "xl": dt("xl", [C, T], E4, kind="ExternalInput").ap(),
        "xilh": dt("xilh", [128, NJP, 32, 256], E4, kind="ExternalInput").ap(),
        "xill": dt("xill", [128, NJP, 32, 256], E4, kind="ExternalInput").ap(),
        "xqh": dt("xqh", [C, QW], E4, kind="ExternalInput").ap(),
        "xql": dt("xql", [C, QW], E4, kind="ExternalInput").ap(),
        "wqilh": dt("wqilh", [128, NJP, HP, 256], E4, kind="ExternalInput").ap(),
        "wqill": dt("wqill", [128, NJP, HP, 256], E4, kind="ExternalInput").ap(),
        "wkilh": dt("wkilh", [128, NJP, HP, 256], E4, kind="ExternalInput").ap(),
        "wkill": dt("wkill", [128, NJP, HP, 256], E4, kind="ExternalInput").ap(),
        "wvrh": dt("wvrh", [C, C], E4, kind="ExternalInput").ap(),
        "wvrl": dt("wvrl", [C, C], E4, kind="ExternalInput").ap(),
        "wP": dt("wP", [C, C], BF16, kind="ExternalInput").ap(),
        "bq": dt("bq", [C], F32, kind="ExternalInput").ap(),
        "bk": dt("bk", [C], F32, kind="ExternalInput").ap(),
        "bP": dt("bP", [C], F32, kind="ExternalInput").ap(),
        "maskq": dt("maskq", [128, cfg.ncores * 128], BF16,
                    kind="ExternalInput").ap(),
    }
    outs = {
        "y": dt("y", [QW, C], F32, kind="ExternalOutput").ap()
    }
    return ins, outs


def build_program(cfg=CFG, repeat=1):
    nc = bacc.Bacc("TRN2", target_bir_lowering=False, debug=False,
                   num_devices=cfg.ncores)
    ins, outs = declare_io(nc, cfg)
    with tile.TileContext(nc) as tc:
        for _ in range(repeat):
            build_kernel_v3(tc, outs, ins, cfg)
    nc.compile()
    return nc


def assemble_output(results, cfg=CFG):
    y = np.empty((cfg.T, cfg.C), np.float32)
    for c in range(cfg.ncores):
        yc = results[c]["y"]
        for g, t in enumerate(cfg.qtiles(c)):
            y[128 * t : 128 * (t + 1)] = yc[128 * g : 128 * (g + 1)]
    return y.reshape(1, cfg.T, cfg.C)


_PROGRAM = None


def kernel(x, w_attn, b_attn, w_proj, b_proj):
    global _PROGRAM
    cfg = CFG
    x = np.asarray(x, np.float32)
    if _PROGRAM is None:
        _PROGRAM = build_program(cfg)
    in_maps = make_in_maps(
        x, np.asarray(w_attn), np.asarray(b_attn), np.asarray(w_proj),
        np.asarray(b_proj), cfg
    )
    res = run_bass_kernel_spmd(_PROGRAM, in_maps, core_ids=list(range(cfg.ncores)))
    return assemble_output(res.results, cfg)


if __name__ == "__main__":
    import reference

    inputs = {k: np.asarray(v) for k, v in reference.setup_inputs().items()}
    out = kernel(**inputs)
    print("kernel output", out.shape, out.dtype)
